# revision 26
# baseline (speedup 1.0000x reference)
"""AttentiveFP forward on 8 Trainium2 NeuronCores (Bass/Tile).

Edges sharded by dst-owner core; per-core nodes sorted by in-degree with a
round-robin slot structure so segment softmax/sum become dense PSUM matmul
accumulation. Edge phase processed in large multi-block groups: one
dma_gather per ~34 rounds, wide DVE ops (parity select in place), DVE
leaky-relu (no ACT table ping-pong), single Exp per group. Layers 1-2 use
256B gather elements (64-f16 rows, att_src dot computed on the fly), halving
gather traffic and AllGather payload. fp16 node state feeds all PE matmuls.
"""
import numpy as np
from contextlib import ExitStack

import concourse.bass as bass
import concourse.tile as tile
from concourse import bacc, mybir
from concourse.bass_utils import run_bass_kernel_spmd
from concourse.masks import make_identity

F32 = mybir.dt.float32
F16 = mybir.dt.float16
I16 = mybir.dt.int16
AF = mybir.ActivationFunctionType
OP = mybir.AluOpType

NCORE = 8
N, E, B = 50000, 500000, 2048
H = 64
NS = 0.01
NPC = N // NCORE
GPC = B // NCORE
NBLK = (NPC + 127) // 128
NPAD = NBLK * 128
GBLK = (GPC + 127) // 128
GPAD = GBLK * 128
TROWS = NCORE * NPAD
MAXR = 10
CH = 512
GT = 34       # max rounds per edge-phase group


def _calls_for(R, maxr=MAXR):
    calls, base, bases = [], 0, []
    for r in R:
        bases.append(base)
        calls.append([(r0, min(r0 + maxr, int(r))) for r0 in range(0, int(r), maxr)])
        base += int(r)
    return calls, bases, base


def _wrap_into(gidx, arr, col0):
    n = arr.shape[0]
    blk = arr.reshape(n // 16, 16).T
    gidx[:16, col0:col0 + n // 16] = blk
    gidx[16:128, col0:col0 + n // 16] = np.tile(blk, (7, 1))


def build_plan(edge_index, batch):
    src = edge_index[0].astype(np.int64)
    dst = edge_index[1].astype(np.int64)
    owner = dst // NPC

    perms, degs_sorted, grp_starts, egrp = [], [], [], []
    sortpos = np.zeros(N, np.int64)
    for c in range(NCORE):
        n0 = c * NPC
        emask = np.nonzero(owner == c)[0]
        deg = np.bincount(dst[emask] - n0, minlength=NPC)
        order = np.argsort(-deg, kind="stable")
        perms.append(order)
        sortpos[n0 + order] = np.arange(NPC)
        dsorted = deg[order]
        degs_sorted.append(dsorted)
        eorder = np.argsort(sortpos[dst[emask]], kind="stable")
        egrp.append(emask[eorder])
        grp_starts.append(np.concatenate([[0], np.cumsum(dsorted)]))
    trow = (np.arange(N) // NPC) * NPAD + sortpos

    R = np.ones(NBLK, np.int64)
    for b in range(NBLK):
        for c in range(NCORE):
            d = degs_sorted[c][b * 128:(b + 1) * 128]
            if len(d):
                R[b] = max(R[b], int(d[0]))
    calls, bases, NCH = _calls_for(R)
    S = NCH * 128

    gsize = np.bincount(batch, minlength=B)
    gstart = np.concatenate([[0], np.cumsum(gsize)])
    gperms, gss = [], []
    for c in range(NCORE):
        gs = gsize[c * GPC:(c + 1) * GPC]
        gorder = np.argsort(-gs, kind="stable")
        gperms.append(gorder)
        gss.append(gs[gorder])
    RG = np.ones(GBLK, np.int64)
    for b in range(GBLK):
        for c in range(NCORE):
            d = gss[c][b * 128:(b + 1) * 128]
            if len(d):
                RG[b] = max(RG[b], int(d[0]))
    gcalls, gbases, GCH = _calls_for(RG, 8)
    SR = GCH * 128

    cores = []
    lanes = np.arange(128)
    for c in range(NCORE):
        gidx = np.zeros((128, S // 16), np.int16)
        par = np.zeros((128, NCH, 1), np.float16)
        msk = np.zeros((128, NCH, 1), np.float32)
        esel = np.full(NCH * 128, -1, np.int64)
        ds = degs_sorted[c]
        gst = grp_starts[c]
        eg = egrp[c]
        for b in range(NBLK):
            for (r0, r1) in calls[b]:
                ia = np.zeros((r1 - r0) * 128, np.int64)
                for r in range(r0, r1):
                    ch = bases[b] + r
                    p = b * 128 + lanes
                    pc = np.minimum(p, NPC - 1)
                    ok = (p < NPC) & (r < ds[pc])
                    eids = gst[pc] + r
                    ge = np.where(ok, eg[np.where(ok, np.minimum(eids, len(eg) - 1), 0)], -1)
                    esel[ch * 128 + lanes] = ge
                    rows = np.where(ok, trow[src[np.maximum(ge, 0)]], 0)
                    ia[(r - r0) * 128 + lanes] = rows >> 1
                    par[:, ch, 0] = (rows & 1).astype(np.float16)
                    msk[:, ch, 0] = ok.astype(np.float32)
                _wrap_into(gidx, ia, (bases[b] + r0) * 8)
        cores.append(dict(gidx=gidx, par=par, msk=msk, esel=esel))

    rcores = []
    for c in range(NCORE):
        rgidx = np.zeros((128, SR // 16), np.int16)
        rpar = np.zeros((128, GCH, 1), np.float16)
        rmsk = np.zeros((128, GCH, 1), np.float32)
        rmsk16 = np.zeros((128, GCH, 1), np.float16)
        gs = gss[c]
        gp = gperms[c]
        for b in range(GBLK):
            for (r0, r1) in gcalls[b]:
                ia = np.zeros((r1 - r0) * 128, np.int64)
                for r in range(r0, r1):
                    ch = gbases[b] + r
                    p = b * 128 + lanes
                    pc = np.minimum(p, GPC - 1)
                    ok = (p < GPC) & (r < gs[pc])
                    g = c * GPC + gp[pc]
                    node = np.where(ok, gstart[g] + r, 0)
                    rows = np.where(ok, trow[node], 0)
                    ia[(r - r0) * 128 + lanes] = rows >> 1
                    rpar[:, ch, 0] = (rows & 1).astype(np.float16)
                    rmsk[:, ch, 0] = ok.astype(np.float32)
                    rmsk16[:, ch, 0] = ok.astype(np.float16)
                _wrap_into(rgidx, ia, (gbases[b] + r0) * 8)
        rcores.append(dict(rgidx=rgidx, rpar=rpar, rmsk=rmsk, rmsk16=rmsk16))

    return dict(R=R, bases=bases, NCH=NCH, S=S,
                RG=RG, gbases=gbases, GCH=GCH, SR=SR,
                cores=cores, rcores=rcores, perms=perms, gperms=gperms)


def build_nc(plan):
    R, bases, NCH, S = plan["R"], plan["bases"], plan["NCH"], plan["S"]
    RG, gbases, GCH, SR = plan["RG"], plan["gbases"], plan["GCH"], plan["SR"]
    RGT = int(max(int(r) for r in RG))

    # pack whole blocks into groups of <= cap rounds
    def pack_groups(cap):
        gs, cur, cur_r = [], [], 0
        for b in range(NBLK):
            r = int(R[b])
            if cur and cur_r + r > cap:
                gs.append(cur)
                cur, cur_r = [], 0
            cur.append(b)
            cur_r += r
        gs.append(cur)
        return gs
    GT0 = max(20, int(max(R)))
    groups0 = pack_groups(GT0)
    groups12 = pack_groups(GT)

    nc = bacc.Bacc("TRN2", target_bir_lowering=False, debug=False,
                   num_devices=NCORE, num_swdge_queues=4)

    def din(name, shape, dt=F32):
        return nc.dram_tensor(name, shape, dt, kind="ExternalInput")

    xT_in = din("xT_in", [H, NPAD])
    gidx_in = din("gidx", [128, S // 16], I16)
    par_in = din("par", [128, NCH, 1], F16)
    msk_in = din("msk", [128, NCH, 1], F32)
    eaT_in = din("eaT", [16, S], F16)
    rgidx_in = din("rgidx", [128, SR // 16], I16)
    rpar_in = din("rpar", [128, GCH, 1], F16)
    rmsk_in = din("rmsk", [128, GCH, 1], F32)
    rmsk16_in = din("rmsk16", [128, GCH, 1], F16)
    lin1T = din("lin1T", [H, H]); lin1_b = din("lin1_b", [H, 1])
    w1aT = din("w1aT", [H, H], F16); w1bT = din("w1bT", [16, H], F16)
    attl_rep = din("attl_rep", [128, 1, H], F16)
    attr_rep = din("attr_rep", [128, H], F16)
    g2T = din("g2T", [H, H], F16); gate_b = din("gate_b", [H, 1])
    atomT = din("atomT", [H, 2, H], F16)
    asrc_rep = din("asrc_rep", [128, 2, H], F16)
    adst_rep = din("adst_rep", [128, 2, H], F16)
    atom_b = din("atom_b", [H, 2])
    molT = din("molT", [H, H], F16)
    mol_lin = din("mol_lin", [H, H])
    matt_src_rep = din("matt_src_rep", [128, 1, H], F16)
    matt_dst = din("matt_dst", [H, 1])
    mol_b = din("mol_b", [H, 1])
    gruW = din("gruW", [128, 4, 2, H], F16)   # [K, widx, gate(r/z), M]
    gruN = din("gruN", [128, 4, 2, H], F16)   # [K, widx, (nx/nh), M] zero-padded
    gbx_rz = din("gbx_rz", [64, 8])
    gbh_rz = din("gbh_rz", [64, 8])
    gbx_n = din("gbx_n", [H, 4])
    gbh_n = din("gbh_n", [H, 4])
    lin2T = din("lin2T", [H, H]); lin2_b = din("lin2_b", [H, 1])
    lng_rep = din("lng_rep", [128, H]); lnb_rep = din("lnb_rep", [128, H])
    h1T = din("h1T", [H, H]); h1_b = din("h1_b", [H, 1])
    h2T = din("h2T", [H, H]); h2_b = din("h2_b", [H, 1])
    h3T = din("h3T", [H, 1]); h3_b = din("h3_b", [1, 1])

    y_out = nc.dram_tensor("y_out", [1, GPAD], F32, kind="ExternalOutput")

    cin0 = nc.dram_tensor("cin0", [NPAD, 128], F16)
    cout0 = nc.dram_tensor("cout0", [TROWS // 2, 256], F16, addr_space="Shared")
    cins = [cin0]
    couts = [cout0]
    for l in (1, 2):
        cins.append(nc.dram_tensor(f"cin{l}", [NPAD, 64], F16))
        couts.append(nc.dram_tensor(f"cout{l}", [TROWS // 2, 128], F16,
                                    addr_space="Shared"))
    cinr = nc.dram_tensor("cinr", [NPAD, 128], F16)
    coutr = nc.dram_tensor("coutr", [TROWS // 2, 256], F16, addr_space="Shared")

    ctx = ExitStack()
    ctx2 = nc.allow_low_precision(reason="fp16 edge tables/messages/state")
    ctx2.__enter__()
    with tile.TileContext(nc) as tc:
        cpool = ctx.enter_context(tc.tile_pool(name="const", bufs=1))
        wpool = ctx.enter_context(tc.tile_pool(name="wts", bufs=1))
        big = ctx.enter_context(tc.tile_pool(name="big", bufs=1))
        stkp = ctx.enter_context(tc.tile_pool(name="stkp", bufs=2))
        xsp = ctx.enter_context(tc.tile_pool(name="xsp", bufs=2))
        g0p = ctx.enter_context(tc.tile_pool(name="g0p", bufs=3))
        g12p = ctx.enter_context(tc.tile_pool(name="g12p", bufs=3))
        grp_ = ctx.enter_context(tc.tile_pool(name="gathr", bufs=1))
        eap = ctx.enter_context(tc.tile_pool(name="eap", bufs=1))
        sp = ctx.enter_context(tc.tile_pool(name="scr", bufs=2))
        s1 = ctx.enter_context(tc.tile_pool(name="scr1", bufs=1))
        pp = ctx.enter_context(tc.tile_pool(name="ps", bufs=2, space="PSUM"))
        prp = ctx.enter_context(tc.tile_pool(name="psp", bufs=2, space="PSUM"))
        rp = ctx.enter_context(tc.tile_pool(name="psr", bufs=2, space="PSUM"))
        zp = ctx.enter_context(tc.tile_pool(name="psz", bufs=2, space="PSUM"))

        id32 = cpool.tile([128, 128], F32)
        make_identity(nc, id32[:])
        id16 = cpool.tile([128, 128], F16)
        nc.vector.tensor_copy(id16[:], id32[:])

        def load(t, shape, dt=F32):
            s = wpool.tile(shape, dt, tag=f"w_{t.name}")
            nc.sync.dma_start(s[:], t[:])
            return s

        gidx_s = load(gidx_in, [128, S // 16], I16)
        rgidx_s = load(rgidx_in, [128, SR // 16], I16)
        par_s = load(par_in, [128, NCH, 1], F16)
        msk_s = load(msk_in, [128, NCH, 1], F32)
        rpar_s = load(rpar_in, [128, GCH, 1], F16)
        rmsk_s = load(rmsk_in, [128, GCH, 1], F32)
        rmsk16_s = load(rmsk16_in, [128, GCH, 1], F16)
        lin1T_s = load(lin1T, [H, H]); lin1b_s = load(lin1_b, [H, 1])
        w1aT_s = load(w1aT, [H, H], F16); w1bT_s = load(w1bT, [16, H], F16)
        attl_s = load(attl_rep, [128, 1, H], F16)
        attr_s = load(attr_rep, [128, H], F16)
        g2T_s = load(g2T, [H, H], F16); gateb_s = load(gate_b, [H, 1])
        atomT_s = load(atomT, [H, 2, H], F16)
        asrc_s = load(asrc_rep, [128, 2, H], F16)
        adst_s = load(adst_rep, [128, 2, H], F16)
        atomb_s = load(atom_b, [H, 2])
        molT_s = load(molT, [H, H], F16); mol_lin_s = load(mol_lin, [H, H])
        msrc_s = load(matt_src_rep, [128, 1, H], F16)
        mdst_s = load(matt_dst, [H, 1])
        molb_s = load(mol_b, [H, 1])
        gruW_s = load(gruW, [128, 4, 2, H], F16)
        gruN_s = load(gruN, [128, 4, 2, H], F16)
        gbxrz_s = load(gbx_rz, [64, 8])
        gbhrz_s = load(gbh_rz, [64, 8])
        gbxn_s = load(gbx_n, [H, 4])
        gbhn_s = load(gbh_n, [H, 4])
        lin2T_s = load(lin2T, [H, H]); lin2b_s = load(lin2_b, [H, 1])
        lng_s = load(lng_rep, [128, H]); lnb_s = load(lnb_rep, [128, H])
        h1T_s = load(h1T, [H, H]); h1b_s = load(h1_b, [H, 1])
        h2T_s = load(h2T, [H, H]); h2b_s = load(h2_b, [H, 1])
        h3T_s = load(h3T, [H, 1]); h3b_s = load(h3_b, [1, 1])

        def fm_mm(out_ap, lhsT, rhs, ncols, func=None, bias=0.0):
            M = lhsT.shape[-1]
            for c0 in range(0, ncols, CH):
                w = min(CH, ncols - c0)
                ps = pp.tile([128, CH], F32, tag="mmq")
                nc.tensor.matmul(ps[0:M, :w], lhsT, rhs[:, c0:c0 + w],
                                 start=True, stop=True)
                f = func
                if f is None:
                    f = AF.Copy if isinstance(bias, float) else AF.Identity
                nc.scalar.activation(out_ap[:, c0:c0 + w], ps[0:M, :w], f, bias=bias)

        def gru_bias(widx):
            br = sp.tile([64, 1], F32, tag=f"brz{widx}")
            nc.vector.tensor_add(br[:], gbxrz_s[:, 2 * widx:2 * widx + 1], gbhrz_s[:, 2 * widx:2 * widx + 1])
            bz = sp.tile([64, 1], F32, tag=f"bz{widx}")
            nc.vector.tensor_add(bz[:], gbxrz_s[:, 2 * widx + 1:2 * widx + 2], gbhrz_s[:, 2 * widx + 1:2 * widx + 2])
            return br, bz

        def gru_chunk(stk, xprev, widx, out_ap, c0, w, br, bz):
            if True:
                prz = pp.tile([128, CH], F32, tag="mmq")
                nc.tensor.matmul(prz[0:64, :w], gruW_s[:, widx, 0], stk[:, c0:c0 + w], start=True, stop=True)
                prz2 = pp.tile([128, CH], F32, tag="mmq")
                nc.tensor.matmul(prz2[0:64, :w], gruW_s[:, widx, 1], stk[:, c0:c0 + w], start=True, stop=True)
                rz = sp.tile([64, CH], F32, tag="rz")
                nc.scalar.activation(rz[:, :w], prz[0:64, :w], AF.Sigmoid, bias=br[:])
                zz = sp.tile([64, CH], F32, tag="zz")
                nc.scalar.activation(zz[:, :w], prz2[0:64, :w], AF.Sigmoid, bias=bz[:])
                pn1 = pp.tile([128, CH], F32, tag="mmq")
                nc.tensor.matmul(pn1[0:64, :w], gruN_s[:, widx, 0], stk[:, c0:c0 + w], start=True, stop=True)
                pn2 = pp.tile([128, CH], F32, tag="mmq")
                nc.tensor.matmul(pn2[0:64, :w], gruN_s[:, widx, 1], stk[:, c0:c0 + w], start=True, stop=True)
                hnb = sp.tile([64, CH], F32, tag="hnb")
                nc.scalar.activation(hnb[:, :w], pn2[0:64, :w], AF.Identity, bias=gbhn_s[:, widx:widx + 1])
                nc.vector.tensor_mul(hnb[:, :w], rz[:, :w], hnb[:, :w])
                nc.vector.tensor_add(hnb[:, :w], hnb[:, :w], pn1[0:64, :w])
                nn = sp.tile([64, CH], F32, tag="nn")
                nc.scalar.activation(nn[:, :w], hnb[:, :w], AF.Tanh, bias=gbxn_s[:, widx:widx + 1])
                d = rz  # rz buffer is dead past the mul above
                nc.vector.tensor_sub(d[:, :w], xprev[:, c0:c0 + w], nn[:, :w])
                nc.vector.tensor_mul(d[:, :w], zz[:, :w], d[:, :w])
                nc.vector.tensor_add(d[:, :w], nn[:, :w], d[:, :w])
                nc.scalar.activation(out_ap[:, c0:c0 + w], d[:, :w], AF.Relu)

        def gru(stk, xprev, widx, out_ap, ncols):
            br, bz = gru_bias(widx)
            for c0 in range(0, ncols, CH):
                gru_chunk(stk, xprev, widx, out_ap, c0, min(CH, ncols - c0), br, bz)

        def elu_chunk(buf, bias_ap, c0, w, pre_lhsT=None):
            if pre_lhsT is not None:
                ps = pp.tile([128, CH], F32, tag="mmq")
                nc.tensor.matmul(ps[0:64, :w], pre_lhsT, buf[0:64, c0:c0 + w], start=True, stop=True)
                src = ps[0:64, :w]
            else:
                src = buf[0:64, c0:c0 + w]
            e1 = sp.tile([64, CH], F32, tag="rz")
            nc.scalar.activation(e1[:, :w], src, AF.Exp, bias=bias_ap)
            t1 = sp.tile([64, CH], F32, tag="zz")
            nc.scalar.activation(t1[:, :w], e1[:, :w], AF.Relu, bias=1.0, scale=-1.0)
            t2 = sp.tile([64, CH], F32, tag="hnb")
            nc.scalar.activation(t2[:, :w], src, AF.Relu, bias=bias_ap)
            nc.vector.tensor_sub(buf[0:64, c0:c0 + w], t2[:, :w], t1[:, :w])

        def elu_inplace(buf, bias_ap, ncols, pre_lhsT=None):
            """buf[0:64, :] = elu((pre_lhsT.T @ buf[0:64]) + bias)."""
            for c0 in range(0, ncols, CH):
                elu_chunk(buf, bias_ap, c0, min(CH, ncols - c0), pre_lhsT)

        qn_state = [0]

        def edge_phase(layer, table, dvals, hdst, window_cb=None):
            """hdst[0:64, :NPAD] (f16) <- normalized aggregation (feature-major)."""
            W = 256 if layer == 0 else 128
            half = W // 2
            pool = g0p if layer == 0 else g12p
            gtile = GT0 if layer == 0 else GT
            for blist in (groups0 if layer == 0 else groups12):
                ch0 = bases[blist[0]]
                Gr = sum(int(R[b]) for b in blist)
                g = pool.tile([128, gtile, W], F16, tag=f"g{min(layer, 1)}")
                nc.gpsimd.dma_gather(
                    g[:, 0:Gr], table[:], gidx_s[:, ch0 * 8:(ch0 + Gr) * 8],
                    Gr * 128, Gr * 128, W, elem_step=W,
                    single_packet=False, queue_num=qn_state[0] % 4)
                qn_state[0] += 1
                # parity select in place: row -> g[:, :, half:W]
                nc.vector.tensor_sub(g[:, 0:Gr, half:W], g[:, 0:Gr, half:W], g[:, 0:Gr, 0:half])
                nc.vector.tensor_mul(g[:, 0:Gr, half:W], g[:, 0:Gr, half:W],
                                     par_s[:, ch0:ch0 + Gr].to_broadcast([128, Gr, half]))
                nc.vector.tensor_add(g[:, 0:Gr, half:W], g[:, 0:Gr, half:W], g[:, 0:Gr, 0:half])
                lg = sp.tile([128, GT, 1], F32, tag="lg")
                ev = g[:, 0:Gr, 0:64]  # dead (g0) half reused as scratch
                if layer == 0:
                    for r0 in range(0, Gr, 8):
                        r1 = min(r0 + 8, Gr)
                        ea = eap.tile([16, 8 * 128], F16, tag="ea")
                        nc.sync.dma_start(ea[:, 0:(r1 - r0) * 128],
                                          eaT_in[:, (ch0 + r0) * 128:(ch0 + r1) * 128])
                        pz = zp.tile([128, 8, 64], F32, tag="z1")
                        for r in range(r0, r1):
                            nc.tensor.matmul(pz[:, r - r0], ea[:, (r - r0) * 128:(r - r0 + 1) * 128],
                                             w1bT_s[:], start=True, stop=True)
                        nc.vector.tensor_add(g[:, r0:r1, 192:256], g[:, r0:r1, 192:256],
                                             pz[:, 0:r1 - r0])
                    nc.vector.tensor_scalar_mul(ev, g[:, 0:Gr, 192:256], NS)
                    nc.vector.tensor_max(g[:, 0:Gr, 192:256], g[:, 0:Gr, 192:256], ev)
                    nc.vector.tensor_mul(ev, g[:, 0:Gr, 192:256],
                                         attl_s[:].to_broadcast([128, Gr, 64]))
                else:
                    nc.vector.tensor_mul(ev, g[:, 0:Gr, 64:128],
                                         asrc_s[:, layer - 1:layer].to_broadcast([128, Gr, 64]))
                nc.vector.tensor_reduce(lg[:, 0:Gr], ev, mybir.AxisListType.X, OP.add)
                nc.vector.tensor_add(lg[:, 0:Gr], lg[:, 0:Gr], dvals[:, ch0:ch0 + Gr])
                nc.vector.scalar_tensor_tensor(lg[:, 0:Gr], lg[:, 0:Gr], NS, lg[:, 0:Gr],
                                               OP.mult, OP.max)
                nc.scalar.activation(lg[:, 0:Gr], lg[:, 0:Gr], AF.Exp)
                p16 = sp.tile([128, GT, 1], F32, tag="p16")
                nc.vector.tensor_mul(p16[:, 0:Gr], lg[:, 0:Gr], msk_s[:, ch0:ch0 + Gr])
                # msg: x part scaled in place; denominator via per-block reduce
                nc.vector.tensor_mul(g[:, 0:Gr, half:half + 64], g[:, 0:Gr, half:half + 64],
                                     p16[:, 0:Gr].to_broadcast([128, Gr, 64]))
                o = 0
                for b in blist:
                    Rb = int(R[b])
                    pred = prp.tile([128, 64], F32, tag="pred")
                    for j in range(o, o + Rb):
                        nc.tensor.matmul(pred[:, 0:64], id16[:], g[:, j, half:half + 64],
                                         start=(j == o), stop=(j == o + Rb - 1))
                    rec = sp.tile([128, 1], F32, tag="rec")
                    nc.vector.tensor_reduce(rec[:], p16[:, o:o + Rb], mybir.AxisListType.XY, OP.add)
                    o += Rb
                    nc.vector.tensor_scalar(rec[:], rec[:], 1e-16, None, OP.add)
                    nc.vector.reciprocal(rec[:], rec[:])
                    hnm = sp.tile([128, 64], F16, tag="hnm")
                    nc.scalar.activation(hnm[:], pred[:, 0:64], AF.Copy, scale=rec[:])
                    ps = transp16(hnm[:], 128, 64)
                    nc.scalar.activation(hdst[0:64, b * 128:(b + 1) * 128], ps[0:64, 0:128], AF.Copy)
                if window_cb is not None:
                    window_cb(blist[-1] + 1)

        def transp16(in_ap, a, bdim):
            """Transpose [a, bdim] f16 SBUF -> PSUM [bdim, a]."""
            ps = rp.tile([128, 128], F16, tag="tp16")
            nc.tensor.transpose(ps[0:bdim, 0:a], in_ap, id16[0:a, 0:a])
            return ps

        def build_rows(cin_t, width, x16, lhsT_A, lhsT_B, dst_rep, dvals, dvx=None):
            """Per block: rows[:, 0:64] = (A @ x_b).T (A None -> x_b.T);
            cols 64:128 = (B @ x_b).T (B 'plain' -> x_b.T). dst_rep -> dvals."""
            cview = cin_t[:].rearrange("(b p) e -> b p e", p=128)
            for b in range(NBLK):
                build_rows_block(cview, width, x16, b, lhsT_A, lhsT_B, dst_rep, dvals, dvx)

        def build_rows_block(cview, width, x16, b, lhsT_A, lhsT_B, dst_rep, dvals, dvx=None):
            rows_b = sp.tile([128, 128], F16, tag="rb")
            xb = x16[:, b * 128:(b + 1) * 128]
            if lhsT_A is not None:
                psm = pp.tile([128, CH], F32, tag="mmq")
                nc.tensor.matmul(psm[0:64, 0:128], lhsT_A, xb, start=True, stop=True)
                xa = sp.tile([64, 128], F16, tag="xa")
                nc.scalar.activation(xa[:], psm[0:64, 0:128], AF.Copy)
                ps = transp16(xa[:], 64, 128)
            else:
                ps = transp16(xb, 64, 128)
            nc.scalar.activation(rows_b[:, 0:64], ps[0:128, 0:64], AF.Copy)
            if lhsT_B is not None:
                if isinstance(lhsT_B, str):
                    ps2 = transp16(xb, 64, 128)
                else:
                    psm2 = pp.tile([128, CH], F32, tag="mmq")
                    nc.tensor.matmul(psm2[0:64, 0:128], lhsT_B, xb, start=True, stop=True)
                    xa2 = sp.tile([64, 128], F16, tag="xa")
                    nc.scalar.activation(xa2[:], psm2[0:64, 0:128], AF.Copy)
                    ps2 = transp16(xa2[:], 64, 128)
                nc.scalar.activation(rows_b[:, 64:128], ps2[0:128, 0:64], AF.Copy)
            if dst_rep is not None:
                m = sp.tile([128, H], F32, tag="dvm")
                nc.vector.tensor_mul(m[:], rows_b[:, 0:64], dst_rep)
                nc.vector.tensor_reduce(dvals[:, b:b + 1], m[:], mybir.AxisListType.X, OP.add)
                Rb = int(R[b])
                nc.scalar.activation(dvx[:, bases[b]:bases[b] + Rb],
                                     msk_s[:, bases[b]:bases[b] + Rb], AF.Identity,
                                     scale=0.0, bias=dvals[:, b:b + 1])
            nc.sync.dma_start(cview[b], rows_b[:, 0:width])

        def make_window_cb(xcur_t, elu_bias, elu_pre, widx, xsep_cur_t, xsep_n_t,
                           xnew_t, cin_t, width, lA, lB, drep, dv_t, dvx_t=None):
            """After edge-phase blocks land in xcur_t[0:64], run the node phase
            (elu+gru) and next layer's row build for completed 512-col windows."""
            cview = cin_t[:].rearrange("(b p) e -> b p e", p=128)
            br, bz = gru_bias(widx)
            state = {"done": 0}

            def cb(done):
                while state["done"] + 4 <= done or (done >= NBLK and state["done"] < NBLK):
                    w0 = state["done"]
                    w1 = min(w0 + 4, NBLK)
                    c0 = w0 * 128
                    w = (w1 - w0) * 128
                    elu_chunk(xcur_t, elu_bias, c0, w, pre_lhsT=elu_pre)
                    gru_chunk(xcur_t, xsep_cur_t[:], widx, xsep_n_t[:], c0, w, br, bz)
                    if xnew_t is not None:
                        nc.sync.dma_start(xnew_t[64:128, c0:c0 + w], xsep_n_t[:, c0:c0 + w])
                    for b in range(w0, w1):
                        build_rows_block(cview, width, xsep_n_t[:], b, lA, lB, drep, dv_t, dvx_t)
                    state["done"] = w1
            return cb

        # ================== forward ==================
        stack0 = stkp.tile([128, NPAD], F16, tag="stk")
        xsep0 = xsp.tile([64, NPAD], F16, tag="xsep")
        dv0 = big.tile([128, NBLK, 1], F32, tag="rvals0")
        dvx0 = big.tile([128, NCH, 1], F32, tag="rvx0")
        cview0 = cins[0][:].rearrange("(b p) e -> b p e", p=128)
        for c0 in range(0, NPAD, CH):
            w = min(CH, NPAD - c0)
            xst = sp.tile([64, CH], F32, tag="nn")
            nc.sync.dma_start(xst[:, 0:w], xT_in[:, c0:c0 + w])
            fm_mm(xsep0[:, c0:c0 + w], lin1T_s[:], xst[:, 0:w], w, func=AF.Lrelu, bias=lin1b_s[:])
            nc.sync.dma_start(stack0[64:128, c0:c0 + w], xsep0[:, c0:c0 + w])
            for b in range(c0 // 128, min((c0 + w) // 128, NBLK)):
                build_rows_block(cview0, 128, xsep0[:], b, None, w1aT_s[:], attr_s[:], dv0, dvx0)
        nc.gpsimd.collective_compute("AllGather", OP.bypass, ins=[cins[0][:]],
                                     outs=[couts[0][:]], replica_groups=[list(range(NCORE))])
        stack1 = stkp.tile([128, NPAD], F16, tag="stk")
        xsep1 = xsp.tile([64, NPAD], F16, tag="xsep")
        dv1 = big.tile([128, NBLK, 1], F32, tag="rvals1")
        dvx1 = big.tile([128, NCH, 1], F32, tag="rvx1")
        cb0 = make_window_cb(stack0, gateb_s[:], g2T_s[:], 0, xsep0, xsep1,
                             stack1, cins[1], 64, atomT_s[:, 0], None, adst_s[:, 0], dv1, dvx1)
        edge_phase(0, couts[0], dvx0, stack0, window_cb=cb0)
        nc.gpsimd.collective_compute("AllGather", OP.bypass, ins=[cins[1][:]],
                                     outs=[couts[1][:]], replica_groups=[list(range(NCORE))])
        stack2 = stkp.tile([128, NPAD], F16, tag="stk")
        xsep2 = xsp.tile([64, NPAD], F16, tag="xsep")
        dv2 = big.tile([128, NBLK, 1], F32, tag="rvals2")
        dvx2 = big.tile([128, NCH, 1], F32, tag="rvx2")
        cb1 = make_window_cb(stack1, atomb_s[:, 0:1], None, 1, xsep1, xsep2,
                             stack2, cins[2], 64, atomT_s[:, 1], None, adst_s[:, 1], dv2, dvx2)
        edge_phase(1, couts[1], dvx1, stack1, window_cb=cb1)
        nc.gpsimd.collective_compute("AllGather", OP.bypass, ins=[cins[2][:]],
                                     outs=[couts[2][:]], replica_groups=[list(range(NCORE))])
        xfin = xsp.tile([64, NPAD], F16, tag="xsep")
        cb2 = make_window_cb(stack2, atomb_s[:, 1:2], None, 2, xsep2, xfin,
                             None, cinr, 128, molT_s[:], "plain", None, None)
        edge_phase(2, couts[2], dvx2, stack2, window_cb=cb2)
        nc.gpsimd.collective_compute("AllGather", OP.bypass, ins=[cinr[:]],
                                     outs=[coutr[:]], replica_groups=[list(range(NCORE))])
        # ---- readout: gather rows, resident rrow [xs | x], on-the-fly a_src
        rrow = big.tile([128, GCH, 128], F16, tag="rrow")
        asrc_ro = big.tile([128, GCH, 1], F32, tag="asr")
        RH = 8
        for b in range(GBLK):
            RGb = int(RG[b])
            gb0 = gbases[b]
            for r0 in range(0, RGb, RH):
                cr = min(RH, RGb - r0)
                c0 = gb0 + r0
                gr = grp_.tile([128, RH, 256], F16, tag="gr")
                nc.gpsimd.dma_gather(
                    gr[:, 0:cr], coutr[:], rgidx_s[:, c0 * 8:(c0 + cr) * 8],
                    cr * 128, cr * 128, 256, elem_step=256,
                    single_packet=False, queue_num=qn_state[0] % 4)
                qn_state[0] += 1
                sl = rrow[:, c0:c0 + cr]
                nc.vector.tensor_sub(sl, gr[:, 0:cr, 128:256], gr[:, 0:cr, 0:128])
                nc.vector.tensor_mul(sl, sl, rpar_s[:, c0:c0 + cr].to_broadcast([128, cr, 128]))
                nc.vector.tensor_add(sl, sl, gr[:, 0:cr, 0:128])
                nc.vector.tensor_mul(rrow[:, c0:c0 + cr, 64:128], rrow[:, c0:c0 + cr, 64:128],
                                     rmsk16_s[:, c0:c0 + cr].to_broadcast([128, cr, 64]))
                evr = gr[:, 0:cr, 0:64]  # dead (g0) half reused as scratch
                nc.vector.tensor_mul(evr, rrow[:, c0:c0 + cr, 0:64],
                                     msrc_s[:].to_broadcast([128, cr, 64]))
                nc.vector.tensor_reduce(asrc_ro[:, c0:c0 + cr], evr,
                                        mybir.AxisListType.X, OP.add)
        ofm = big.tile([64, GPAD], F32, tag="ofm")
        hro = big.tile([64, GPAD], F32, tag="hro")
        mol_stk = big.tile([128, GPAD], F16, tag="mstk")
        for b in range(GBLK):
            RGb = int(RG[b])
            pred = prp.tile([128, 64], F32, tag="pred")
            for j in range(RGb):
                nc.tensor.matmul(pred[:, 0:64], id16[:], rrow[:, gbases[b] + j, 64:128],
                                 start=(j == 0), stop=(j == RGb - 1))
            s0 = sp.tile([128, 64], F16, tag="hnm")
            nc.scalar.activation(s0[:], pred[:, 0:64], AF.Copy)
            ps = transp16(s0[:], 128, 64)
            nc.scalar.activation(ofm[:, b * 128:(b + 1) * 128], ps[0:64, 0:128], AF.Relu)
        wtil_ps = pp.tile([128, CH], F32, tag="mmq")
        nc.tensor.matmul(wtil_ps[0:64, 0:1], mol_lin_s[:], mdst_s[:], start=True, stop=True)
        wtil = cpool.tile([64, 1], F32)
        nc.vector.tensor_copy(wtil[:], wtil_ps[0:64, 0:1])
        rmsg = big.tile([128, GCH, 64], F16, tag="rmsg")
        for t in range(3):
            ddp = pp.tile([128, CH], F32, tag="mmq")
            nc.tensor.matmul(ddp[0:1, 0:GPAD], wtil[:], ofm[:], start=True, stop=True)
            dds = s1.tile([1, GPAD], F16, tag="dds")
            nc.scalar.activation(dds[:], ddp[0:1, 0:GPAD], AF.Copy)
            lgr = sp.tile([128, GCH, 1], F32, tag="lgr")
            for b in range(GBLK):
                RGb = int(RG[b])
                gb0 = gbases[b]
                psb = transp16(dds[:, b * 128:(b + 1) * 128], 1, 128)
                ddb = sp.tile([128, 1], F32, tag="ddb")
                nc.scalar.activation(ddb[:], psb[0:128, 0:1], AF.Copy)
                nc.vector.tensor_scalar(lgr[:, gb0:gb0 + RGb], asrc_ro[:, gb0:gb0 + RGb],
                                        ddb[:, 0:1], None, OP.add)
            nc.vector.scalar_tensor_tensor(lgr[:], lgr[:], NS, lgr[:], OP.mult, OP.max)
            nc.scalar.activation(lgr[:], lgr[:], AF.Exp)
            p16r = sp.tile([128, GCH, 1], F32, tag="p16r")
            nc.vector.tensor_mul(p16r[:], lgr[:], rmsk_s[:])
            nc.vector.tensor_mul(rmsg[:, :, 0:64], rrow[:, :, 0:64],
                                 p16r[:].to_broadcast([128, GCH, 64]))
            for b in range(GBLK):
                RGb = int(RG[b])
                gb0 = gbases[b]
                pred = prp.tile([128, 64], F32, tag="pred")
                for j in range(RGb):
                    nc.tensor.matmul(pred[:, 0:64], id16[:], rmsg[:, gb0 + j, 0:64],
                                     start=(j == 0), stop=(j == RGb - 1))
                rec = sp.tile([128, 1], F32, tag="rec")
                nc.vector.tensor_reduce(rec[:], p16r[:, gb0:gb0 + RGb], mybir.AxisListType.XY, OP.add)
                nc.vector.tensor_scalar(rec[:], rec[:], 1e-16, None, OP.add)
                nc.vector.reciprocal(rec[:], rec[:])
                hnm = sp.tile([128, 64], F16, tag="hnm")
                nc.scalar.activation(hnm[:], pred[:, 0:64], AF.Copy, scale=rec[:])
                ps = transp16(hnm[:], 128, 64)
                nc.scalar.activation(hro[:, b * 128:(b + 1) * 128], ps[0:64, 0:128], AF.Copy)
            elu_inplace(hro, molb_s[:], GPAD)
            nc.scalar.activation(mol_stk[0:64, :], hro[:], AF.Copy)
            nc.gpsimd.dma_start(mol_stk[64:128, :], ofm[:])
            onew = s1.tile([64, GPAD], F32, tag="onew")
            gru(mol_stk, ofm, 3, onew[:], GPAD)
            nc.scalar.activation(ofm[:], onew[:], AF.Copy)
        emb = s1.tile([64, GPAD], F16, tag="emb")
        fm_mm(emb[:], lin2T_s[:], ofm[:], GPAD, bias=lin2b_s[:])
        nemb = s1.tile([64, GPAD], F32, tag="nemb")
        for b in range(GBLK):
            gps = transp16(emb[:, b * 128:(b + 1) * 128], 64, 128)
            gm = sp.tile([128, 64], F32, tag="gm")
            nc.scalar.activation(gm[:], gps[0:128, 0:64], AF.Copy)
            mu = sp.tile([128, 1], F32, tag="mu")
            nc.vector.tensor_reduce(mu[:], gm[:, 0:64], mybir.AxisListType.X, OP.add)
            nc.vector.tensor_scalar(mu[:], mu[:], 1.0 / 64, None, OP.mult)
            xc = sp.tile([128, 64], F32, tag="xc")
            nc.vector.tensor_scalar(xc[:], gm[:, 0:64], mu[:], None, OP.subtract)
            sq = sp.tile([128, 64], F32, tag="sq")
            nc.scalar.activation(sq[:], xc[:], AF.Square)
            var = sp.tile([128, 1], F32, tag="var")
            nc.vector.tensor_reduce(var[:], sq[:], mybir.AxisListType.X, OP.add)
            nc.vector.tensor_scalar(var[:], var[:], 1.0 / 64, None, OP.mult)
            nc.vector.tensor_scalar(var[:], var[:], 1e-5, None, OP.add)
            nc.scalar.activation(var[:], var[:], AF.Sqrt)
            nc.vector.reciprocal(var[:], var[:])
            nc.scalar.activation(xc[:], xc[:], AF.Copy, scale=var[:])
            nc.vector.tensor_mul(xc[:], xc[:], lng_s[:, 0:64])
            nc.vector.tensor_add(xc[:], xc[:], lnb_s[:, 0:64])
            xc16 = sp.tile([128, 64], F16, tag="hnm")
            nc.scalar.activation(xc16[:], xc[:], AF.Copy)
            ps2 = transp16(xc16[:], 128, 64)
            nc.scalar.activation(nemb[:, b * 128:(b + 1) * 128], ps2[0:64, 0:128], AF.Copy)
        m1 = s1.tile([64, GPAD], F32, tag="m1")
        fm_mm(m1[:], h1T_s[:], nemb[:], GPAD, func=AF.Relu, bias=h1b_s[:])
        m2 = s1.tile([64, GPAD], F32, tag="m2")
        fm_mm(m2[:], h2T_s[:], m1[:], GPAD, func=AF.Relu, bias=h2b_s[:])
        yps = pp.tile([128, CH], F32, tag="mmq")
        nc.tensor.matmul(yps[0:1, 0:GPAD], h3T_s[:], m2[:], start=True, stop=True)
        ysb = s1.tile([1, GPAD], F32, tag="ysb")
        nc.scalar.activation(ysb[:], yps[0:1, 0:GPAD], AF.Identity, bias=h3b_s[:])
        nc.sync.dma_start(y_out[:], ysb[:])
        ctx.close()
    ctx2.__exit__(None, None, None)
    nc.finalize()
    return nc


_CACHE = {}


def kernel(**inputs):
    x = np.asarray(inputs["x"], np.float32)
    edge_attr = np.asarray(inputs["edge_attr"], np.float32)
    ei = np.asarray(inputs["edge_index"])
    batch = np.asarray(inputs["batch"])
    if "k" not in _CACHE:
        plan = build_plan(ei, batch)
        nc = build_nc(plan)
        _CACHE["k"] = (plan, nc)
    plan, nc = _CACHE["k"]

    gw = np.zeros((128, 4, 2, H), np.float16)
    gn = np.zeros((128, 4, 2, H), np.float16)
    gbx_rz = np.zeros((64, 8), np.float32)
    gbh_rz = np.zeros((64, 8), np.float32)
    gbx_n = np.zeros((H, 4), np.float32)
    gbh_n = np.zeros((H, 4), np.float32)
    packs = [
        (inputs["gru0_wx"], inputs["gru0_wh"], inputs["gru0_bx"], inputs["gru0_bh"]),
        (inputs["atom_gru_wx"][0], inputs["atom_gru_wh"][0], inputs["atom_gru_bx"][0], inputs["atom_gru_bh"][0]),
        (inputs["atom_gru_wx"][1], inputs["atom_gru_wh"][1], inputs["atom_gru_bx"][1], inputs["atom_gru_bh"][1]),
        (inputs["mol_gru_wx"], inputs["mol_gru_wh"], inputs["mol_gru_bx"], inputs["mol_gru_bh"]),
    ]
    for i, (wx, wh, bx, bh) in enumerate(packs):
        wx = np.asarray(wx, np.float32); wh = np.asarray(wh, np.float32)
        bx = np.asarray(bx, np.float32); bh = np.asarray(bh, np.float32)
        gw[0:64, i, 0] = wx[0:64].T; gw[64:128, i, 0] = wh[0:64].T
        gw[0:64, i, 1] = wx[64:128].T; gw[64:128, i, 1] = wh[64:128].T
        gn[0:64, i, 0] = wx[128:192].T; gn[64:128, i, 1] = wh[128:192].T
        gbx_rz[:, 2 * i] = bx[0:64]; gbx_rz[:, 2 * i + 1] = bx[64:128]
        gbh_rz[:, 2 * i] = bh[0:64]; gbh_rz[:, 2 * i + 1] = bh[64:128]
        gbx_n[:, i] = bx[128:192]; gbh_n[:, i] = bh[128:192]

    glw = np.asarray(inputs["gate_lin1_w"], np.float32)
    rep16 = lambda v: np.tile(np.asarray(v, np.float16).reshape(1, -1), (128, 1))
    rep32 = lambda v: np.tile(np.asarray(v, np.float32).reshape(1, -1), (128, 1))
    a = lambda k: np.asarray(inputs[k], np.float32)
    a16 = lambda k: np.asarray(inputs[k], np.float16)
    wts = dict(
        lin1T=a("lin1_w").T.copy(), lin1_b=a("lin1_b").reshape(H, 1),
        w1aT=glw[:, 0:64].T.astype(np.float16).copy(),
        w1bT=glw[:, 64:80].T.astype(np.float16).copy(),
        attl_rep=rep16(inputs["gate_att_l"]).reshape(128, 1, H),
        attr_rep=rep16(inputs["gate_att_r"]),
        g2T=a16("gate_lin2_w").T.copy(), gate_b=a("gate_bias").reshape(H, 1),
        atomT=np.stack([a16("atom_lin_w")[l].T for l in range(2)], 1),
        asrc_rep=np.stack([rep16(inputs["atom_att_src"][l]) for l in range(2)], 1),
        adst_rep=np.stack([rep16(inputs["atom_att_dst"][l]) for l in range(2)], 1),
        atom_b=a("atom_bias").T.copy(),
        molT=a16("mol_lin_w").T.copy(), mol_lin=a("mol_lin_w").copy(),
        matt_src_rep=rep16(inputs["mol_att_src"]).reshape(128, 1, H),
        matt_dst=a("mol_att_dst").reshape(H, 1),
        mol_b=a("mol_bias").reshape(H, 1),
        gruW=gw, gruN=gn, gbx_rz=gbx_rz, gbh_rz=gbh_rz, gbx_n=gbx_n, gbh_n=gbh_n,
        lin2T=a("lin2_w").T.copy(), lin2_b=a("lin2_b").reshape(H, 1),
        lng_rep=rep32(inputs["ln_g"]), lnb_rep=rep32(inputs["ln_b"]),
        h1T=a("h1_w").T.copy(), h1_b=a("h1_b").reshape(H, 1),
        h2T=a("h2_w").T.copy(), h2_b=a("h2_b").reshape(H, 1),
        h3T=a("h3_w").T.copy(), h3_b=a("h3_b").reshape(1, 1),
    )
    in_maps = []
    for c in range(NCORE):
        pc = plan["cores"][c]
        rc = plan["rcores"][c]
        xT = np.zeros((H, NPAD), np.float32)
        xT[:, 0:NPC] = x[c * NPC:(c + 1) * NPC][plan["perms"][c]].T
        eaT = np.zeros((16, plan["S"]), np.float16)
        val = pc["esel"] >= 0
        eaT[:, val] = edge_attr[pc["esel"][val]].T.astype(np.float16)
        im = dict(xT_in=xT, gidx=pc["gidx"], par=pc["par"], msk=pc["msk"], eaT=eaT,
                  rgidx=rc["rgidx"], rpar=rc["rpar"], rmsk=rc["rmsk"],
                  rmsk16=rc["rmsk16"], **wts)
        in_maps.append(im)
    res = run_bass_kernel_spmd(nc, in_maps, core_ids=list(range(NCORE)))
    if getattr(res, "exec_time_ns", None):
        print(f"HW exec time: {res.exec_time_ns} ns", flush=True)
    y = np.zeros(B, np.float32)
    for c in range(NCORE):
        ys = res.results[c]["y_out"].reshape(GPAD)[0:GPC]
        y[c * GPC + plan["gperms"][c]] = ys
    return y.reshape(B, 1)


# revision 30
# speedup vs baseline: 1.0777x; 1.0777x over previous
"""AttentiveFP forward on 8 Trainium2 NeuronCores (Bass/Tile).

Edges sharded by dst-owner core; per-core nodes sorted by in-degree with a
round-robin slot structure so segment softmax/sum become dense PSUM matmul
accumulation. Edge phase processed in large multi-block groups: one
dma_gather per ~34 rounds, wide DVE ops (parity select in place), DVE
leaky-relu (no ACT table ping-pong), single Exp per group. Layers 1-2 use
256B gather elements (64-f16 rows, att_src dot computed on the fly), halving
gather traffic and AllGather payload. fp16 node state feeds all PE matmuls.
"""
import numpy as np
from contextlib import ExitStack

import concourse.bass as bass
import concourse.tile as tile
from concourse import bacc, mybir
from concourse.bass_utils import run_bass_kernel_spmd
from concourse.masks import make_identity

F32 = mybir.dt.float32
F16 = mybir.dt.float16
I16 = mybir.dt.int16
AF = mybir.ActivationFunctionType
OP = mybir.AluOpType

NCORE = 8
N, E, B = 50000, 500000, 2048
H = 64
NS = 0.01
NPC = N // NCORE
GPC = B // NCORE
NBLK = (NPC + 127) // 128
NPAD = NBLK * 128
GBLK = (GPC + 127) // 128
GPAD = GBLK * 128
TROWS = NCORE * NPAD
MAXR = 10
CH = 512
GT = 24       # max rounds per edge-phase group


def _calls_for(R, maxr=MAXR):
    calls, base, bases = [], 0, []
    for r in R:
        bases.append(base)
        calls.append([(r0, min(r0 + maxr, int(r))) for r0 in range(0, int(r), maxr)])
        base += int(r)
    return calls, bases, base


def _wrap_into(gidx, arr, col0):
    n = arr.shape[0]
    blk = arr.reshape(n // 16, 16).T
    gidx[:16, col0:col0 + n // 16] = blk
    gidx[16:128, col0:col0 + n // 16] = np.tile(blk, (7, 1))


def build_plan(edge_index, batch):
    src = edge_index[0].astype(np.int64)
    dst = edge_index[1].astype(np.int64)
    owner = dst // NPC

    perms, degs_sorted, grp_starts, egrp = [], [], [], []
    sortpos = np.zeros(N, np.int64)
    for c in range(NCORE):
        n0 = c * NPC
        emask = np.nonzero(owner == c)[0]
        deg = np.bincount(dst[emask] - n0, minlength=NPC)
        order = np.argsort(-deg, kind="stable")
        perms.append(order)
        sortpos[n0 + order] = np.arange(NPC)
        dsorted = deg[order]
        degs_sorted.append(dsorted)
        eorder = np.argsort(sortpos[dst[emask]], kind="stable")
        egrp.append(emask[eorder])
        grp_starts.append(np.concatenate([[0], np.cumsum(dsorted)]))
    trow = (np.arange(N) // NPC) * NPAD + sortpos

    R = np.ones(NBLK, np.int64)
    for b in range(NBLK):
        for c in range(NCORE):
            d = degs_sorted[c][b * 128:(b + 1) * 128]
            if len(d):
                R[b] = max(R[b], int(d[0]))
    calls, bases, NCH = _calls_for(R)
    S = NCH * 128

    gsize = np.bincount(batch, minlength=B)
    gstart = np.concatenate([[0], np.cumsum(gsize)])
    gperms, gss = [], []
    for c in range(NCORE):
        gs = gsize[c * GPC:(c + 1) * GPC]
        gorder = np.argsort(-gs, kind="stable")
        gperms.append(gorder)
        gss.append(gs[gorder])
    RG = np.ones(GBLK, np.int64)
    for b in range(GBLK):
        for c in range(NCORE):
            d = gss[c][b * 128:(b + 1) * 128]
            if len(d):
                RG[b] = max(RG[b], int(d[0]))
    gcalls, gbases, GCH = _calls_for(RG, 8)
    SR = GCH * 128

    cores = []
    lanes = np.arange(128)
    for c in range(NCORE):
        gidx = np.zeros((128, S // 16), np.int16)
        par = np.zeros((128, NCH, 1), np.float16)
        msk = np.zeros((128, NCH, 1), np.float32)
        esel = np.full(NCH * 128, -1, np.int64)
        ds = degs_sorted[c]
        gst = grp_starts[c]
        eg = egrp[c]
        for b in range(NBLK):
            for (r0, r1) in calls[b]:
                ia = np.zeros((r1 - r0) * 128, np.int64)
                for r in range(r0, r1):
                    ch = bases[b] + r
                    p = b * 128 + lanes
                    pc = np.minimum(p, NPC - 1)
                    ok = (p < NPC) & (r < ds[pc])
                    eids = gst[pc] + r
                    ge = np.where(ok, eg[np.where(ok, np.minimum(eids, len(eg) - 1), 0)], -1)
                    esel[ch * 128 + lanes] = ge
                    rows = np.where(ok, trow[src[np.maximum(ge, 0)]], 0)
                    ia[(r - r0) * 128 + lanes] = rows >> 1
                    par[:, ch, 0] = (rows & 1).astype(np.float16)
                    msk[:, ch, 0] = ok.astype(np.float32)
                _wrap_into(gidx, ia, (bases[b] + r0) * 8)
        cores.append(dict(gidx=gidx, par=par, msk=msk, esel=esel))

    rcores = []
    for c in range(NCORE):
        rgidx = np.zeros((128, SR // 16), np.int16)
        rpar = np.zeros((128, GCH, 1), np.float16)
        rmsk = np.zeros((128, GCH, 1), np.float32)
        rmsk16 = np.zeros((128, GCH, 1), np.float16)
        gs = gss[c]
        gp = gperms[c]
        for b in range(GBLK):
            for (r0, r1) in gcalls[b]:
                ia = np.zeros((r1 - r0) * 128, np.int64)
                for r in range(r0, r1):
                    ch = gbases[b] + r
                    p = b * 128 + lanes
                    pc = np.minimum(p, GPC - 1)
                    ok = (p < GPC) & (r < gs[pc])
                    g = c * GPC + gp[pc]
                    node = np.where(ok, gstart[g] + r, 0)
                    rows = np.where(ok, trow[node], 0)
                    ia[(r - r0) * 128 + lanes] = rows >> 1
                    rpar[:, ch, 0] = (rows & 1).astype(np.float16)
                    rmsk[:, ch, 0] = ok.astype(np.float32)
                    rmsk16[:, ch, 0] = ok.astype(np.float16)
                _wrap_into(rgidx, ia, (gbases[b] + r0) * 8)
        rcores.append(dict(rgidx=rgidx, rpar=rpar, rmsk=rmsk, rmsk16=rmsk16))

    return dict(R=R, bases=bases, NCH=NCH, S=S,
                RG=RG, gbases=gbases, GCH=GCH, SR=SR,
                cores=cores, rcores=rcores, perms=perms, gperms=gperms)


def build_nc(plan):
    R, bases, NCH, S = plan["R"], plan["bases"], plan["NCH"], plan["S"]
    RG, gbases, GCH, SR = plan["RG"], plan["gbases"], plan["GCH"], plan["SR"]
    RGT = int(max(int(r) for r in RG))

    # pack whole blocks into groups of <= cap rounds
    def pack_groups(cap):
        gs, cur, cur_r = [], [], 0
        for b in range(NBLK):
            r = int(R[b])
            if cur and cur_r + r > cap:
                gs.append(cur)
                cur, cur_r = [], 0
            cur.append(b)
            cur_r += r
        gs.append(cur)
        return gs
    groups0 = pack_groups(int(max(R)))
    groups12 = pack_groups(GT)
    GT0 = max(sum(int(R[b]) for b in bl) for bl in groups0)
    GT12 = max(sum(int(R[b]) for b in bl) for bl in groups12)

    nc = bacc.Bacc("TRN2", target_bir_lowering=False, debug=False,
                   num_devices=NCORE, num_swdge_queues=4)

    def din(name, shape, dt=F32):
        return nc.dram_tensor(name, shape, dt, kind="ExternalInput")

    xT_in = din("xT_in", [H, NPAD])
    gidx_in = din("gidx", [128, S // 16], I16)
    par_in = din("par", [128, NCH, 1], F16)
    msk_in = din("msk", [128, NCH, 1], F32)
    eaT_in = din("eaT", [16, S], F16)
    rgidx_in = din("rgidx", [128, SR // 16], I16)
    rpar_in = din("rpar", [128, GCH, 1], F16)
    rmsk_in = din("rmsk", [128, GCH, 1], F32)
    rmsk16_in = din("rmsk16", [128, GCH, 1], F16)
    lin1T = din("lin1T", [H, H]); lin1_b = din("lin1_b", [H, 1])
    w1aT = din("w1aT", [H, H], F16); w1bT = din("w1bT", [16, H], F16)
    attl_rep = din("attl_rep", [128, 1, H], F16)
    attr_rep = din("attr_rep", [128, H], F16)
    g2T = din("g2T", [H, H], F16); gate_b = din("gate_b", [H, 1])
    atomT = din("atomT", [H, 2, H], F16)
    asrc_rep = din("asrc_rep", [128, 2, H], F16)
    adst_rep = din("adst_rep", [128, 2, H], F16)
    atom_b = din("atom_b", [H, 2])
    molT = din("molT", [H, H], F16)
    mol_lin = din("mol_lin", [H, H])
    matt_src_rep = din("matt_src_rep", [128, 1, H], F16)
    matt_dst = din("matt_dst", [H, 1])
    mol_b = din("mol_b", [H, 1])
    gruW = din("gruW", [128, 4, 2, H], F16)   # [K, widx, gate(r/z), M]
    gruN = din("gruN", [128, 4, 2, H], F16)   # [K, widx, (nx/nh), M] zero-padded
    gbx_rz = din("gbx_rz", [64, 8])
    gbh_rz = din("gbh_rz", [64, 8])
    gbx_n = din("gbx_n", [H, 4])
    gbh_n = din("gbh_n", [H, 4])
    lin2T = din("lin2T", [H, H]); lin2_b = din("lin2_b", [H, 1])
    lng_rep = din("lng_rep", [128, H]); lnb_rep = din("lnb_rep", [128, H])
    h1T = din("h1T", [H, H]); h1_b = din("h1_b", [H, 1])
    h2T = din("h2T", [H, H]); h2_b = din("h2_b", [H, 1])
    h3T = din("h3T", [H, 1]); h3_b = din("h3_b", [1, 1])

    y_out = nc.dram_tensor("y_out", [1, GPAD], F32, kind="ExternalOutput")

    cin0 = nc.dram_tensor("cin0", [NPAD, 128], F16)
    cout0 = nc.dram_tensor("cout0", [TROWS // 2, 256], F16, addr_space="Shared")
    cins = [cin0]
    couts = [cout0]
    for l in (1, 2):
        cins.append(nc.dram_tensor(f"cin{l}", [NPAD, 64], F16))
        couts.append(nc.dram_tensor(f"cout{l}", [TROWS // 2, 128], F16,
                                    addr_space="Shared"))
    cinr = nc.dram_tensor("cinr", [NPAD, 128], F16)
    coutr = nc.dram_tensor("coutr", [TROWS // 2, 256], F16, addr_space="Shared")

    ctx = ExitStack()
    ctx2 = nc.allow_low_precision(reason="fp16 edge tables/messages/state")
    ctx2.__enter__()
    with tile.TileContext(nc) as tc:
        cpool = ctx.enter_context(tc.tile_pool(name="const", bufs=1))
        wpool = ctx.enter_context(tc.tile_pool(name="wts", bufs=1))
        big = ctx.enter_context(tc.tile_pool(name="big", bufs=1))
        stkp = ctx.enter_context(tc.tile_pool(name="stkp", bufs=2))
        xsp = ctx.enter_context(tc.tile_pool(name="xsp", bufs=2))
        g0p = ctx.enter_context(tc.tile_pool(name="g0p", bufs=3))
        g12p = ctx.enter_context(tc.tile_pool(name="g12p", bufs=4))
        grp_ = ctx.enter_context(tc.tile_pool(name="gathr", bufs=1))
        eap = ctx.enter_context(tc.tile_pool(name="eap", bufs=1))
        sp = ctx.enter_context(tc.tile_pool(name="scr", bufs=2))
        s1 = ctx.enter_context(tc.tile_pool(name="scr1", bufs=1))
        pp = ctx.enter_context(tc.tile_pool(name="ps", bufs=2, space="PSUM"))
        prp = ctx.enter_context(tc.tile_pool(name="psp", bufs=2, space="PSUM"))
        rp = ctx.enter_context(tc.tile_pool(name="psr", bufs=2, space="PSUM"))
        zp = ctx.enter_context(tc.tile_pool(name="psz", bufs=2, space="PSUM"))

        id32 = cpool.tile([128, 128], F32)
        make_identity(nc, id32[:])
        id16 = cpool.tile([128, 128], F16)
        nc.vector.tensor_copy(id16[:], id32[:])

        def load(t, shape, dt=F32):
            s = wpool.tile(shape, dt, tag=f"w_{t.name}")
            nc.sync.dma_start(s[:], t[:])
            return s

        gidx_s = load(gidx_in, [128, S // 16], I16)
        rgidx_s = load(rgidx_in, [128, SR // 16], I16)
        par_s = load(par_in, [128, NCH, 1], F16)
        msk_s = load(msk_in, [128, NCH, 1], F32)
        rpar_s = load(rpar_in, [128, GCH, 1], F16)
        rmsk_s = load(rmsk_in, [128, GCH, 1], F32)
        rmsk16_s = load(rmsk16_in, [128, GCH, 1], F16)
        lin1T_s = load(lin1T, [H, H]); lin1b_s = load(lin1_b, [H, 1])
        w1aT_s = load(w1aT, [H, H], F16); w1bT_s = load(w1bT, [16, H], F16)
        attl_s = load(attl_rep, [128, 1, H], F16)
        attr_s = load(attr_rep, [128, H], F16)
        g2T_s = load(g2T, [H, H], F16); gateb_s = load(gate_b, [H, 1])
        atomT_s = load(atomT, [H, 2, H], F16)
        asrc_s = load(asrc_rep, [128, 2, H], F16)
        adst_s = load(adst_rep, [128, 2, H], F16)
        atomb_s = load(atom_b, [H, 2])
        molT_s = load(molT, [H, H], F16); mol_lin_s = load(mol_lin, [H, H])
        msrc_s = load(matt_src_rep, [128, 1, H], F16)
        mdst_s = load(matt_dst, [H, 1])
        molb_s = load(mol_b, [H, 1])
        gruW_s = load(gruW, [128, 4, 2, H], F16)
        gruN_s = load(gruN, [128, 4, 2, H], F16)
        gbxrz_s = load(gbx_rz, [64, 8])
        gbhrz_s = load(gbh_rz, [64, 8])
        gbxn_s = load(gbx_n, [H, 4])
        gbhn_s = load(gbh_n, [H, 4])
        lin2T_s = load(lin2T, [H, H]); lin2b_s = load(lin2_b, [H, 1])
        lng_s = load(lng_rep, [128, H]); lnb_s = load(lnb_rep, [128, H])
        h1T_s = load(h1T, [H, H]); h1b_s = load(h1_b, [H, 1])
        h2T_s = load(h2T, [H, H]); h2b_s = load(h2_b, [H, 1])
        h3T_s = load(h3T, [H, 1]); h3b_s = load(h3_b, [1, 1])

        def fm_mm(out_ap, lhsT, rhs, ncols, func=None, bias=0.0):
            M = lhsT.shape[-1]
            for c0 in range(0, ncols, CH):
                w = min(CH, ncols - c0)
                ps = pp.tile([128, CH], F32, tag="mmq")
                nc.tensor.matmul(ps[0:M, :w], lhsT, rhs[:, c0:c0 + w],
                                 start=True, stop=True)
                f = func
                if f is None:
                    f = AF.Copy if isinstance(bias, float) else AF.Identity
                nc.scalar.activation(out_ap[:, c0:c0 + w], ps[0:M, :w], f, bias=bias)

        def gru_bias(widx):
            br = sp.tile([64, 1], F32, tag=f"brz{widx}")
            nc.vector.tensor_add(br[:], gbxrz_s[:, 2 * widx:2 * widx + 1], gbhrz_s[:, 2 * widx:2 * widx + 1])
            bz = sp.tile([64, 1], F32, tag=f"bz{widx}")
            nc.vector.tensor_add(bz[:], gbxrz_s[:, 2 * widx + 1:2 * widx + 2], gbhrz_s[:, 2 * widx + 1:2 * widx + 2])
            return br, bz

        def gru_chunk(stk, xprev, widx, out_ap, c0, w, br, bz):
            if True:
                prz = pp.tile([128, CH], F32, tag="mmq")
                nc.tensor.matmul(prz[0:64, :w], gruW_s[:, widx, 0], stk[:, c0:c0 + w], start=True, stop=True)
                prz2 = pp.tile([128, CH], F32, tag="mmq")
                nc.tensor.matmul(prz2[0:64, :w], gruW_s[:, widx, 1], stk[:, c0:c0 + w], start=True, stop=True)
                rz = sp.tile([64, CH], F32, tag="rz")
                nc.scalar.activation(rz[:, :w], prz[0:64, :w], AF.Sigmoid, bias=br[:])
                zz = sp.tile([64, CH], F32, tag="zz")
                nc.scalar.activation(zz[:, :w], prz2[0:64, :w], AF.Sigmoid, bias=bz[:])
                pn1 = pp.tile([128, CH], F32, tag="mmq")
                nc.tensor.matmul(pn1[0:64, :w], gruN_s[:, widx, 0], stk[:, c0:c0 + w], start=True, stop=True)
                pn2 = pp.tile([128, CH], F32, tag="mmq")
                nc.tensor.matmul(pn2[0:64, :w], gruN_s[:, widx, 1], stk[:, c0:c0 + w], start=True, stop=True)
                hnb = sp.tile([64, CH], F32, tag="hnb")
                nc.scalar.activation(hnb[:, :w], pn2[0:64, :w], AF.Identity, bias=gbhn_s[:, widx:widx + 1])
                nc.vector.tensor_mul(hnb[:, :w], rz[:, :w], hnb[:, :w])
                nc.vector.tensor_add(hnb[:, :w], hnb[:, :w], pn1[0:64, :w])
                nn = sp.tile([64, CH], F32, tag="nn")
                nc.scalar.activation(nn[:, :w], hnb[:, :w], AF.Tanh, bias=gbxn_s[:, widx:widx + 1])
                d = rz  # rz buffer is dead past the mul above
                nc.vector.tensor_sub(d[:, :w], xprev[:, c0:c0 + w], nn[:, :w])
                nc.vector.tensor_mul(d[:, :w], zz[:, :w], d[:, :w])
                nc.vector.tensor_add(d[:, :w], nn[:, :w], d[:, :w])
                nc.scalar.activation(out_ap[:, c0:c0 + w], d[:, :w], AF.Relu)

        def gru(stk, xprev, widx, out_ap, ncols):
            br, bz = gru_bias(widx)
            for c0 in range(0, ncols, CH):
                gru_chunk(stk, xprev, widx, out_ap, c0, min(CH, ncols - c0), br, bz)

        def elu_chunk(buf, bias_ap, c0, w, pre_lhsT=None):
            if pre_lhsT is not None:
                ps = pp.tile([128, CH], F32, tag="mmq")
                nc.tensor.matmul(ps[0:64, :w], pre_lhsT, buf[0:64, c0:c0 + w], start=True, stop=True)
                src = ps[0:64, :w]
            else:
                src = buf[0:64, c0:c0 + w]
            e1 = sp.tile([64, CH], F32, tag="rz")
            nc.scalar.activation(e1[:, :w], src, AF.Exp, bias=bias_ap)
            t1 = sp.tile([64, CH], F32, tag="zz")
            nc.scalar.activation(t1[:, :w], e1[:, :w], AF.Relu, bias=1.0, scale=-1.0)
            t2 = sp.tile([64, CH], F32, tag="hnb")
            nc.scalar.activation(t2[:, :w], src, AF.Relu, bias=bias_ap)
            nc.vector.tensor_sub(buf[0:64, c0:c0 + w], t2[:, :w], t1[:, :w])

        def elu_inplace(buf, bias_ap, ncols, pre_lhsT=None):
            """buf[0:64, :] = elu((pre_lhsT.T @ buf[0:64]) + bias)."""
            for c0 in range(0, ncols, CH):
                elu_chunk(buf, bias_ap, c0, min(CH, ncols - c0), pre_lhsT)

        qn_state = [0]

        def edge_phase(layer, table, dvals, hdst, window_cb=None):
            """hdst[0:64, :NPAD] (f16) <- normalized aggregation (feature-major)."""
            W = 256 if layer == 0 else 128
            half = W // 2
            pool = g0p if layer == 0 else g12p
            gtile = GT0 if layer == 0 else GT12
            for blist in (groups0 if layer == 0 else groups12):
                ch0 = bases[blist[0]]
                Gr = sum(int(R[b]) for b in blist)
                g = pool.tile([128, gtile, W], F16, tag=f"g{min(layer, 1)}")
                nc.gpsimd.dma_gather(
                    g[:, 0:Gr], table[:], gidx_s[:, ch0 * 8:(ch0 + Gr) * 8],
                    Gr * 128, Gr * 128, W, elem_step=W,
                    single_packet=False, queue_num=qn_state[0] % 4)
                qn_state[0] += 1
                # parity select in place: row -> g[:, :, half:W]
                nc.vector.tensor_sub(g[:, 0:Gr, half:W], g[:, 0:Gr, half:W], g[:, 0:Gr, 0:half])
                nc.vector.tensor_mul(g[:, 0:Gr, half:W], g[:, 0:Gr, half:W],
                                     par_s[:, ch0:ch0 + Gr].to_broadcast([128, Gr, half]))
                nc.vector.tensor_add(g[:, 0:Gr, half:W], g[:, 0:Gr, half:W], g[:, 0:Gr, 0:half])
                lg = sp.tile([128, gtile, 1], F32, tag=f"lg{min(layer,1)}")
                ev = g[:, 0:Gr, 0:64]  # dead (g0) half reused as scratch
                if layer == 0:
                    for r0 in range(0, Gr, 8):
                        r1 = min(r0 + 8, Gr)
                        ea = eap.tile([16, 8 * 128], F16, tag="ea")
                        nc.sync.dma_start(ea[:, 0:(r1 - r0) * 128],
                                          eaT_in[:, (ch0 + r0) * 128:(ch0 + r1) * 128])
                        pz = zp.tile([128, 8, 64], F32, tag="z1")
                        for r in range(r0, r1):
                            nc.tensor.matmul(pz[:, r - r0], ea[:, (r - r0) * 128:(r - r0 + 1) * 128],
                                             w1bT_s[:], start=True, stop=True)
                        nc.vector.tensor_add(g[:, r0:r1, 192:256], g[:, r0:r1, 192:256],
                                             pz[:, 0:r1 - r0])
                    nc.vector.tensor_scalar_mul(ev, g[:, 0:Gr, 192:256], NS)
                    nc.vector.tensor_max(g[:, 0:Gr, 192:256], g[:, 0:Gr, 192:256], ev)
                    nc.vector.tensor_mul(ev, g[:, 0:Gr, 192:256],
                                         attl_s[:].to_broadcast([128, Gr, 64]))
                else:
                    nc.vector.tensor_mul(ev, g[:, 0:Gr, 64:128],
                                         asrc_s[:, layer - 1:layer].to_broadcast([128, Gr, 64]))
                nc.vector.tensor_reduce(lg[:, 0:Gr], ev, mybir.AxisListType.X, OP.add)
                nc.vector.tensor_add(lg[:, 0:Gr], lg[:, 0:Gr], dvals[:, ch0:ch0 + Gr])
                nc.vector.scalar_tensor_tensor(lg[:, 0:Gr], lg[:, 0:Gr], NS, lg[:, 0:Gr],
                                               OP.mult, OP.max)
                nc.scalar.activation(lg[:, 0:Gr], lg[:, 0:Gr], AF.Exp)
                p16 = sp.tile([128, gtile, 1], F32, tag=f"p16{min(layer,1)}")
                nc.vector.tensor_mul(p16[:, 0:Gr], lg[:, 0:Gr], msk_s[:, ch0:ch0 + Gr])
                # msg: x part scaled in place; denominator via per-block reduce
                nc.vector.tensor_mul(g[:, 0:Gr, half:half + 64], g[:, 0:Gr, half:half + 64],
                                     p16[:, 0:Gr].to_broadcast([128, Gr, 64]))
                o = 0
                for b in blist:
                    Rb = int(R[b])
                    pred = prp.tile([128, 64], F32, tag="pred")
                    for j in range(o, o + Rb):
                        nc.tensor.matmul(pred[:, 0:64], id16[:], g[:, j, half:half + 64],
                                         start=(j == o), stop=(j == o + Rb - 1))
                    rec = sp.tile([128, 1], F32, tag="rec")
                    nc.vector.tensor_reduce(rec[:], p16[:, o:o + Rb], mybir.AxisListType.XY, OP.add)
                    o += Rb
                    nc.vector.tensor_scalar(rec[:], rec[:], 1e-16, None, OP.add)
                    nc.vector.reciprocal(rec[:], rec[:])
                    hnm = sp.tile([128, 64], F16, tag="hnm")
                    nc.scalar.activation(hnm[:], pred[:, 0:64], AF.Copy, scale=rec[:])
                    ps = transp16(hnm[:], 128, 64)
                    nc.scalar.activation(hdst[0:64, b * 128:(b + 1) * 128], ps[0:64, 0:128], AF.Copy)
                if window_cb is not None:
                    window_cb(blist[-1] + 1)

        def transp16(in_ap, a, bdim):
            """Transpose [a, bdim] f16 SBUF -> PSUM [bdim, a]."""
            ps = rp.tile([128, 128], F16, tag="tp16")
            nc.tensor.transpose(ps[0:bdim, 0:a], in_ap, id16[0:a, 0:a])
            return ps

        def build_rows(cin_t, width, x16, lhsT_A, lhsT_B, dst_rep, dvals, dvx=None):
            """Per block: rows[:, 0:64] = (A @ x_b).T (A None -> x_b.T);
            cols 64:128 = (B @ x_b).T (B 'plain' -> x_b.T). dst_rep -> dvals."""
            cview = cin_t[:].rearrange("(b p) e -> b p e", p=128)
            for b in range(NBLK):
                build_rows_block(cview, width, x16, b, lhsT_A, lhsT_B, dst_rep, dvals, dvx)

        def build_rows_block(cview, width, x16, b, lhsT_A, lhsT_B, dst_rep, dvals, dvx=None):
            rows_b = sp.tile([128, 128], F16, tag="rb")
            xb = x16[:, b * 128:(b + 1) * 128]
            if lhsT_A is not None:
                psm = pp.tile([128, CH], F32, tag="mmq")
                nc.tensor.matmul(psm[0:64, 0:128], lhsT_A, xb, start=True, stop=True)
                xa = sp.tile([64, 128], F16, tag="xa")
                nc.scalar.activation(xa[:], psm[0:64, 0:128], AF.Copy)
                ps = transp16(xa[:], 64, 128)
            else:
                ps = transp16(xb, 64, 128)
            nc.scalar.activation(rows_b[:, 0:64], ps[0:128, 0:64], AF.Copy)
            if lhsT_B is not None:
                if isinstance(lhsT_B, str):
                    ps2 = transp16(xb, 64, 128)
                else:
                    psm2 = pp.tile([128, CH], F32, tag="mmq")
                    nc.tensor.matmul(psm2[0:64, 0:128], lhsT_B, xb, start=True, stop=True)
                    xa2 = sp.tile([64, 128], F16, tag="xa")
                    nc.scalar.activation(xa2[:], psm2[0:64, 0:128], AF.Copy)
                    ps2 = transp16(xa2[:], 64, 128)
                nc.scalar.activation(rows_b[:, 64:128], ps2[0:128, 0:64], AF.Copy)
            if dst_rep is not None:
                m = sp.tile([128, H], F32, tag="dvm")
                nc.vector.tensor_mul(m[:], rows_b[:, 0:64], dst_rep)
                nc.vector.tensor_reduce(dvals[:, b:b + 1], m[:], mybir.AxisListType.X, OP.add)
                Rb = int(R[b])
                nc.scalar.activation(dvx[:, bases[b]:bases[b] + Rb],
                                     msk_s[:, bases[b]:bases[b] + Rb], AF.Identity,
                                     scale=0.0, bias=dvals[:, b:b + 1])
            nc.sync.dma_start(cview[b], rows_b[:, 0:width])

        def make_window_cb(xcur_t, elu_bias, elu_pre, widx, xsep_cur_t, xsep_n_t,
                           xnew_t, cin_t, width, lA, lB, drep, dv_t, dvx_t=None):
            """After edge-phase blocks land in xcur_t[0:64], run the node phase
            (elu+gru) and next layer's row build for completed 512-col windows."""
            cview = cin_t[:].rearrange("(b p) e -> b p e", p=128)
            br, bz = gru_bias(widx)
            state = {"done": 0}

            def cb(done):
                while state["done"] + 4 <= done or (done >= NBLK and state["done"] < NBLK):
                    w0 = state["done"]
                    w1 = min(w0 + 4, NBLK)
                    c0 = w0 * 128
                    w = (w1 - w0) * 128
                    elu_chunk(xcur_t, elu_bias, c0, w, pre_lhsT=elu_pre)
                    gru_chunk(xcur_t, xsep_cur_t[:], widx, xsep_n_t[:], c0, w, br, bz)
                    if xnew_t is not None:
                        nc.sync.dma_start(xnew_t[64:128, c0:c0 + w], xsep_n_t[:, c0:c0 + w])
                    for b in range(w0, w1):
                        build_rows_block(cview, width, xsep_n_t[:], b, lA, lB, drep, dv_t, dvx_t)
                    state["done"] = w1
            return cb

        # ================== forward ==================
        stack0 = stkp.tile([128, NPAD], F16, tag="stk")
        xsep0 = xsp.tile([64, NPAD], F16, tag="xsep")
        dv0 = big.tile([128, NBLK, 1], F32, tag="rvals0")
        dvx0 = big.tile([128, NCH, 1], F32, tag="rvx0")
        cview0 = cins[0][:].rearrange("(b p) e -> b p e", p=128)
        for c0 in range(0, NPAD, CH):
            w = min(CH, NPAD - c0)
            xst = sp.tile([64, CH], F32, tag="nn")
            nc.sync.dma_start(xst[:, 0:w], xT_in[:, c0:c0 + w])
            fm_mm(xsep0[:, c0:c0 + w], lin1T_s[:], xst[:, 0:w], w, func=AF.Lrelu, bias=lin1b_s[:])
            nc.sync.dma_start(stack0[64:128, c0:c0 + w], xsep0[:, c0:c0 + w])
            for b in range(c0 // 128, min((c0 + w) // 128, NBLK)):
                build_rows_block(cview0, 128, xsep0[:], b, None, w1aT_s[:], attr_s[:], dv0, dvx0)
        nc.gpsimd.collective_compute("AllGather", OP.bypass, ins=[cins[0][:]],
                                     outs=[couts[0][:]], replica_groups=[list(range(NCORE))])
        stack1 = stkp.tile([128, NPAD], F16, tag="stk")
        xsep1 = xsp.tile([64, NPAD], F16, tag="xsep")
        dv1 = big.tile([128, NBLK, 1], F32, tag="rvals1")
        dvx1 = big.tile([128, NCH, 1], F32, tag="rvx1")
        cb0 = make_window_cb(stack0, gateb_s[:], g2T_s[:], 0, xsep0, xsep1,
                             stack1, cins[1], 64, atomT_s[:, 0], None, adst_s[:, 0], dv1, dvx1)
        edge_phase(0, couts[0], dvx0, stack0, window_cb=cb0)
        nc.gpsimd.collective_compute("AllGather", OP.bypass, ins=[cins[1][:]],
                                     outs=[couts[1][:]], replica_groups=[list(range(NCORE))])
        stack2 = stkp.tile([128, NPAD], F16, tag="stk")
        xsep2 = xsp.tile([64, NPAD], F16, tag="xsep")
        dv2 = big.tile([128, NBLK, 1], F32, tag="rvals2")
        dvx2 = big.tile([128, NCH, 1], F32, tag="rvx2")
        cb1 = make_window_cb(stack1, atomb_s[:, 0:1], None, 1, xsep1, xsep2,
                             stack2, cins[2], 64, atomT_s[:, 1], None, adst_s[:, 1], dv2, dvx2)
        edge_phase(1, couts[1], dvx1, stack1, window_cb=cb1)
        nc.gpsimd.collective_compute("AllGather", OP.bypass, ins=[cins[2][:]],
                                     outs=[couts[2][:]], replica_groups=[list(range(NCORE))])
        xfin = xsp.tile([64, NPAD], F16, tag="xsep")
        cb2 = make_window_cb(stack2, atomb_s[:, 1:2], None, 2, xsep2, xfin,
                             None, cinr, 128, molT_s[:], "plain", None, None)
        edge_phase(2, couts[2], dvx2, stack2, window_cb=cb2)
        nc.gpsimd.collective_compute("AllGather", OP.bypass, ins=[cinr[:]],
                                     outs=[coutr[:]], replica_groups=[list(range(NCORE))])
        # ---- readout: gather rows, resident rrow [xs | x], on-the-fly a_src
        rrow = big.tile([128, GCH, 128], F16, tag="rrow")
        asrc_ro = big.tile([128, GCH, 1], F32, tag="asr")
        RH = 8
        for b in range(GBLK):
            RGb = int(RG[b])
            gb0 = gbases[b]
            for r0 in range(0, RGb, RH):
                cr = min(RH, RGb - r0)
                c0 = gb0 + r0
                gr = grp_.tile([128, RH, 256], F16, tag="gr")
                nc.gpsimd.dma_gather(
                    gr[:, 0:cr], coutr[:], rgidx_s[:, c0 * 8:(c0 + cr) * 8],
                    cr * 128, cr * 128, 256, elem_step=256,
                    single_packet=False, queue_num=qn_state[0] % 4)
                qn_state[0] += 1
                sl = rrow[:, c0:c0 + cr]
                nc.vector.tensor_sub(sl, gr[:, 0:cr, 128:256], gr[:, 0:cr, 0:128])
                nc.vector.tensor_mul(sl, sl, rpar_s[:, c0:c0 + cr].to_broadcast([128, cr, 128]))
                nc.vector.tensor_add(sl, sl, gr[:, 0:cr, 0:128])
                nc.vector.tensor_mul(rrow[:, c0:c0 + cr, 64:128], rrow[:, c0:c0 + cr, 64:128],
                                     rmsk16_s[:, c0:c0 + cr].to_broadcast([128, cr, 64]))
                evr = gr[:, 0:cr, 0:64]  # dead (g0) half reused as scratch
                nc.vector.tensor_mul(evr, rrow[:, c0:c0 + cr, 0:64],
                                     msrc_s[:].to_broadcast([128, cr, 64]))
                nc.vector.tensor_reduce(asrc_ro[:, c0:c0 + cr], evr,
                                        mybir.AxisListType.X, OP.add)
        ofm = big.tile([64, GPAD], F32, tag="ofm")
        hro = big.tile([64, GPAD], F32, tag="hro")
        mol_stk = big.tile([128, GPAD], F16, tag="mstk")
        for b in range(GBLK):
            RGb = int(RG[b])
            pred = prp.tile([128, 64], F32, tag="pred")
            for j in range(RGb):
                nc.tensor.matmul(pred[:, 0:64], id16[:], rrow[:, gbases[b] + j, 64:128],
                                 start=(j == 0), stop=(j == RGb - 1))
            s0 = sp.tile([128, 64], F16, tag="hnm")
            nc.scalar.activation(s0[:], pred[:, 0:64], AF.Copy)
            ps = transp16(s0[:], 128, 64)
            nc.scalar.activation(ofm[:, b * 128:(b + 1) * 128], ps[0:64, 0:128], AF.Relu)
        wtil_ps = pp.tile([128, CH], F32, tag="mmq")
        nc.tensor.matmul(wtil_ps[0:64, 0:1], mol_lin_s[:], mdst_s[:], start=True, stop=True)
        wtil = cpool.tile([64, 1], F32)
        nc.vector.tensor_copy(wtil[:], wtil_ps[0:64, 0:1])
        rmsg = big.tile([128, GCH, 64], F16, tag="rmsg")
        for t in range(3):
            ddp = pp.tile([128, CH], F32, tag="mmq")
            nc.tensor.matmul(ddp[0:1, 0:GPAD], wtil[:], ofm[:], start=True, stop=True)
            dds = s1.tile([1, GPAD], F16, tag="dds")
            nc.scalar.activation(dds[:], ddp[0:1, 0:GPAD], AF.Copy)
            lgr = sp.tile([128, GCH, 1], F32, tag="lgr")
            for b in range(GBLK):
                RGb = int(RG[b])
                gb0 = gbases[b]
                psb = transp16(dds[:, b * 128:(b + 1) * 128], 1, 128)
                ddb = sp.tile([128, 1], F32, tag="ddb")
                nc.scalar.activation(ddb[:], psb[0:128, 0:1], AF.Copy)
                nc.vector.tensor_scalar(lgr[:, gb0:gb0 + RGb], asrc_ro[:, gb0:gb0 + RGb],
                                        ddb[:, 0:1], None, OP.add)
            nc.vector.scalar_tensor_tensor(lgr[:], lgr[:], NS, lgr[:], OP.mult, OP.max)
            nc.scalar.activation(lgr[:], lgr[:], AF.Exp)
            p16r = sp.tile([128, GCH, 1], F32, tag="p16r")
            nc.vector.tensor_mul(p16r[:], lgr[:], rmsk_s[:])
            nc.vector.tensor_mul(rmsg[:, :, 0:64], rrow[:, :, 0:64],
                                 p16r[:].to_broadcast([128, GCH, 64]))
            for b in range(GBLK):
                RGb = int(RG[b])
                gb0 = gbases[b]
                pred = prp.tile([128, 64], F32, tag="pred")
                for j in range(RGb):
                    nc.tensor.matmul(pred[:, 0:64], id16[:], rmsg[:, gb0 + j, 0:64],
                                     start=(j == 0), stop=(j == RGb - 1))
                rec = sp.tile([128, 1], F32, tag="rec")
                nc.vector.tensor_reduce(rec[:], p16r[:, gb0:gb0 + RGb], mybir.AxisListType.XY, OP.add)
                nc.vector.tensor_scalar(rec[:], rec[:], 1e-16, None, OP.add)
                nc.vector.reciprocal(rec[:], rec[:])
                hnm = sp.tile([128, 64], F16, tag="hnm")
                nc.scalar.activation(hnm[:], pred[:, 0:64], AF.Copy, scale=rec[:])
                ps = transp16(hnm[:], 128, 64)
                nc.scalar.activation(hro[:, b * 128:(b + 1) * 128], ps[0:64, 0:128], AF.Copy)
            elu_inplace(hro, molb_s[:], GPAD)
            nc.scalar.activation(mol_stk[0:64, :], hro[:], AF.Copy)
            nc.gpsimd.dma_start(mol_stk[64:128, :], ofm[:])
            onew = s1.tile([64, GPAD], F32, tag="onew")
            gru(mol_stk, ofm, 3, onew[:], GPAD)
            nc.scalar.activation(ofm[:], onew[:], AF.Copy)
        emb = s1.tile([64, GPAD], F16, tag="emb")
        fm_mm(emb[:], lin2T_s[:], ofm[:], GPAD, bias=lin2b_s[:])
        nemb = s1.tile([64, GPAD], F32, tag="nemb")
        for b in range(GBLK):
            gps = transp16(emb[:, b * 128:(b + 1) * 128], 64, 128)
            gm = sp.tile([128, 64], F32, tag="gm")
            nc.scalar.activation(gm[:], gps[0:128, 0:64], AF.Copy)
            mu = sp.tile([128, 1], F32, tag="mu")
            nc.vector.tensor_reduce(mu[:], gm[:, 0:64], mybir.AxisListType.X, OP.add)
            nc.vector.tensor_scalar(mu[:], mu[:], 1.0 / 64, None, OP.mult)
            xc = sp.tile([128, 64], F32, tag="xc")
            nc.vector.tensor_scalar(xc[:], gm[:, 0:64], mu[:], None, OP.subtract)
            sq = sp.tile([128, 64], F32, tag="sq")
            nc.scalar.activation(sq[:], xc[:], AF.Square)
            var = sp.tile([128, 1], F32, tag="var")
            nc.vector.tensor_reduce(var[:], sq[:], mybir.AxisListType.X, OP.add)
            nc.vector.tensor_scalar(var[:], var[:], 1.0 / 64, None, OP.mult)
            nc.vector.tensor_scalar(var[:], var[:], 1e-5, None, OP.add)
            nc.scalar.activation(var[:], var[:], AF.Sqrt)
            nc.vector.reciprocal(var[:], var[:])
            nc.scalar.activation(xc[:], xc[:], AF.Copy, scale=var[:])
            nc.vector.tensor_mul(xc[:], xc[:], lng_s[:, 0:64])
            nc.vector.tensor_add(xc[:], xc[:], lnb_s[:, 0:64])
            xc16 = sp.tile([128, 64], F16, tag="hnm")
            nc.scalar.activation(xc16[:], xc[:], AF.Copy)
            ps2 = transp16(xc16[:], 128, 64)
            nc.scalar.activation(nemb[:, b * 128:(b + 1) * 128], ps2[0:64, 0:128], AF.Copy)
        m1 = s1.tile([64, GPAD], F32, tag="m1")
        fm_mm(m1[:], h1T_s[:], nemb[:], GPAD, func=AF.Relu, bias=h1b_s[:])
        m2 = s1.tile([64, GPAD], F32, tag="m2")
        fm_mm(m2[:], h2T_s[:], m1[:], GPAD, func=AF.Relu, bias=h2b_s[:])
        yps = pp.tile([128, CH], F32, tag="mmq")
        nc.tensor.matmul(yps[0:1, 0:GPAD], h3T_s[:], m2[:], start=True, stop=True)
        ysb = s1.tile([1, GPAD], F32, tag="ysb")
        nc.scalar.activation(ysb[:], yps[0:1, 0:GPAD], AF.Identity, bias=h3b_s[:])
        nc.sync.dma_start(y_out[:], ysb[:])
        ctx.close()
    ctx2.__exit__(None, None, None)
    nc.finalize()
    return nc


_CACHE = {}


def kernel(**inputs):
    x = np.asarray(inputs["x"], np.float32)
    edge_attr = np.asarray(inputs["edge_attr"], np.float32)
    ei = np.asarray(inputs["edge_index"])
    batch = np.asarray(inputs["batch"])
    if "k" not in _CACHE:
        plan = build_plan(ei, batch)
        nc = build_nc(plan)
        _CACHE["k"] = (plan, nc)
    plan, nc = _CACHE["k"]

    gw = np.zeros((128, 4, 2, H), np.float16)
    gn = np.zeros((128, 4, 2, H), np.float16)
    gbx_rz = np.zeros((64, 8), np.float32)
    gbh_rz = np.zeros((64, 8), np.float32)
    gbx_n = np.zeros((H, 4), np.float32)
    gbh_n = np.zeros((H, 4), np.float32)
    packs = [
        (inputs["gru0_wx"], inputs["gru0_wh"], inputs["gru0_bx"], inputs["gru0_bh"]),
        (inputs["atom_gru_wx"][0], inputs["atom_gru_wh"][0], inputs["atom_gru_bx"][0], inputs["atom_gru_bh"][0]),
        (inputs["atom_gru_wx"][1], inputs["atom_gru_wh"][1], inputs["atom_gru_bx"][1], inputs["atom_gru_bh"][1]),
        (inputs["mol_gru_wx"], inputs["mol_gru_wh"], inputs["mol_gru_bx"], inputs["mol_gru_bh"]),
    ]
    for i, (wx, wh, bx, bh) in enumerate(packs):
        wx = np.asarray(wx, np.float32); wh = np.asarray(wh, np.float32)
        bx = np.asarray(bx, np.float32); bh = np.asarray(bh, np.float32)
        gw[0:64, i, 0] = wx[0:64].T; gw[64:128, i, 0] = wh[0:64].T
        gw[0:64, i, 1] = wx[64:128].T; gw[64:128, i, 1] = wh[64:128].T
        gn[0:64, i, 0] = wx[128:192].T; gn[64:128, i, 1] = wh[128:192].T
        gbx_rz[:, 2 * i] = bx[0:64]; gbx_rz[:, 2 * i + 1] = bx[64:128]
        gbh_rz[:, 2 * i] = bh[0:64]; gbh_rz[:, 2 * i + 1] = bh[64:128]
        gbx_n[:, i] = bx[128:192]; gbh_n[:, i] = bh[128:192]

    glw = np.asarray(inputs["gate_lin1_w"], np.float32)
    rep16 = lambda v: np.tile(np.asarray(v, np.float16).reshape(1, -1), (128, 1))
    rep32 = lambda v: np.tile(np.asarray(v, np.float32).reshape(1, -1), (128, 1))
    a = lambda k: np.asarray(inputs[k], np.float32)
    a16 = lambda k: np.asarray(inputs[k], np.float16)
    wts = dict(
        lin1T=a("lin1_w").T.copy(), lin1_b=a("lin1_b").reshape(H, 1),
        w1aT=glw[:, 0:64].T.astype(np.float16).copy(),
        w1bT=glw[:, 64:80].T.astype(np.float16).copy(),
        attl_rep=rep16(inputs["gate_att_l"]).reshape(128, 1, H),
        attr_rep=rep16(inputs["gate_att_r"]),
        g2T=a16("gate_lin2_w").T.copy(), gate_b=a("gate_bias").reshape(H, 1),
        atomT=np.stack([a16("atom_lin_w")[l].T for l in range(2)], 1),
        asrc_rep=np.stack([rep16(inputs["atom_att_src"][l]) for l in range(2)], 1),
        adst_rep=np.stack([rep16(inputs["atom_att_dst"][l]) for l in range(2)], 1),
        atom_b=a("atom_bias").T.copy(),
        molT=a16("mol_lin_w").T.copy(), mol_lin=a("mol_lin_w").copy(),
        matt_src_rep=rep16(inputs["mol_att_src"]).reshape(128, 1, H),
        matt_dst=a("mol_att_dst").reshape(H, 1),
        mol_b=a("mol_bias").reshape(H, 1),
        gruW=gw, gruN=gn, gbx_rz=gbx_rz, gbh_rz=gbh_rz, gbx_n=gbx_n, gbh_n=gbh_n,
        lin2T=a("lin2_w").T.copy(), lin2_b=a("lin2_b").reshape(H, 1),
        lng_rep=rep32(inputs["ln_g"]), lnb_rep=rep32(inputs["ln_b"]),
        h1T=a("h1_w").T.copy(), h1_b=a("h1_b").reshape(H, 1),
        h2T=a("h2_w").T.copy(), h2_b=a("h2_b").reshape(H, 1),
        h3T=a("h3_w").T.copy(), h3_b=a("h3_b").reshape(1, 1),
    )
    in_maps = []
    for c in range(NCORE):
        pc = plan["cores"][c]
        rc = plan["rcores"][c]
        xT = np.zeros((H, NPAD), np.float32)
        xT[:, 0:NPC] = x[c * NPC:(c + 1) * NPC][plan["perms"][c]].T
        eaT = np.zeros((16, plan["S"]), np.float16)
        val = pc["esel"] >= 0
        eaT[:, val] = edge_attr[pc["esel"][val]].T.astype(np.float16)
        im = dict(xT_in=xT, gidx=pc["gidx"], par=pc["par"], msk=pc["msk"], eaT=eaT,
                  rgidx=rc["rgidx"], rpar=rc["rpar"], rmsk=rc["rmsk"],
                  rmsk16=rc["rmsk16"], **wts)
        in_maps.append(im)
    res = run_bass_kernel_spmd(nc, in_maps, core_ids=list(range(NCORE)))
    if getattr(res, "exec_time_ns", None):
        print(f"HW exec time: {res.exec_time_ns} ns", flush=True)
    y = np.zeros(B, np.float32)
    for c in range(NCORE):
        ys = res.results[c]["y_out"].reshape(GPAD)[0:GPC]
        y[c * GPC + plan["gperms"][c]] = ys
    return y.reshape(B, 1)


# revision 33
# speedup vs baseline: 1.3077x; 1.2135x over previous
"""AttentiveFP forward on 8 Trainium2 NeuronCores (Bass/Tile).

Edges sharded by dst-owner core; per-core nodes sorted by in-degree with a
round-robin slot structure so segment softmax/sum become dense PSUM matmul
accumulation. Edge phase processed in large multi-block groups: one
dma_gather per ~34 rounds, wide DVE ops (parity select in place), DVE
leaky-relu (no ACT table ping-pong), single Exp per group. Layers 1-2 use
256B gather elements (64-f16 rows, att_src dot computed on the fly), halving
gather traffic and AllGather payload. fp16 node state feeds all PE matmuls.
"""
import numpy as np
from contextlib import ExitStack

import concourse.bass as bass
import concourse.tile as tile
from concourse import bacc, mybir
from concourse.bass_utils import run_bass_kernel_spmd
from concourse.masks import make_identity

F32 = mybir.dt.float32
F16 = mybir.dt.float16
I16 = mybir.dt.int16
AF = mybir.ActivationFunctionType
OP = mybir.AluOpType

NCORE = 8
N, E, B = 50000, 500000, 2048
H = 64
NS = 0.01
NPC = N // NCORE
GPC = B // NCORE
NBLK = (NPC + 127) // 128
NPAD = NBLK * 128
GBLK = (GPC + 127) // 128
GPAD = GBLK * 128
TROWS = NCORE * NPAD
MAXR = 10
CH = 512
GT = 16       # max rounds per edge-phase group


def _calls_for(R, maxr=MAXR):
    calls, base, bases = [], 0, []
    for r in R:
        bases.append(base)
        calls.append([(r0, min(r0 + maxr, int(r))) for r0 in range(0, int(r), maxr)])
        base += int(r)
    return calls, bases, base


def _wrap_into(gidx, arr, col0):
    n = arr.shape[0]
    blk = arr.reshape(n // 16, 16).T
    gidx[:16, col0:col0 + n // 16] = blk
    gidx[16:128, col0:col0 + n // 16] = np.tile(blk, (7, 1))


def build_plan(edge_index, batch):
    src = edge_index[0].astype(np.int64)
    dst = edge_index[1].astype(np.int64)
    owner = dst // NPC

    perms, degs_sorted, grp_starts, egrp = [], [], [], []
    sortpos = np.zeros(N, np.int64)
    for c in range(NCORE):
        n0 = c * NPC
        emask = np.nonzero(owner == c)[0]
        deg = np.bincount(dst[emask] - n0, minlength=NPC)
        order = np.argsort(-deg, kind="stable")
        perms.append(order)
        sortpos[n0 + order] = np.arange(NPC)
        dsorted = deg[order]
        degs_sorted.append(dsorted)
        eorder = np.argsort(sortpos[dst[emask]], kind="stable")
        egrp.append(emask[eorder])
        grp_starts.append(np.concatenate([[0], np.cumsum(dsorted)]))
    trow = (np.arange(N) // NPC) * NPAD + sortpos

    R = np.ones(NBLK, np.int64)
    for b in range(NBLK):
        for c in range(NCORE):
            d = degs_sorted[c][b * 128:(b + 1) * 128]
            if len(d):
                R[b] = max(R[b], int(d[0]))
    calls, bases, NCH = _calls_for(R)
    S = NCH * 128

    gsize = np.bincount(batch, minlength=B)
    gstart = np.concatenate([[0], np.cumsum(gsize)])
    gperms, gss = [], []
    for c in range(NCORE):
        gs = gsize[c * GPC:(c + 1) * GPC]
        gorder = np.argsort(-gs, kind="stable")
        gperms.append(gorder)
        gss.append(gs[gorder])
    RG = np.ones(GBLK, np.int64)
    for b in range(GBLK):
        for c in range(NCORE):
            d = gss[c][b * 128:(b + 1) * 128]
            if len(d):
                RG[b] = max(RG[b], int(d[0]))
    gcalls, gbases, GCH = _calls_for(RG, 8)
    SR = GCH * 128

    cores = []
    lanes = np.arange(128)
    for c in range(NCORE):
        gidx = np.zeros((128, S // 16), np.int16)
        par = np.zeros((128, NCH, 1), np.float16)
        msk = np.zeros((128, NCH, 1), np.float32)
        esel = np.full(NCH * 128, -1, np.int64)
        ds = degs_sorted[c]
        gst = grp_starts[c]
        eg = egrp[c]
        for b in range(NBLK):
            for (r0, r1) in calls[b]:
                ia = np.zeros((r1 - r0) * 128, np.int64)
                for r in range(r0, r1):
                    ch = bases[b] + r
                    p = b * 128 + lanes
                    pc = np.minimum(p, NPC - 1)
                    ok = (p < NPC) & (r < ds[pc])
                    eids = gst[pc] + r
                    ge = np.where(ok, eg[np.where(ok, np.minimum(eids, len(eg) - 1), 0)], -1)
                    esel[ch * 128 + lanes] = ge
                    rows = np.where(ok, trow[src[np.maximum(ge, 0)]], 0)
                    ia[(r - r0) * 128 + lanes] = rows >> 1
                    par[:, ch, 0] = (rows & 1).astype(np.float16)
                    msk[:, ch, 0] = ok.astype(np.float32)
                _wrap_into(gidx, ia, (bases[b] + r0) * 8)
        cores.append(dict(gidx=gidx, par=par, msk=msk, esel=esel))

    rcores = []
    for c in range(NCORE):
        rgidx = np.zeros((128, SR // 16), np.int16)
        rpar = np.zeros((128, GCH, 1), np.float16)
        rmsk = np.zeros((128, GCH, 1), np.float32)
        rmsk16 = np.zeros((128, GCH, 1), np.float16)
        gs = gss[c]
        gp = gperms[c]
        for b in range(GBLK):
            for (r0, r1) in gcalls[b]:
                ia = np.zeros((r1 - r0) * 128, np.int64)
                for r in range(r0, r1):
                    ch = gbases[b] + r
                    p = b * 128 + lanes
                    pc = np.minimum(p, GPC - 1)
                    ok = (p < GPC) & (r < gs[pc])
                    g = c * GPC + gp[pc]
                    node = np.where(ok, gstart[g] + r, 0)
                    rows = np.where(ok, trow[node], 0)
                    ia[(r - r0) * 128 + lanes] = rows >> 1
                    rpar[:, ch, 0] = (rows & 1).astype(np.float16)
                    rmsk[:, ch, 0] = ok.astype(np.float32)
                    rmsk16[:, ch, 0] = ok.astype(np.float16)
                _wrap_into(rgidx, ia, (gbases[b] + r0) * 8)
        rcores.append(dict(rgidx=rgidx, rpar=rpar, rmsk=rmsk, rmsk16=rmsk16))

    return dict(R=R, bases=bases, NCH=NCH, S=S,
                RG=RG, gbases=gbases, GCH=GCH, SR=SR,
                cores=cores, rcores=rcores, perms=perms, gperms=gperms)


def build_nc(plan):
    R, bases, NCH, S = plan["R"], plan["bases"], plan["NCH"], plan["S"]
    RG, gbases, GCH, SR = plan["RG"], plan["gbases"], plan["GCH"], plan["SR"]
    RGT = int(max(int(r) for r in RG))

    # split blocks into spans of <= GT rounds, pack spans into groups
    spans = []
    for b in range(NBLK):
        r = int(R[b])
        for s0 in range(0, r, GT):
            spans.append((b, s0, min(s0 + GT, r)))
    def pack_span_groups(cap):
        gs, cur, cur_r = [], [], 0
        for spn in spans:
            ln = spn[2] - spn[1]
            if cur and cur_r + ln > cap:
                gs.append(cur)
                cur, cur_r = [], 0
            cur.append(spn)
            cur_r += ln
        gs.append(cur)
        return gs
    sgroups = pack_span_groups(GT)
    GTS = max(sum(s[2] - s[1] for s in gl) for gl in sgroups)

    nc = bacc.Bacc("TRN2", target_bir_lowering=False, debug=False,
                   num_devices=NCORE, num_swdge_queues=4)

    def din(name, shape, dt=F32):
        return nc.dram_tensor(name, shape, dt, kind="ExternalInput")

    xT_in = din("xT_in", [H, NPAD])
    gidx_in = din("gidx", [128, S // 16], I16)
    par_in = din("par", [128, NCH, 1], F16)
    msk_in = din("msk", [128, NCH, 1], F32)
    eaT_in = din("eaT", [16, S], F16)
    rgidx_in = din("rgidx", [128, SR // 16], I16)
    rpar_in = din("rpar", [128, GCH, 1], F16)
    rmsk_in = din("rmsk", [128, GCH, 1], F32)
    rmsk16_in = din("rmsk16", [128, GCH, 1], F16)
    lin1T = din("lin1T", [H, H]); lin1_b = din("lin1_b", [H, 1])
    w1aT = din("w1aT", [H, H], F16); w1bT = din("w1bT", [16, H], F16)
    attl_rep = din("attl_rep", [128, 1, H], F16)
    attr_rep = din("attr_rep", [128, H], F16)
    g2T = din("g2T", [H, H], F16); gate_b = din("gate_b", [H, 1])
    atomT = din("atomT", [H, 2, H], F16)
    asrc_rep = din("asrc_rep", [128, 2, H], F16)
    adst_rep = din("adst_rep", [128, 2, H], F16)
    atom_b = din("atom_b", [H, 2])
    molT = din("molT", [H, H], F16)
    mol_lin = din("mol_lin", [H, H])
    matt_src_rep = din("matt_src_rep", [128, 1, H], F16)
    matt_dst = din("matt_dst", [H, 1])
    mol_b = din("mol_b", [H, 1])
    gruW = din("gruW", [128, 4, 2, H], F16)   # [K, widx, gate(r/z), M]
    gruN = din("gruN", [128, 4, 2, H], F16)   # [K, widx, (nx/nh), M] zero-padded
    gbx_rz = din("gbx_rz", [64, 8])
    gbh_rz = din("gbh_rz", [64, 8])
    gbx_n = din("gbx_n", [H, 4])
    gbh_n = din("gbh_n", [H, 4])
    lin2T = din("lin2T", [H, H]); lin2_b = din("lin2_b", [H, 1])
    lng_rep = din("lng_rep", [128, H]); lnb_rep = din("lnb_rep", [128, H])
    h1T = din("h1T", [H, H]); h1_b = din("h1_b", [H, 1])
    h2T = din("h2T", [H, H]); h2_b = din("h2_b", [H, 1])
    h3T = din("h3T", [H, 1]); h3_b = din("h3_b", [1, 1])

    y_out = nc.dram_tensor("y_out", [1, GPAD], F32, kind="ExternalOutput")

    cin0 = nc.dram_tensor("cin0", [NPAD, 128], F16)
    cout0 = nc.dram_tensor("cout0", [TROWS // 2, 256], F16, addr_space="Shared")
    cins = [cin0]
    couts = [cout0]
    for l in (1, 2):
        cins.append(nc.dram_tensor(f"cin{l}", [NPAD, 64], F16))
        couts.append(nc.dram_tensor(f"cout{l}", [TROWS // 2, 128], F16,
                                    addr_space="Shared"))
    cinr = nc.dram_tensor("cinr", [NPAD, 128], F16)
    coutr = nc.dram_tensor("coutr", [TROWS // 2, 256], F16, addr_space="Shared")

    ctx = ExitStack()
    ctx2 = nc.allow_low_precision(reason="fp16 edge tables/messages/state")
    ctx2.__enter__()
    with tile.TileContext(nc) as tc:
        cpool = ctx.enter_context(tc.tile_pool(name="const", bufs=1))
        wpool = ctx.enter_context(tc.tile_pool(name="wts", bufs=1))
        big = ctx.enter_context(tc.tile_pool(name="big", bufs=1))
        stkp = ctx.enter_context(tc.tile_pool(name="stkp", bufs=2))
        xsp = ctx.enter_context(tc.tile_pool(name="xsp", bufs=2))
        g0p = ctx.enter_context(tc.tile_pool(name="g0p", bufs=4))
        g12p = ctx.enter_context(tc.tile_pool(name="g12p", bufs=5))
        grp_ = ctx.enter_context(tc.tile_pool(name="gathr", bufs=1))
        eap = ctx.enter_context(tc.tile_pool(name="eap", bufs=1))
        sp = ctx.enter_context(tc.tile_pool(name="scr", bufs=2))
        s1 = ctx.enter_context(tc.tile_pool(name="scr1", bufs=1))
        pp = ctx.enter_context(tc.tile_pool(name="ps", bufs=2, space="PSUM"))
        prp = ctx.enter_context(tc.tile_pool(name="psp", bufs=2, space="PSUM"))
        rp = ctx.enter_context(tc.tile_pool(name="psr", bufs=2, space="PSUM"))
        zp = ctx.enter_context(tc.tile_pool(name="psz", bufs=2, space="PSUM"))

        id32 = cpool.tile([128, 128], F32)
        make_identity(nc, id32[:])
        id16 = cpool.tile([128, 128], F16)
        nc.vector.tensor_copy(id16[:], id32[:])

        def load(t, shape, dt=F32):
            s = wpool.tile(shape, dt, tag=f"w_{t.name}")
            nc.sync.dma_start(s[:], t[:])
            return s

        gidx_s = load(gidx_in, [128, S // 16], I16)
        rgidx_s = load(rgidx_in, [128, SR // 16], I16)
        par_s = load(par_in, [128, NCH, 1], F16)
        msk_s = load(msk_in, [128, NCH, 1], F32)
        rpar_s = load(rpar_in, [128, GCH, 1], F16)
        rmsk_s = load(rmsk_in, [128, GCH, 1], F32)
        rmsk16_s = load(rmsk16_in, [128, GCH, 1], F16)
        lin1T_s = load(lin1T, [H, H]); lin1b_s = load(lin1_b, [H, 1])
        w1aT_s = load(w1aT, [H, H], F16); w1bT_s = load(w1bT, [16, H], F16)
        attl_s = load(attl_rep, [128, 1, H], F16)
        attr_s = load(attr_rep, [128, H], F16)
        g2T_s = load(g2T, [H, H], F16); gateb_s = load(gate_b, [H, 1])
        atomT_s = load(atomT, [H, 2, H], F16)
        asrc_s = load(asrc_rep, [128, 2, H], F16)
        adst_s = load(adst_rep, [128, 2, H], F16)
        atomb_s = load(atom_b, [H, 2])
        molT_s = load(molT, [H, H], F16); mol_lin_s = load(mol_lin, [H, H])
        msrc_s = load(matt_src_rep, [128, 1, H], F16)
        mdst_s = load(matt_dst, [H, 1])
        molb_s = load(mol_b, [H, 1])
        gruW_s = load(gruW, [128, 4, 2, H], F16)
        gruN_s = load(gruN, [128, 4, 2, H], F16)
        gbxrz_s = load(gbx_rz, [64, 8])
        gbhrz_s = load(gbh_rz, [64, 8])
        gbxn_s = load(gbx_n, [H, 4])
        gbhn_s = load(gbh_n, [H, 4])
        lin2T_s = load(lin2T, [H, H]); lin2b_s = load(lin2_b, [H, 1])
        lng_s = load(lng_rep, [128, H]); lnb_s = load(lnb_rep, [128, H])
        h1T_s = load(h1T, [H, H]); h1b_s = load(h1_b, [H, 1])
        h2T_s = load(h2T, [H, H]); h2b_s = load(h2_b, [H, 1])
        h3T_s = load(h3T, [H, 1]); h3b_s = load(h3_b, [1, 1])

        def fm_mm(out_ap, lhsT, rhs, ncols, func=None, bias=0.0):
            M = lhsT.shape[-1]
            for c0 in range(0, ncols, CH):
                w = min(CH, ncols - c0)
                ps = pp.tile([128, CH], F32, tag="mmq")
                nc.tensor.matmul(ps[0:M, :w], lhsT, rhs[:, c0:c0 + w],
                                 start=True, stop=True)
                f = func
                if f is None:
                    f = AF.Copy if isinstance(bias, float) else AF.Identity
                nc.scalar.activation(out_ap[:, c0:c0 + w], ps[0:M, :w], f, bias=bias)

        def gru_bias(widx):
            br = sp.tile([64, 1], F32, tag=f"brz{widx}")
            nc.vector.tensor_add(br[:], gbxrz_s[:, 2 * widx:2 * widx + 1], gbhrz_s[:, 2 * widx:2 * widx + 1])
            bz = sp.tile([64, 1], F32, tag=f"bz{widx}")
            nc.vector.tensor_add(bz[:], gbxrz_s[:, 2 * widx + 1:2 * widx + 2], gbhrz_s[:, 2 * widx + 1:2 * widx + 2])
            return br, bz

        def gru_chunk(stk, xprev, widx, out_ap, c0, w, br, bz):
            if True:
                prz = pp.tile([128, CH], F32, tag="mmq")
                nc.tensor.matmul(prz[0:64, :w], gruW_s[:, widx, 0], stk[:, c0:c0 + w], start=True, stop=True)
                prz2 = pp.tile([128, CH], F32, tag="mmq")
                nc.tensor.matmul(prz2[0:64, :w], gruW_s[:, widx, 1], stk[:, c0:c0 + w], start=True, stop=True)
                rz = sp.tile([64, CH], F32, tag="rz")
                nc.scalar.activation(rz[:, :w], prz[0:64, :w], AF.Sigmoid, bias=br[:])
                zz = sp.tile([64, CH], F32, tag="zz")
                nc.scalar.activation(zz[:, :w], prz2[0:64, :w], AF.Sigmoid, bias=bz[:])
                pn1 = pp.tile([128, CH], F32, tag="mmq")
                nc.tensor.matmul(pn1[0:64, :w], gruN_s[:, widx, 0], stk[:, c0:c0 + w], start=True, stop=True)
                pn2 = pp.tile([128, CH], F32, tag="mmq")
                nc.tensor.matmul(pn2[0:64, :w], gruN_s[:, widx, 1], stk[:, c0:c0 + w], start=True, stop=True)
                hnb = sp.tile([64, CH], F32, tag="hnb")
                nc.scalar.activation(hnb[:, :w], pn2[0:64, :w], AF.Identity, bias=gbhn_s[:, widx:widx + 1])
                nc.vector.tensor_mul(hnb[:, :w], rz[:, :w], hnb[:, :w])
                nc.vector.tensor_add(hnb[:, :w], hnb[:, :w], pn1[0:64, :w])
                nn = sp.tile([64, CH], F32, tag="nn")
                nc.scalar.activation(nn[:, :w], hnb[:, :w], AF.Tanh, bias=gbxn_s[:, widx:widx + 1])
                d = rz  # rz buffer is dead past the mul above
                nc.vector.tensor_sub(d[:, :w], xprev[:, c0:c0 + w], nn[:, :w])
                nc.vector.tensor_mul(d[:, :w], zz[:, :w], d[:, :w])
                nc.vector.tensor_add(d[:, :w], nn[:, :w], d[:, :w])
                nc.scalar.activation(out_ap[:, c0:c0 + w], d[:, :w], AF.Relu)

        def gru(stk, xprev, widx, out_ap, ncols):
            br, bz = gru_bias(widx)
            for c0 in range(0, ncols, CH):
                gru_chunk(stk, xprev, widx, out_ap, c0, min(CH, ncols - c0), br, bz)

        def elu_chunk(buf, bias_ap, c0, w, pre_lhsT=None, out_buf=None):
            if pre_lhsT is not None:
                ps = pp.tile([128, CH], F32, tag="mmq")
                nc.tensor.matmul(ps[0:64, :w], pre_lhsT, buf[0:64, c0:c0 + w], start=True, stop=True)
                src = ps[0:64, :w]
            else:
                src = buf[0:64, c0:c0 + w]
            e1 = sp.tile([64, CH], F32, tag="rz")
            nc.scalar.activation(e1[:, :w], src, AF.Exp, bias=bias_ap)
            t1 = sp.tile([64, CH], F32, tag="zz")
            nc.scalar.activation(t1[:, :w], e1[:, :w], AF.Relu, bias=1.0, scale=-1.0)
            t2 = sp.tile([64, CH], F32, tag="hnb")
            nc.scalar.activation(t2[:, :w], src, AF.Relu, bias=bias_ap)
            dst = buf if out_buf is None else out_buf
            nc.vector.tensor_sub(dst[0:64, c0:c0 + w], t2[:, :w], t1[:, :w])

        def elu_inplace(buf, bias_ap, ncols, pre_lhsT=None):
            """buf[0:64, :] = elu((pre_lhsT.T @ buf[0:64]) + bias)."""
            for c0 in range(0, ncols, CH):
                elu_chunk(buf, bias_ap, c0, min(CH, ncols - c0), pre_lhsT)

        qn_state = [0]

        def edge_phase(layer, table, dvals, hdst, window_cb=None):
            """hdst[0:64, :NPAD] (f16) <- normalized aggregation (feature-major)."""
            W = 256 if layer == 0 else 128
            half = W // 2
            pool = g0p if layer == 0 else g12p
            gtile = GTS
            p16L = big.tile([128, NCH, 1], F32, tag=f"p16L{layer}")
            preds = {}
            for glist in sgroups:
                ch0 = bases[glist[0][0]] + glist[0][1]
                Gr = sum(s[2] - s[1] for s in glist)
                g = pool.tile([128, gtile, W], F16, tag=f"g{min(layer, 1)}")
                nc.gpsimd.dma_gather(
                    g[:, 0:Gr], table[:], gidx_s[:, ch0 * 8:(ch0 + Gr) * 8],
                    Gr * 128, Gr * 128, W, elem_step=W,
                    single_packet=False, queue_num=qn_state[0] % 4)
                qn_state[0] += 1
                # parity select in place: row -> g[:, :, half:W]
                nc.vector.tensor_sub(g[:, 0:Gr, half:W], g[:, 0:Gr, half:W], g[:, 0:Gr, 0:half])
                nc.vector.tensor_mul(g[:, 0:Gr, half:W], g[:, 0:Gr, half:W],
                                     par_s[:, ch0:ch0 + Gr].to_broadcast([128, Gr, half]))
                nc.vector.tensor_add(g[:, 0:Gr, half:W], g[:, 0:Gr, half:W], g[:, 0:Gr, 0:half])
                lg = sp.tile([128, gtile, 1], F32, tag="lg")
                ev = g[:, 0:Gr, 0:64]  # dead (g0) half reused as scratch
                if layer == 0:
                    for r0 in range(0, Gr, 8):
                        r1 = min(r0 + 8, Gr)
                        ea = eap.tile([16, 8 * 128], F16, tag="ea")
                        nc.sync.dma_start(ea[:, 0:(r1 - r0) * 128],
                                          eaT_in[:, (ch0 + r0) * 128:(ch0 + r1) * 128])
                        pz = zp.tile([128, 8, 64], F32, tag="z1")
                        for r in range(r0, r1):
                            nc.tensor.matmul(pz[:, r - r0], ea[:, (r - r0) * 128:(r - r0 + 1) * 128],
                                             w1bT_s[:], start=True, stop=True)
                        nc.vector.tensor_add(g[:, r0:r1, 192:256], g[:, r0:r1, 192:256],
                                             pz[:, 0:r1 - r0])
                    nc.vector.tensor_scalar_mul(ev, g[:, 0:Gr, 192:256], NS)
                    nc.vector.tensor_max(g[:, 0:Gr, 192:256], g[:, 0:Gr, 192:256], ev)
                    nc.vector.tensor_mul(ev, g[:, 0:Gr, 192:256],
                                         attl_s[:].to_broadcast([128, Gr, 64]))
                else:
                    nc.vector.tensor_mul(ev, g[:, 0:Gr, 64:128],
                                         asrc_s[:, layer - 1:layer].to_broadcast([128, Gr, 64]))
                nc.vector.tensor_reduce(lg[:, 0:Gr], ev, mybir.AxisListType.X, OP.add)
                nc.vector.tensor_add(lg[:, 0:Gr], lg[:, 0:Gr], dvals[:, ch0:ch0 + Gr])
                nc.vector.scalar_tensor_tensor(lg[:, 0:Gr], lg[:, 0:Gr], NS, lg[:, 0:Gr],
                                               OP.mult, OP.max)
                nc.scalar.activation(lg[:, 0:Gr], lg[:, 0:Gr], AF.Exp)
                p16 = p16L[:, ch0:ch0 + Gr]
                nc.vector.tensor_mul(p16, lg[:, 0:Gr], msk_s[:, ch0:ch0 + Gr])
                # msg: x part scaled in place; denominator via per-block reduce
                nc.vector.tensor_mul(g[:, 0:Gr, half:half + 64], g[:, 0:Gr, half:half + 64],
                                     p16.to_broadcast([128, Gr, 64]))
                o = 0
                done_b = 0
                for (b, r0, r1) in glist:
                    Rb = int(R[b])
                    if b not in preds:
                        predt = prp.tile([128, 64], F32, tag="pred")
                        preds[b] = predt
                    pred = preds[b]
                    for j in range(r1 - r0):
                        nc.tensor.matmul(pred[:, 0:64], id16[:], g[:, o + j, half:half + 64],
                                         start=(r0 + j == 0), stop=(r0 + j == Rb - 1))
                    o += r1 - r0
                    if r1 < Rb:
                        continue
                    del preds[b]
                    done_b = b + 1
                    rec = sp.tile([128, 1], F32, tag="rec")
                    nc.vector.tensor_reduce(rec[:], p16L[:, bases[b]:bases[b] + Rb],
                                            mybir.AxisListType.XY, OP.add)
                    nc.vector.tensor_scalar(rec[:], rec[:], 1e-16, None, OP.add)
                    nc.vector.reciprocal(rec[:], rec[:])
                    hnm = sp.tile([128, 64], F16, tag="hnm")
                    nc.scalar.activation(hnm[:], pred[:, 0:64], AF.Copy, scale=rec[:])
                    ps = transp16(hnm[:], 128, 64)
                    nc.scalar.activation(hdst[0:64, b * 128:(b + 1) * 128], ps[0:64, 0:128], AF.Copy)
                if window_cb is not None and done_b:
                    window_cb(done_b)

        def transp16(in_ap, a, bdim):
            """Transpose [a, bdim] f16 SBUF -> PSUM [bdim, a]."""
            ps = rp.tile([128, 128], F16, tag="tp16")
            nc.tensor.transpose(ps[0:bdim, 0:a], in_ap, id16[0:a, 0:a])
            return ps

        def build_rows(cin_t, width, x16, lhsT_A, lhsT_B, dst_rep, dvals, dvx=None):
            """Per block: rows[:, 0:64] = (A @ x_b).T (A None -> x_b.T);
            cols 64:128 = (B @ x_b).T (B 'plain' -> x_b.T). dst_rep -> dvals."""
            cview = cin_t[:].rearrange("(b p) e -> b p e", p=128)
            for b in range(NBLK):
                build_rows_block(cview, width, x16, b, lhsT_A, lhsT_B, dst_rep, dvals, dvx)

        def build_rows_block(cview, width, x16, b, lhsT_A, lhsT_B, dst_rep, dvals, dvx=None):
            rows_b = sp.tile([128, 128], F16, tag="rb")
            xb = x16[:, b * 128:(b + 1) * 128]
            if lhsT_A is not None:
                psm = pp.tile([128, CH], F32, tag="mmq")
                nc.tensor.matmul(psm[0:64, 0:128], lhsT_A, xb, start=True, stop=True)
                xa = sp.tile([64, 128], F16, tag="xa")
                nc.scalar.activation(xa[:], psm[0:64, 0:128], AF.Copy)
                ps = transp16(xa[:], 64, 128)
            else:
                ps = transp16(xb, 64, 128)
            nc.scalar.activation(rows_b[:, 0:64], ps[0:128, 0:64], AF.Copy)
            if lhsT_B is not None:
                if isinstance(lhsT_B, str):
                    ps2 = transp16(xb, 64, 128)
                else:
                    psm2 = pp.tile([128, CH], F32, tag="mmq")
                    nc.tensor.matmul(psm2[0:64, 0:128], lhsT_B, xb, start=True, stop=True)
                    xa2 = sp.tile([64, 128], F16, tag="xa")
                    nc.scalar.activation(xa2[:], psm2[0:64, 0:128], AF.Copy)
                    ps2 = transp16(xa2[:], 64, 128)
                nc.scalar.activation(rows_b[:, 64:128], ps2[0:128, 0:64], AF.Copy)
            if dst_rep is not None:
                m = sp.tile([128, H], F32, tag="dvm")
                nc.vector.tensor_mul(m[:], rows_b[:, 0:64], dst_rep)
                nc.vector.tensor_reduce(dvals[:, b:b + 1], m[:], mybir.AxisListType.X, OP.add)
                Rb = int(R[b])
                nc.scalar.activation(dvx[:, bases[b]:bases[b] + Rb],
                                     msk_s[:, bases[b]:bases[b] + Rb], AF.Identity,
                                     scale=0.0, bias=dvals[:, b:b + 1])
            nc.sync.dma_start(cview[b], rows_b[:, 0:width])

        def make_window_cb(xcur_t, elu_bias, elu_pre, widx, xsep_cur_t, xsep_n_t,
                           xnew_t, cin_t, width, lA, lB, drep, dv_t, dvx_t=None):
            """After edge-phase blocks land in xcur_t[0:64], run the node phase
            (elu+gru) and next layer's row build for completed 512-col windows."""
            cview = cin_t[:].rearrange("(b p) e -> b p e", p=128)
            br, bz = gru_bias(widx)
            state = {"done": 0}

            def cb(done):
                while state["done"] + 4 <= done or (done >= NBLK and state["done"] < NBLK):
                    w0 = state["done"]
                    w1 = min(w0 + 4, NBLK)
                    c0 = w0 * 128
                    w = (w1 - w0) * 128
                    elu_chunk(xcur_t, elu_bias, c0, w, pre_lhsT=elu_pre)
                    gru_chunk(xcur_t, xsep_cur_t[:], widx, xsep_n_t[:], c0, w, br, bz)
                    if xnew_t is not None:
                        nc.sync.dma_start(xnew_t[64:128, c0:c0 + w], xsep_n_t[:, c0:c0 + w])
                    for b in range(w0, w1):
                        build_rows_block(cview, width, xsep_n_t[:], b, lA, lB, drep, dv_t, dvx_t)
                    state["done"] = w1
            return cb

        # ================== forward ==================
        stack0 = stkp.tile([128, NPAD], F16, tag="stk")
        xsep0 = xsp.tile([64, NPAD], F16, tag="xsep")
        dv0 = big.tile([128, NBLK, 1], F32, tag="rvals0")
        dvx0 = big.tile([128, NCH, 1], F32, tag="rvx0")
        cview0 = cins[0][:].rearrange("(b p) e -> b p e", p=128)
        for c0 in range(0, NPAD, CH):
            w = min(CH, NPAD - c0)
            xst = sp.tile([64, CH], F32, tag="nn")
            nc.sync.dma_start(xst[:, 0:w], xT_in[:, c0:c0 + w])
            fm_mm(xsep0[:, c0:c0 + w], lin1T_s[:], xst[:, 0:w], w, func=AF.Lrelu, bias=lin1b_s[:])
            nc.sync.dma_start(stack0[64:128, c0:c0 + w], xsep0[:, c0:c0 + w])
            for b in range(c0 // 128, min((c0 + w) // 128, NBLK)):
                build_rows_block(cview0, 128, xsep0[:], b, None, w1aT_s[:], attr_s[:], dv0, dvx0)
        nc.gpsimd.collective_compute("AllGather", OP.bypass, ins=[cins[0][:]],
                                     outs=[couts[0][:]], replica_groups=[list(range(NCORE))])
        stack1 = stkp.tile([128, NPAD], F16, tag="stk")
        xsep1 = xsp.tile([64, NPAD], F16, tag="xsep")
        dv1 = big.tile([128, NBLK, 1], F32, tag="rvals1")
        dvx1 = big.tile([128, NCH, 1], F32, tag="rvx1")
        cb0 = make_window_cb(stack0, gateb_s[:], g2T_s[:], 0, xsep0, xsep1,
                             stack1, cins[1], 64, atomT_s[:, 0], None, adst_s[:, 0], dv1, dvx1)
        edge_phase(0, couts[0], dvx0, stack0, window_cb=cb0)
        nc.gpsimd.collective_compute("AllGather", OP.bypass, ins=[cins[1][:]],
                                     outs=[couts[1][:]], replica_groups=[list(range(NCORE))])
        stack2 = stkp.tile([128, NPAD], F16, tag="stk")
        xsep2 = xsp.tile([64, NPAD], F16, tag="xsep")
        dv2 = big.tile([128, NBLK, 1], F32, tag="rvals2")
        dvx2 = big.tile([128, NCH, 1], F32, tag="rvx2")
        cb1 = make_window_cb(stack1, atomb_s[:, 0:1], None, 1, xsep1, xsep2,
                             stack2, cins[2], 64, atomT_s[:, 1], None, adst_s[:, 1], dv2, dvx2)
        edge_phase(1, couts[1], dvx1, stack1, window_cb=cb1)
        nc.gpsimd.collective_compute("AllGather", OP.bypass, ins=[cins[2][:]],
                                     outs=[couts[2][:]], replica_groups=[list(range(NCORE))])
        xfin = xsp.tile([64, NPAD], F16, tag="xsep")
        cb2 = make_window_cb(stack2, atomb_s[:, 1:2], None, 2, xsep2, xfin,
                             None, cinr, 128, molT_s[:], "plain", None, None)
        edge_phase(2, couts[2], dvx2, stack2, window_cb=cb2)
        nc.gpsimd.collective_compute("AllGather", OP.bypass, ins=[cinr[:]],
                                     outs=[coutr[:]], replica_groups=[list(range(NCORE))])
        # ---- readout: gather rows, resident rrow [xs | x], on-the-fly a_src
        rrow = big.tile([128, GCH, 128], F16, tag="rrow")
        asrc_ro = big.tile([128, GCH, 1], F32, tag="asr")
        RH = 8
        for b in range(GBLK):
            RGb = int(RG[b])
            gb0 = gbases[b]
            for r0 in range(0, RGb, RH):
                cr = min(RH, RGb - r0)
                c0 = gb0 + r0
                gr = grp_.tile([128, RH, 256], F16, tag="gr")
                nc.gpsimd.dma_gather(
                    gr[:, 0:cr], coutr[:], rgidx_s[:, c0 * 8:(c0 + cr) * 8],
                    cr * 128, cr * 128, 256, elem_step=256,
                    single_packet=False, queue_num=qn_state[0] % 4)
                qn_state[0] += 1
                sl = rrow[:, c0:c0 + cr]
                nc.vector.tensor_sub(sl, gr[:, 0:cr, 128:256], gr[:, 0:cr, 0:128])
                nc.vector.tensor_mul(sl, sl, rpar_s[:, c0:c0 + cr].to_broadcast([128, cr, 128]))
                nc.vector.tensor_add(sl, sl, gr[:, 0:cr, 0:128])
                nc.vector.tensor_mul(rrow[:, c0:c0 + cr, 64:128], rrow[:, c0:c0 + cr, 64:128],
                                     rmsk16_s[:, c0:c0 + cr].to_broadcast([128, cr, 64]))
                evr = gr[:, 0:cr, 0:64]  # dead (g0) half reused as scratch
                nc.vector.tensor_mul(evr, rrow[:, c0:c0 + cr, 0:64],
                                     msrc_s[:].to_broadcast([128, cr, 64]))
                nc.vector.tensor_reduce(asrc_ro[:, c0:c0 + cr], evr,
                                        mybir.AxisListType.X, OP.add)
        ofm = big.tile([64, GPAD], F32, tag="ofm")
        hro = big.tile([64, GPAD], F32, tag="hro")
        mol_stk = big.tile([128, GPAD], F16, tag="mstk")
        for b in range(GBLK):
            RGb = int(RG[b])
            pred = prp.tile([128, 64], F32, tag="pred")
            for j in range(RGb):
                nc.tensor.matmul(pred[:, 0:64], id16[:], rrow[:, gbases[b] + j, 64:128],
                                 start=(j == 0), stop=(j == RGb - 1))
            s0 = sp.tile([128, 64], F16, tag="hnm")
            nc.scalar.activation(s0[:], pred[:, 0:64], AF.Copy)
            ps = transp16(s0[:], 128, 64)
            nc.scalar.activation(ofm[:, b * 128:(b + 1) * 128], ps[0:64, 0:128], AF.Relu)
        wtil_ps = pp.tile([128, CH], F32, tag="mmq")
        nc.tensor.matmul(wtil_ps[0:64, 0:1], mol_lin_s[:], mdst_s[:], start=True, stop=True)
        wtil = cpool.tile([64, 1], F32)
        nc.vector.tensor_copy(wtil[:], wtil_ps[0:64, 0:1])
        rmsg = big.tile([128, GCH, 64], F16, tag="rmsg")
        for t in range(3):
            ddp = pp.tile([128, CH], F32, tag="mmq")
            nc.tensor.matmul(ddp[0:1, 0:GPAD], wtil[:], ofm[:], start=True, stop=True)
            dds = s1.tile([1, GPAD], F16, tag="dds")
            nc.scalar.activation(dds[:], ddp[0:1, 0:GPAD], AF.Copy)
            lgr = sp.tile([128, GCH, 1], F32, tag="lgr")
            for b in range(GBLK):
                RGb = int(RG[b])
                gb0 = gbases[b]
                psb = transp16(dds[:, b * 128:(b + 1) * 128], 1, 128)
                ddb = sp.tile([128, 1], F32, tag="ddb")
                nc.scalar.activation(ddb[:], psb[0:128, 0:1], AF.Copy)
                nc.vector.tensor_scalar(lgr[:, gb0:gb0 + RGb], asrc_ro[:, gb0:gb0 + RGb],
                                        ddb[:, 0:1], None, OP.add)
            nc.vector.scalar_tensor_tensor(lgr[:], lgr[:], NS, lgr[:], OP.mult, OP.max)
            nc.scalar.activation(lgr[:], lgr[:], AF.Exp)
            p16r = sp.tile([128, GCH, 1], F32, tag="p16r")
            nc.vector.tensor_mul(p16r[:], lgr[:], rmsk_s[:])
            nc.vector.tensor_mul(rmsg[:, :, 0:64], rrow[:, :, 0:64],
                                 p16r[:].to_broadcast([128, GCH, 64]))
            for b in range(GBLK):
                RGb = int(RG[b])
                gb0 = gbases[b]
                pred = prp.tile([128, 64], F32, tag="pred")
                for j in range(RGb):
                    nc.tensor.matmul(pred[:, 0:64], id16[:], rmsg[:, gb0 + j, 0:64],
                                     start=(j == 0), stop=(j == RGb - 1))
                rec = sp.tile([128, 1], F32, tag="rec")
                nc.vector.tensor_reduce(rec[:], p16r[:, gb0:gb0 + RGb], mybir.AxisListType.XY, OP.add)
                nc.vector.tensor_scalar(rec[:], rec[:], 1e-16, None, OP.add)
                nc.vector.reciprocal(rec[:], rec[:])
                hnm = sp.tile([128, 64], F16, tag="hnm")
                nc.scalar.activation(hnm[:], pred[:, 0:64], AF.Copy, scale=rec[:])
                ps = transp16(hnm[:], 128, 64)
                nc.scalar.activation(hro[:, b * 128:(b + 1) * 128], ps[0:64, 0:128], AF.Copy)
            elu_chunk(hro, molb_s[:], 0, GPAD, out_buf=mol_stk)
            nc.gpsimd.dma_start(mol_stk[64:128, :], ofm[:])
            gru(mol_stk, ofm, 3, ofm[:], GPAD)
        emb = s1.tile([64, GPAD], F16, tag="emb")
        fm_mm(emb[:], lin2T_s[:], ofm[:], GPAD, bias=lin2b_s[:])
        nemb = s1.tile([64, GPAD], F32, tag="nemb")
        for b in range(GBLK):
            gps = transp16(emb[:, b * 128:(b + 1) * 128], 64, 128)
            gm = sp.tile([128, 64], F32, tag="gm")
            nc.scalar.activation(gm[:], gps[0:128, 0:64], AF.Copy)
            mu = sp.tile([128, 1], F32, tag="mu")
            nc.vector.tensor_reduce(mu[:], gm[:, 0:64], mybir.AxisListType.X, OP.add)
            nc.vector.tensor_scalar(mu[:], mu[:], 1.0 / 64, None, OP.mult)
            xc = sp.tile([128, 64], F32, tag="xc")
            nc.vector.tensor_scalar(xc[:], gm[:, 0:64], mu[:], None, OP.subtract)
            sq = sp.tile([128, 64], F32, tag="sq")
            nc.scalar.activation(sq[:], xc[:], AF.Square)
            var = sp.tile([128, 1], F32, tag="var")
            nc.vector.tensor_reduce(var[:], sq[:], mybir.AxisListType.X, OP.add)
            nc.vector.tensor_scalar(var[:], var[:], 1.0 / 64, None, OP.mult)
            nc.vector.tensor_scalar(var[:], var[:], 1e-5, None, OP.add)
            nc.scalar.activation(var[:], var[:], AF.Sqrt)
            nc.vector.reciprocal(var[:], var[:])
            nc.scalar.activation(xc[:], xc[:], AF.Copy, scale=var[:])
            nc.vector.tensor_mul(xc[:], xc[:], lng_s[:, 0:64])
            nc.vector.tensor_add(xc[:], xc[:], lnb_s[:, 0:64])
            xc16 = sp.tile([128, 64], F16, tag="hnm")
            nc.scalar.activation(xc16[:], xc[:], AF.Copy)
            ps2 = transp16(xc16[:], 128, 64)
            nc.scalar.activation(nemb[:, b * 128:(b + 1) * 128], ps2[0:64, 0:128], AF.Copy)
        m1 = s1.tile([64, GPAD], F32, tag="m1")
        fm_mm(m1[:], h1T_s[:], nemb[:], GPAD, func=AF.Relu, bias=h1b_s[:])
        m2 = s1.tile([64, GPAD], F32, tag="m2")
        fm_mm(m2[:], h2T_s[:], m1[:], GPAD, func=AF.Relu, bias=h2b_s[:])
        yps = pp.tile([128, CH], F32, tag="mmq")
        nc.tensor.matmul(yps[0:1, 0:GPAD], h3T_s[:], m2[:], start=True, stop=True)
        ysb = s1.tile([1, GPAD], F32, tag="ysb")
        nc.scalar.activation(ysb[:], yps[0:1, 0:GPAD], AF.Identity, bias=h3b_s[:])
        nc.sync.dma_start(y_out[:], ysb[:])
        ctx.close()
    ctx2.__exit__(None, None, None)
    nc.finalize()
    return nc


_CACHE = {}


def kernel(**inputs):
    x = np.asarray(inputs["x"], np.float32)
    edge_attr = np.asarray(inputs["edge_attr"], np.float32)
    ei = np.asarray(inputs["edge_index"])
    batch = np.asarray(inputs["batch"])
    if "k" not in _CACHE:
        plan = build_plan(ei, batch)
        nc = build_nc(plan)
        _CACHE["k"] = (plan, nc)
    plan, nc = _CACHE["k"]

    gw = np.zeros((128, 4, 2, H), np.float16)
    gn = np.zeros((128, 4, 2, H), np.float16)
    gbx_rz = np.zeros((64, 8), np.float32)
    gbh_rz = np.zeros((64, 8), np.float32)
    gbx_n = np.zeros((H, 4), np.float32)
    gbh_n = np.zeros((H, 4), np.float32)
    packs = [
        (inputs["gru0_wx"], inputs["gru0_wh"], inputs["gru0_bx"], inputs["gru0_bh"]),
        (inputs["atom_gru_wx"][0], inputs["atom_gru_wh"][0], inputs["atom_gru_bx"][0], inputs["atom_gru_bh"][0]),
        (inputs["atom_gru_wx"][1], inputs["atom_gru_wh"][1], inputs["atom_gru_bx"][1], inputs["atom_gru_bh"][1]),
        (inputs["mol_gru_wx"], inputs["mol_gru_wh"], inputs["mol_gru_bx"], inputs["mol_gru_bh"]),
    ]
    for i, (wx, wh, bx, bh) in enumerate(packs):
        wx = np.asarray(wx, np.float32); wh = np.asarray(wh, np.float32)
        bx = np.asarray(bx, np.float32); bh = np.asarray(bh, np.float32)
        gw[0:64, i, 0] = wx[0:64].T; gw[64:128, i, 0] = wh[0:64].T
        gw[0:64, i, 1] = wx[64:128].T; gw[64:128, i, 1] = wh[64:128].T
        gn[0:64, i, 0] = wx[128:192].T; gn[64:128, i, 1] = wh[128:192].T
        gbx_rz[:, 2 * i] = bx[0:64]; gbx_rz[:, 2 * i + 1] = bx[64:128]
        gbh_rz[:, 2 * i] = bh[0:64]; gbh_rz[:, 2 * i + 1] = bh[64:128]
        gbx_n[:, i] = bx[128:192]; gbh_n[:, i] = bh[128:192]

    glw = np.asarray(inputs["gate_lin1_w"], np.float32)
    rep16 = lambda v: np.tile(np.asarray(v, np.float16).reshape(1, -1), (128, 1))
    rep32 = lambda v: np.tile(np.asarray(v, np.float32).reshape(1, -1), (128, 1))
    a = lambda k: np.asarray(inputs[k], np.float32)
    a16 = lambda k: np.asarray(inputs[k], np.float16)
    wts = dict(
        lin1T=a("lin1_w").T.copy(), lin1_b=a("lin1_b").reshape(H, 1),
        w1aT=glw[:, 0:64].T.astype(np.float16).copy(),
        w1bT=glw[:, 64:80].T.astype(np.float16).copy(),
        attl_rep=rep16(inputs["gate_att_l"]).reshape(128, 1, H),
        attr_rep=rep16(inputs["gate_att_r"]),
        g2T=a16("gate_lin2_w").T.copy(), gate_b=a("gate_bias").reshape(H, 1),
        atomT=np.stack([a16("atom_lin_w")[l].T for l in range(2)], 1),
        asrc_rep=np.stack([rep16(inputs["atom_att_src"][l]) for l in range(2)], 1),
        adst_rep=np.stack([rep16(inputs["atom_att_dst"][l]) for l in range(2)], 1),
        atom_b=a("atom_bias").T.copy(),
        molT=a16("mol_lin_w").T.copy(), mol_lin=a("mol_lin_w").copy(),
        matt_src_rep=rep16(inputs["mol_att_src"]).reshape(128, 1, H),
        matt_dst=a("mol_att_dst").reshape(H, 1),
        mol_b=a("mol_bias").reshape(H, 1),
        gruW=gw, gruN=gn, gbx_rz=gbx_rz, gbh_rz=gbh_rz, gbx_n=gbx_n, gbh_n=gbh_n,
        lin2T=a("lin2_w").T.copy(), lin2_b=a("lin2_b").reshape(H, 1),
        lng_rep=rep32(inputs["ln_g"]), lnb_rep=rep32(inputs["ln_b"]),
        h1T=a("h1_w").T.copy(), h1_b=a("h1_b").reshape(H, 1),
        h2T=a("h2_w").T.copy(), h2_b=a("h2_b").reshape(H, 1),
        h3T=a("h3_w").T.copy(), h3_b=a("h3_b").reshape(1, 1),
    )
    in_maps = []
    for c in range(NCORE):
        pc = plan["cores"][c]
        rc = plan["rcores"][c]
        xT = np.zeros((H, NPAD), np.float32)
        xT[:, 0:NPC] = x[c * NPC:(c + 1) * NPC][plan["perms"][c]].T
        eaT = np.zeros((16, plan["S"]), np.float16)
        val = pc["esel"] >= 0
        eaT[:, val] = edge_attr[pc["esel"][val]].T.astype(np.float16)
        im = dict(xT_in=xT, gidx=pc["gidx"], par=pc["par"], msk=pc["msk"], eaT=eaT,
                  rgidx=rc["rgidx"], rpar=rc["rpar"], rmsk=rc["rmsk"],
                  rmsk16=rc["rmsk16"], **wts)
        in_maps.append(im)
    res = run_bass_kernel_spmd(nc, in_maps, core_ids=list(range(NCORE)))
    if getattr(res, "exec_time_ns", None):
        print(f"HW exec time: {res.exec_time_ns} ns", flush=True)
    y = np.zeros(B, np.float32)
    for c in range(NCORE):
        ys = res.results[c]["y_out"].reshape(GPAD)[0:GPC]
        y[c * GPC + plan["gperms"][c]] = ys
    return y.reshape(B, 1)


# revision 35
# speedup vs baseline: 1.4084x; 1.0770x over previous
"""AttentiveFP forward on 8 Trainium2 NeuronCores (Bass/Tile).

Edges sharded by dst-owner core; per-core nodes sorted by in-degree with a
round-robin slot structure so segment softmax/sum become dense PSUM matmul
accumulation. Edge phase processed in large multi-block groups: one
dma_gather per ~34 rounds, wide DVE ops (parity select in place), DVE
leaky-relu (no ACT table ping-pong), single Exp per group. Layers 1-2 use
256B gather elements (64-f16 rows, att_src dot computed on the fly), halving
gather traffic and AllGather payload. fp16 node state feeds all PE matmuls.
"""
import numpy as np
from contextlib import ExitStack

import concourse.bass as bass
import concourse.tile as tile
from concourse import bacc, mybir
from concourse.bass_utils import run_bass_kernel_spmd
from concourse.masks import make_identity

F32 = mybir.dt.float32
F16 = mybir.dt.float16
I16 = mybir.dt.int16
AF = mybir.ActivationFunctionType
OP = mybir.AluOpType

NCORE = 8
N, E, B = 50000, 500000, 2048
H = 64
NS = 0.01
NPC = N // NCORE
GPC = B // NCORE
NBLK = (NPC + 127) // 128
NPAD = NBLK * 128
GBLK = (GPC + 127) // 128
GPAD = GBLK * 128
TROWS = NCORE * NPAD
MAXR = 10
CH = 512
GT = 16       # max rounds per edge-phase group


def _calls_for(R, maxr=MAXR):
    calls, base, bases = [], 0, []
    for r in R:
        bases.append(base)
        calls.append([(r0, min(r0 + maxr, int(r))) for r0 in range(0, int(r), maxr)])
        base += int(r)
    return calls, bases, base


def _wrap_into(gidx, arr, col0):
    n = arr.shape[0]
    blk = arr.reshape(n // 16, 16).T
    gidx[:16, col0:col0 + n // 16] = blk
    gidx[16:128, col0:col0 + n // 16] = np.tile(blk, (7, 1))


def build_plan(edge_index, batch):
    src = edge_index[0].astype(np.int64)
    dst = edge_index[1].astype(np.int64)
    owner = dst // NPC

    perms, degs_sorted, grp_starts, egrp = [], [], [], []
    sortpos = np.zeros(N, np.int64)
    for c in range(NCORE):
        n0 = c * NPC
        emask = np.nonzero(owner == c)[0]
        deg = np.bincount(dst[emask] - n0, minlength=NPC)
        order = np.argsort(-deg, kind="stable")
        perms.append(order)
        sortpos[n0 + order] = np.arange(NPC)
        dsorted = deg[order]
        degs_sorted.append(dsorted)
        eorder = np.argsort(sortpos[dst[emask]], kind="stable")
        egrp.append(emask[eorder])
        grp_starts.append(np.concatenate([[0], np.cumsum(dsorted)]))
    trow = (np.arange(N) // NPC) * NPAD + sortpos

    R = np.ones(NBLK, np.int64)
    for b in range(NBLK):
        for c in range(NCORE):
            d = degs_sorted[c][b * 128:(b + 1) * 128]
            if len(d):
                R[b] = max(R[b], int(d[0]))
    calls, bases, NCH = _calls_for(R)
    S = NCH * 128

    gsize = np.bincount(batch, minlength=B)
    gstart = np.concatenate([[0], np.cumsum(gsize)])
    gperms, gss = [], []
    for c in range(NCORE):
        gs = gsize[c * GPC:(c + 1) * GPC]
        gorder = np.argsort(-gs, kind="stable")
        gperms.append(gorder)
        gss.append(gs[gorder])
    RG = np.ones(GBLK, np.int64)
    for b in range(GBLK):
        for c in range(NCORE):
            d = gss[c][b * 128:(b + 1) * 128]
            if len(d):
                RG[b] = max(RG[b], int(d[0]))
    gcalls, gbases, GCH = _calls_for(RG, 8)
    SR = GCH * 128

    cores = []
    lanes = np.arange(128)
    for c in range(NCORE):
        gidx = np.zeros((128, S // 16), np.int16)
        par = np.zeros((128, NCH, 1), np.float16)
        msk = np.zeros((128, NCH, 1), np.float32)
        esel = np.full(NCH * 128, -1, np.int64)
        ds = degs_sorted[c]
        gst = grp_starts[c]
        eg = egrp[c]
        for b in range(NBLK):
            for (r0, r1) in calls[b]:
                ia = np.zeros((r1 - r0) * 128, np.int64)
                for r in range(r0, r1):
                    ch = bases[b] + r
                    p = b * 128 + lanes
                    pc = np.minimum(p, NPC - 1)
                    ok = (p < NPC) & (r < ds[pc])
                    eids = gst[pc] + r
                    ge = np.where(ok, eg[np.where(ok, np.minimum(eids, len(eg) - 1), 0)], -1)
                    esel[ch * 128 + lanes] = ge
                    rows = np.where(ok, trow[src[np.maximum(ge, 0)]], 0)
                    ia[(r - r0) * 128 + lanes] = rows >> 1
                    par[:, ch, 0] = (rows & 1).astype(np.float16)
                    msk[:, ch, 0] = ok.astype(np.float32)
                _wrap_into(gidx, ia, (bases[b] + r0) * 8)
        cores.append(dict(gidx=gidx, par=par, msk=msk, esel=esel))

    rcores = []
    for c in range(NCORE):
        rgidx = np.zeros((128, SR // 16), np.int16)
        rpar = np.zeros((128, GCH, 1), np.float16)
        rmsk = np.zeros((128, GCH, 1), np.float32)
        rmsk16 = np.zeros((128, GCH, 1), np.float16)
        gs = gss[c]
        gp = gperms[c]
        for b in range(GBLK):
            for (r0, r1) in gcalls[b]:
                ia = np.zeros((r1 - r0) * 128, np.int64)
                for r in range(r0, r1):
                    ch = gbases[b] + r
                    p = b * 128 + lanes
                    pc = np.minimum(p, GPC - 1)
                    ok = (p < GPC) & (r < gs[pc])
                    g = c * GPC + gp[pc]
                    node = np.where(ok, gstart[g] + r, 0)
                    rows = np.where(ok, trow[node], 0)
                    ia[(r - r0) * 128 + lanes] = rows >> 1
                    rpar[:, ch, 0] = (rows & 1).astype(np.float16)
                    rmsk[:, ch, 0] = ok.astype(np.float32)
                    rmsk16[:, ch, 0] = ok.astype(np.float16)
                _wrap_into(rgidx, ia, (gbases[b] + r0) * 8)
        rcores.append(dict(rgidx=rgidx, rpar=rpar, rmsk=rmsk, rmsk16=rmsk16))

    return dict(R=R, bases=bases, NCH=NCH, S=S,
                RG=RG, gbases=gbases, GCH=GCH, SR=SR,
                cores=cores, rcores=rcores, perms=perms, gperms=gperms)


def build_nc(plan):
    R, bases, NCH, S = plan["R"], plan["bases"], plan["NCH"], plan["S"]
    RG, gbases, GCH, SR = plan["RG"], plan["gbases"], plan["GCH"], plan["SR"]
    RGT = int(max(int(r) for r in RG))

    # split blocks into spans of <= GT rounds, pack spans into groups
    spans = []
    for b in range(NBLK):
        r = int(R[b])
        for s0 in range(0, r, GT):
            spans.append((b, s0, min(s0 + GT, r)))
    def pack_span_groups(cap):
        gs, cur, cur_r = [], [], 0
        for spn in spans:
            ln = spn[2] - spn[1]
            if cur and cur_r + ln > cap:
                gs.append(cur)
                cur, cur_r = [], 0
            cur.append(spn)
            cur_r += ln
        gs.append(cur)
        return gs
    sgroups = pack_span_groups(GT)
    GTS = max(sum(s[2] - s[1] for s in gl) for gl in sgroups)

    nc = bacc.Bacc("TRN2", target_bir_lowering=False, debug=False,
                   num_devices=NCORE, num_swdge_queues=4)

    def din(name, shape, dt=F32):
        return nc.dram_tensor(name, shape, dt, kind="ExternalInput")

    xT_in = din("xT_in", [H, NPAD])
    gidx_in = din("gidx", [128, S // 16], I16)
    par_in = din("par", [128, NCH, 1], F16)
    msk_in = din("msk", [128, NCH, 1], F32)
    eaT_in = din("eaT", [16, S], F16)
    rgidx_in = din("rgidx", [128, SR // 16], I16)
    rpar_in = din("rpar", [128, GCH, 1], F16)
    rmsk_in = din("rmsk", [128, GCH, 1], F32)
    rmsk16_in = din("rmsk16", [128, GCH, 1], F16)
    lin1T = din("lin1T", [H, H]); lin1_b = din("lin1_b", [H, 1])
    w1aT = din("w1aT", [H, H], F16); w1bT = din("w1bT", [16, H], F16)
    attl_rep = din("attl_rep", [128, 1, H], F16)
    attr_rep = din("attr_rep", [128, H], F16)
    g2T = din("g2T", [H, H], F16); gate_b = din("gate_b", [H, 1])
    atomT = din("atomT", [H, 2, H], F16)
    asrc_rep = din("asrc_rep", [128, 2, H], F16)
    adst_rep = din("adst_rep", [128, 2, H], F16)
    atom_b = din("atom_b", [H, 2])
    molT = din("molT", [H, H], F16)
    mol_lin = din("mol_lin", [H, H])
    matt_src_rep = din("matt_src_rep", [128, 1, H], F16)
    matt_dst = din("matt_dst", [H, 1])
    mol_b = din("mol_b", [H, 1])
    gruW = din("gruW", [128, 4, 2, H], F16)   # [K, widx, gate(r/z), M]
    gruN = din("gruN", [128, 4, 2, H], F16)   # [K, widx, (nx/nh), M] zero-padded
    gbx_rz = din("gbx_rz", [64, 8])
    gbh_rz = din("gbh_rz", [64, 8])
    gbx_n = din("gbx_n", [H, 4])
    gbh_n = din("gbh_n", [H, 4])
    lin2T = din("lin2T", [H, H]); lin2_b = din("lin2_b", [H, 1])
    lng_rep = din("lng_rep", [128, H]); lnb_rep = din("lnb_rep", [128, H])
    h1T = din("h1T", [H, H]); h1_b = din("h1_b", [H, 1])
    h2T = din("h2T", [H, H]); h2_b = din("h2_b", [H, 1])
    h3T = din("h3T", [H, 1]); h3_b = din("h3_b", [1, 1])

    y_out = nc.dram_tensor("y_out", [1, GPAD], F32, kind="ExternalOutput")

    cin0 = nc.dram_tensor("cin0", [NPAD, 128], F16)
    cout0 = nc.dram_tensor("cout0", [TROWS // 2, 256], F16, addr_space="Shared")
    cins = [cin0]
    couts = [cout0]
    for l in (1, 2):
        cins.append(nc.dram_tensor(f"cin{l}", [NPAD, 64], F16))
        couts.append(nc.dram_tensor(f"cout{l}", [TROWS // 2, 128], F16,
                                    addr_space="Shared"))
    cinr = nc.dram_tensor("cinr", [NPAD, 128], F16)
    coutr = nc.dram_tensor("coutr", [TROWS // 2, 256], F16, addr_space="Shared")

    ctx = ExitStack()
    ctx2 = nc.allow_low_precision(reason="fp16 edge tables/messages/state")
    ctx2.__enter__()
    with tile.TileContext(nc) as tc:
        cpool = ctx.enter_context(tc.tile_pool(name="const", bufs=1))
        wpool = ctx.enter_context(tc.tile_pool(name="wts", bufs=1))
        big = ctx.enter_context(tc.tile_pool(name="big", bufs=1))
        stkp = ctx.enter_context(tc.tile_pool(name="stkp", bufs=2))
        xsp = ctx.enter_context(tc.tile_pool(name="xsp", bufs=2))
        g0p = ctx.enter_context(tc.tile_pool(name="g0p", bufs=4))
        g12p = ctx.enter_context(tc.tile_pool(name="g12p", bufs=5))
        grp_ = ctx.enter_context(tc.tile_pool(name="gathr", bufs=3))
        eap = ctx.enter_context(tc.tile_pool(name="eap", bufs=1))
        sp = ctx.enter_context(tc.tile_pool(name="scr", bufs=2))
        s1 = ctx.enter_context(tc.tile_pool(name="scr1", bufs=1))
        pp = ctx.enter_context(tc.tile_pool(name="ps", bufs=2, space="PSUM"))
        prp = ctx.enter_context(tc.tile_pool(name="psp", bufs=2, space="PSUM"))
        rp = ctx.enter_context(tc.tile_pool(name="psr", bufs=2, space="PSUM"))
        zp = ctx.enter_context(tc.tile_pool(name="psz", bufs=2, space="PSUM"))

        id32 = cpool.tile([128, 128], F32)
        make_identity(nc, id32[:])
        id16 = cpool.tile([128, 128], F16)
        nc.vector.tensor_copy(id16[:], id32[:])

        def load(t, shape, dt=F32):
            s = wpool.tile(shape, dt, tag=f"w_{t.name}")
            nc.sync.dma_start(s[:], t[:])
            return s

        gidx_s = load(gidx_in, [128, S // 16], I16)
        rgidx_s = load(rgidx_in, [128, SR // 16], I16)
        par_s = load(par_in, [128, NCH, 1], F16)
        msk_s = load(msk_in, [128, NCH, 1], F32)
        rpar_s = load(rpar_in, [128, GCH, 1], F16)
        rmsk_s = load(rmsk_in, [128, GCH, 1], F32)
        rmsk16_s = load(rmsk16_in, [128, GCH, 1], F16)
        lin1T_s = load(lin1T, [H, H]); lin1b_s = load(lin1_b, [H, 1])
        w1aT_s = load(w1aT, [H, H], F16); w1bT_s = load(w1bT, [16, H], F16)
        attl_s = load(attl_rep, [128, 1, H], F16)
        attr_s = load(attr_rep, [128, H], F16)
        g2T_s = load(g2T, [H, H], F16); gateb_s = load(gate_b, [H, 1])
        atomT_s = load(atomT, [H, 2, H], F16)
        asrc_s = load(asrc_rep, [128, 2, H], F16)
        adst_s = load(adst_rep, [128, 2, H], F16)
        atomb_s = load(atom_b, [H, 2])
        molT_s = load(molT, [H, H], F16); mol_lin_s = load(mol_lin, [H, H])
        msrc_s = load(matt_src_rep, [128, 1, H], F16)
        mdst_s = load(matt_dst, [H, 1])
        molb_s = load(mol_b, [H, 1])
        gruW_s = load(gruW, [128, 4, 2, H], F16)
        gruN_s = load(gruN, [128, 4, 2, H], F16)
        gbxrz_s = load(gbx_rz, [64, 8])
        gbhrz_s = load(gbh_rz, [64, 8])
        gbxn_s = load(gbx_n, [H, 4])
        gbhn_s = load(gbh_n, [H, 4])
        lin2T_s = load(lin2T, [H, H]); lin2b_s = load(lin2_b, [H, 1])
        lng_s = load(lng_rep, [128, H]); lnb_s = load(lnb_rep, [128, H])
        h1T_s = load(h1T, [H, H]); h1b_s = load(h1_b, [H, 1])
        h2T_s = load(h2T, [H, H]); h2b_s = load(h2_b, [H, 1])
        h3T_s = load(h3T, [H, 1]); h3b_s = load(h3_b, [1, 1])

        def fm_mm(out_ap, lhsT, rhs, ncols, func=None, bias=0.0):
            M = lhsT.shape[-1]
            for c0 in range(0, ncols, CH):
                w = min(CH, ncols - c0)
                ps = pp.tile([128, CH], F32, tag="mmq")
                nc.tensor.matmul(ps[0:M, :w], lhsT, rhs[:, c0:c0 + w],
                                 start=True, stop=True)
                f = func
                if f is None:
                    f = AF.Copy if isinstance(bias, float) else AF.Identity
                nc.scalar.activation(out_ap[:, c0:c0 + w], ps[0:M, :w], f, bias=bias)

        def gru_bias(widx):
            br = sp.tile([64, 1], F32, tag=f"brz{widx}")
            nc.vector.tensor_add(br[:], gbxrz_s[:, 2 * widx:2 * widx + 1], gbhrz_s[:, 2 * widx:2 * widx + 1])
            bz = sp.tile([64, 1], F32, tag=f"bz{widx}")
            nc.vector.tensor_add(bz[:], gbxrz_s[:, 2 * widx + 1:2 * widx + 2], gbhrz_s[:, 2 * widx + 1:2 * widx + 2])
            return br, bz

        def gru_chunk(stk, xprev, widx, out_ap, c0, w, br, bz):
            if True:
                prz = pp.tile([128, CH], F32, tag="mmq")
                nc.tensor.matmul(prz[0:64, :w], gruW_s[:, widx, 0], stk[:, c0:c0 + w], start=True, stop=True)
                prz2 = pp.tile([128, CH], F32, tag="mmq")
                nc.tensor.matmul(prz2[0:64, :w], gruW_s[:, widx, 1], stk[:, c0:c0 + w], start=True, stop=True)
                rz = sp.tile([64, CH], F32, tag="rz")
                nc.scalar.activation(rz[:, :w], prz[0:64, :w], AF.Sigmoid, bias=br[:])
                zz = sp.tile([64, CH], F32, tag="zz")
                nc.scalar.activation(zz[:, :w], prz2[0:64, :w], AF.Sigmoid, bias=bz[:])
                pn1 = pp.tile([128, CH], F32, tag="mmq")
                nc.tensor.matmul(pn1[0:64, :w], gruN_s[:, widx, 0], stk[:, c0:c0 + w], start=True, stop=True)
                pn2 = pp.tile([128, CH], F32, tag="mmq")
                nc.tensor.matmul(pn2[0:64, :w], gruN_s[:, widx, 1], stk[:, c0:c0 + w], start=True, stop=True)
                hnb = sp.tile([64, CH], F32, tag="hnb")
                nc.scalar.activation(hnb[:, :w], pn2[0:64, :w], AF.Identity, bias=gbhn_s[:, widx:widx + 1])
                nc.vector.tensor_mul(hnb[:, :w], rz[:, :w], hnb[:, :w])
                nc.vector.tensor_add(hnb[:, :w], hnb[:, :w], pn1[0:64, :w])
                nn = sp.tile([64, CH], F32, tag="nn")
                nc.scalar.activation(nn[:, :w], hnb[:, :w], AF.Tanh, bias=gbxn_s[:, widx:widx + 1])
                d = rz  # rz buffer is dead past the mul above
                nc.vector.tensor_sub(d[:, :w], xprev[:, c0:c0 + w], nn[:, :w])
                nc.vector.tensor_mul(d[:, :w], zz[:, :w], d[:, :w])
                nc.vector.tensor_add(d[:, :w], nn[:, :w], d[:, :w])
                nc.scalar.activation(out_ap[:, c0:c0 + w], d[:, :w], AF.Relu)

        def gru(stk, xprev, widx, out_ap, ncols):
            br, bz = gru_bias(widx)
            for c0 in range(0, ncols, CH):
                gru_chunk(stk, xprev, widx, out_ap, c0, min(CH, ncols - c0), br, bz)

        def elu_chunk(buf, bias_ap, c0, w, pre_lhsT=None, out_buf=None):
            if pre_lhsT is not None:
                ps = pp.tile([128, CH], F32, tag="mmq")
                nc.tensor.matmul(ps[0:64, :w], pre_lhsT, buf[0:64, c0:c0 + w], start=True, stop=True)
                src = ps[0:64, :w]
            else:
                src = buf[0:64, c0:c0 + w]
            e1 = sp.tile([64, CH], F32, tag="rz")
            nc.scalar.activation(e1[:, :w], src, AF.Exp, bias=bias_ap)
            t1 = sp.tile([64, CH], F32, tag="zz")
            nc.scalar.activation(t1[:, :w], e1[:, :w], AF.Relu, bias=1.0, scale=-1.0)
            t2 = sp.tile([64, CH], F32, tag="hnb")
            nc.scalar.activation(t2[:, :w], src, AF.Relu, bias=bias_ap)
            dst = buf if out_buf is None else out_buf
            nc.vector.tensor_sub(dst[0:64, c0:c0 + w], t2[:, :w], t1[:, :w])

        def elu_inplace(buf, bias_ap, ncols, pre_lhsT=None):
            """buf[0:64, :] = elu((pre_lhsT.T @ buf[0:64]) + bias)."""
            for c0 in range(0, ncols, CH):
                elu_chunk(buf, bias_ap, c0, min(CH, ncols - c0), pre_lhsT)

        qn_state = [0]

        def edge_phase(layer, table, dvals, hdst, window_cb=None):
            """hdst[0:64, :NPAD] (f16) <- normalized aggregation (feature-major)."""
            W = 256 if layer == 0 else 128
            half = W // 2
            pool = g0p if layer == 0 else g12p
            gtile = GTS
            p16L = big.tile([128, NCH, 1], F32, tag=f"p16L{layer}")
            preds = {}
            for glist in sgroups:
                ch0 = bases[glist[0][0]] + glist[0][1]
                Gr = sum(s[2] - s[1] for s in glist)
                g = pool.tile([128, gtile, W], F16, tag=f"g{min(layer, 1)}")
                nc.gpsimd.dma_gather(
                    g[:, 0:Gr], table[:], gidx_s[:, ch0 * 8:(ch0 + Gr) * 8],
                    Gr * 128, Gr * 128, W, elem_step=W,
                    single_packet=False, queue_num=qn_state[0] % 4)
                qn_state[0] += 1
                # parity select in place: row -> g[:, :, half:W]
                nc.vector.tensor_sub(g[:, 0:Gr, half:W], g[:, 0:Gr, half:W], g[:, 0:Gr, 0:half])
                nc.vector.tensor_mul(g[:, 0:Gr, half:W], g[:, 0:Gr, half:W],
                                     par_s[:, ch0:ch0 + Gr].to_broadcast([128, Gr, half]))
                nc.vector.tensor_add(g[:, 0:Gr, half:W], g[:, 0:Gr, half:W], g[:, 0:Gr, 0:half])
                lg = sp.tile([128, gtile, 1], F32, tag="lg")
                ev = g[:, 0:Gr, 0:64]  # dead (g0) half reused as scratch
                if layer == 0:
                    for r0 in range(0, Gr, 8):
                        r1 = min(r0 + 8, Gr)
                        ea = eap.tile([16, 8 * 128], F16, tag="ea")
                        nc.sync.dma_start(ea[:, 0:(r1 - r0) * 128],
                                          eaT_in[:, (ch0 + r0) * 128:(ch0 + r1) * 128])
                        pz = zp.tile([128, 8, 64], F32, tag="z1")
                        for r in range(r0, r1):
                            nc.tensor.matmul(pz[:, r - r0], ea[:, (r - r0) * 128:(r - r0 + 1) * 128],
                                             w1bT_s[:], start=True, stop=True)
                        nc.vector.tensor_add(g[:, r0:r1, 192:256], g[:, r0:r1, 192:256],
                                             pz[:, 0:r1 - r0])
                        nc.vector.scalar_tensor_tensor(g[:, r0:r1, 192:256], g[:, r0:r1, 192:256],
                                                       NS, g[:, r0:r1, 192:256], OP.mult, OP.max)
                    nc.vector.tensor_mul(ev, g[:, 0:Gr, 192:256],
                                         attl_s[:].to_broadcast([128, Gr, 64]))
                else:
                    nc.vector.tensor_mul(ev, g[:, 0:Gr, 64:128],
                                         asrc_s[:, layer - 1:layer].to_broadcast([128, Gr, 64]))
                nc.vector.tensor_reduce(lg[:, 0:Gr], ev, mybir.AxisListType.X, OP.add)
                nc.vector.tensor_add(lg[:, 0:Gr], lg[:, 0:Gr], dvals[:, ch0:ch0 + Gr])
                nc.vector.scalar_tensor_tensor(lg[:, 0:Gr], lg[:, 0:Gr], NS, lg[:, 0:Gr],
                                               OP.mult, OP.max)
                nc.scalar.activation(lg[:, 0:Gr], lg[:, 0:Gr], AF.Exp)
                p16 = p16L[:, ch0:ch0 + Gr]
                nc.vector.tensor_mul(p16, lg[:, 0:Gr], msk_s[:, ch0:ch0 + Gr])
                # msg: x part scaled in place; denominator via per-block reduce
                nc.vector.tensor_mul(g[:, 0:Gr, half:half + 64], g[:, 0:Gr, half:half + 64],
                                     p16.to_broadcast([128, Gr, 64]))
                o = 0
                done_b = 0
                for (b, r0, r1) in glist:
                    Rb = int(R[b])
                    if b not in preds:
                        predt = prp.tile([128, 64], F32, tag="pred")
                        preds[b] = predt
                    pred = preds[b]
                    for j in range(r1 - r0):
                        nc.tensor.matmul(pred[:, 0:64], id16[:], g[:, o + j, half:half + 64],
                                         start=(r0 + j == 0), stop=(r0 + j == Rb - 1))
                    o += r1 - r0
                    if r1 < Rb:
                        continue
                    del preds[b]
                    done_b = b + 1
                    rec = sp.tile([128, 1], F32, tag="rec")
                    nc.vector.tensor_reduce(rec[:], p16L[:, bases[b]:bases[b] + Rb],
                                            mybir.AxisListType.XY, OP.add)
                    nc.vector.tensor_scalar(rec[:], rec[:], 1e-16, None, OP.add)
                    nc.vector.reciprocal(rec[:], rec[:])
                    hnm = sp.tile([128, 64], F16, tag="hnm")
                    nc.scalar.activation(hnm[:], pred[:, 0:64], AF.Copy, scale=rec[:])
                    ps = transp16(hnm[:], 128, 64)
                    nc.scalar.activation(hdst[0:64, b * 128:(b + 1) * 128], ps[0:64, 0:128], AF.Copy)
                if window_cb is not None and done_b:
                    window_cb(done_b)

        def transp16(in_ap, a, bdim):
            """Transpose [a, bdim] f16 SBUF -> PSUM [bdim, a]."""
            ps = rp.tile([128, 128], F16, tag="tp16")
            nc.tensor.transpose(ps[0:bdim, 0:a], in_ap, id16[0:a, 0:a])
            return ps

        def build_rows(cin_t, width, x16, lhsT_A, lhsT_B, dst_rep, dvals, dvx=None):
            """Per block: rows[:, 0:64] = (A @ x_b).T (A None -> x_b.T);
            cols 64:128 = (B @ x_b).T (B 'plain' -> x_b.T). dst_rep -> dvals."""
            cview = cin_t[:].rearrange("(b p) e -> b p e", p=128)
            for b in range(NBLK):
                build_rows_block(cview, width, x16, b, lhsT_A, lhsT_B, dst_rep, dvals, dvx)

        def build_rows_block(cview, width, x16, b, lhsT_A, lhsT_B, dst_rep, dvals, dvx=None):
            rows_b = sp.tile([128, 128], F16, tag="rb")
            xb = x16[:, b * 128:(b + 1) * 128]
            if lhsT_A is not None:
                psm = pp.tile([128, CH], F32, tag="mmq")
                nc.tensor.matmul(psm[0:64, 0:128], lhsT_A, xb, start=True, stop=True)
                xa = sp.tile([64, 128], F16, tag="xa")
                nc.scalar.activation(xa[:], psm[0:64, 0:128], AF.Copy)
                ps = transp16(xa[:], 64, 128)
            else:
                ps = transp16(xb, 64, 128)
            nc.scalar.activation(rows_b[:, 0:64], ps[0:128, 0:64], AF.Copy)
            if lhsT_B is not None:
                if isinstance(lhsT_B, str):
                    ps2 = transp16(xb, 64, 128)
                else:
                    psm2 = pp.tile([128, CH], F32, tag="mmq")
                    nc.tensor.matmul(psm2[0:64, 0:128], lhsT_B, xb, start=True, stop=True)
                    xa2 = sp.tile([64, 128], F16, tag="xa")
                    nc.scalar.activation(xa2[:], psm2[0:64, 0:128], AF.Copy)
                    ps2 = transp16(xa2[:], 64, 128)
                nc.scalar.activation(rows_b[:, 64:128], ps2[0:128, 0:64], AF.Copy)
            if dst_rep is not None:
                m = sp.tile([128, H], F32, tag="dvm")
                nc.vector.tensor_mul(m[:], rows_b[:, 0:64], dst_rep)
                nc.vector.tensor_reduce(dvals[:, b:b + 1], m[:], mybir.AxisListType.X, OP.add)
                Rb = int(R[b])
                nc.scalar.activation(dvx[:, bases[b]:bases[b] + Rb],
                                     msk_s[:, bases[b]:bases[b] + Rb], AF.Identity,
                                     scale=0.0, bias=dvals[:, b:b + 1])
            nc.sync.dma_start(cview[b], rows_b[:, 0:width])

        def make_window_cb(xcur_t, elu_bias, elu_pre, widx, xsep_cur_t, xsep_n_t,
                           xnew_t, cin_t, width, lA, lB, drep, dv_t, dvx_t=None):
            """After edge-phase blocks land in xcur_t[0:64], run the node phase
            (elu+gru) and next layer's row build for completed 512-col windows."""
            cview = cin_t[:].rearrange("(b p) e -> b p e", p=128)
            br, bz = gru_bias(widx)
            state = {"done": 0}

            def cb(done):
                while state["done"] + 4 <= done or (done >= NBLK and state["done"] < NBLK):
                    w0 = state["done"]
                    w1 = min(w0 + 4, NBLK)
                    c0 = w0 * 128
                    w = (w1 - w0) * 128
                    elu_chunk(xcur_t, elu_bias, c0, w, pre_lhsT=elu_pre)
                    gru_chunk(xcur_t, xsep_cur_t[:], widx, xsep_n_t[:], c0, w, br, bz)
                    if xnew_t is not None:
                        nc.sync.dma_start(xnew_t[64:128, c0:c0 + w], xsep_n_t[:, c0:c0 + w])
                    for b in range(w0, w1):
                        build_rows_block(cview, width, xsep_n_t[:], b, lA, lB, drep, dv_t, dvx_t)
                    state["done"] = w1
            return cb

        # ================== forward ==================
        stack0 = stkp.tile([128, NPAD], F16, tag="stk")
        xsep0 = xsp.tile([64, NPAD], F16, tag="xsep")
        dv0 = big.tile([128, NBLK, 1], F32, tag="rvals0")
        dvx0 = big.tile([128, NCH, 1], F32, tag="rvx0")
        cview0 = cins[0][:].rearrange("(b p) e -> b p e", p=128)
        for c0 in range(0, NPAD, CH):
            w = min(CH, NPAD - c0)
            xst = sp.tile([64, CH], F32, tag="nn")
            nc.sync.dma_start(xst[:, 0:w], xT_in[:, c0:c0 + w])
            fm_mm(xsep0[:, c0:c0 + w], lin1T_s[:], xst[:, 0:w], w, func=AF.Lrelu, bias=lin1b_s[:])
            nc.sync.dma_start(stack0[64:128, c0:c0 + w], xsep0[:, c0:c0 + w])
            for b in range(c0 // 128, min((c0 + w) // 128, NBLK)):
                build_rows_block(cview0, 128, xsep0[:], b, None, w1aT_s[:], attr_s[:], dv0, dvx0)
        nc.gpsimd.collective_compute("AllGather", OP.bypass, ins=[cins[0][:]],
                                     outs=[couts[0][:]], replica_groups=[list(range(NCORE))])
        stack1 = stkp.tile([128, NPAD], F16, tag="stk")
        xsep1 = xsp.tile([64, NPAD], F16, tag="xsep")
        dv1 = big.tile([128, NBLK, 1], F32, tag="rvals1")
        dvx1 = big.tile([128, NCH, 1], F32, tag="rvx1")
        cb0 = make_window_cb(stack0, gateb_s[:], g2T_s[:], 0, xsep0, xsep1,
                             stack1, cins[1], 64, atomT_s[:, 0], None, adst_s[:, 0], dv1, dvx1)
        edge_phase(0, couts[0], dvx0, stack0, window_cb=cb0)
        nc.gpsimd.collective_compute("AllGather", OP.bypass, ins=[cins[1][:]],
                                     outs=[couts[1][:]], replica_groups=[list(range(NCORE))])
        stack2 = stkp.tile([128, NPAD], F16, tag="stk")
        xsep2 = xsp.tile([64, NPAD], F16, tag="xsep")
        dv2 = big.tile([128, NBLK, 1], F32, tag="rvals2")
        dvx2 = big.tile([128, NCH, 1], F32, tag="rvx2")
        cb1 = make_window_cb(stack1, atomb_s[:, 0:1], None, 1, xsep1, xsep2,
                             stack2, cins[2], 64, atomT_s[:, 1], None, adst_s[:, 1], dv2, dvx2)
        edge_phase(1, couts[1], dvx1, stack1, window_cb=cb1)
        nc.gpsimd.collective_compute("AllGather", OP.bypass, ins=[cins[2][:]],
                                     outs=[couts[2][:]], replica_groups=[list(range(NCORE))])
        xfin = xsp.tile([64, NPAD], F16, tag="xsep")
        cb2 = make_window_cb(stack2, atomb_s[:, 1:2], None, 2, xsep2, xfin,
                             None, cinr, 128, molT_s[:], "plain", None, None)
        edge_phase(2, couts[2], dvx2, stack2, window_cb=cb2)
        nc.gpsimd.collective_compute("AllGather", OP.bypass, ins=[cinr[:]],
                                     outs=[coutr[:]], replica_groups=[list(range(NCORE))])
        # ---- readout: gather rows, resident rrow [xs | x], on-the-fly a_src
        rrow = big.tile([128, GCH, 128], F16, tag="rrow")
        asrc_ro = big.tile([128, GCH, 1], F32, tag="asr")
        RH = 8
        for b in range(GBLK):
            RGb = int(RG[b])
            gb0 = gbases[b]
            for r0 in range(0, RGb, RH):
                cr = min(RH, RGb - r0)
                c0 = gb0 + r0
                gr = grp_.tile([128, RH, 256], F16, tag="gr")
                nc.gpsimd.dma_gather(
                    gr[:, 0:cr], coutr[:], rgidx_s[:, c0 * 8:(c0 + cr) * 8],
                    cr * 128, cr * 128, 256, elem_step=256,
                    single_packet=False, queue_num=qn_state[0] % 4)
                qn_state[0] += 1
                sl = rrow[:, c0:c0 + cr]
                nc.vector.tensor_sub(sl, gr[:, 0:cr, 128:256], gr[:, 0:cr, 0:128])
                nc.vector.tensor_mul(sl, sl, rpar_s[:, c0:c0 + cr].to_broadcast([128, cr, 128]))
                nc.vector.tensor_add(sl, sl, gr[:, 0:cr, 0:128])
                nc.vector.tensor_mul(rrow[:, c0:c0 + cr, 64:128], rrow[:, c0:c0 + cr, 64:128],
                                     rmsk16_s[:, c0:c0 + cr].to_broadcast([128, cr, 64]))
                evr = gr[:, 0:cr, 0:64]  # dead (g0) half reused as scratch
                nc.vector.tensor_mul(evr, rrow[:, c0:c0 + cr, 0:64],
                                     msrc_s[:].to_broadcast([128, cr, 64]))
                nc.vector.tensor_reduce(asrc_ro[:, c0:c0 + cr], evr,
                                        mybir.AxisListType.X, OP.add)
        ofm = big.tile([64, GPAD], F32, tag="ofm")
        hro = big.tile([64, GPAD], F32, tag="hro")
        mol_stk = big.tile([128, GPAD], F16, tag="mstk")
        for b in range(GBLK):
            RGb = int(RG[b])
            pred = prp.tile([128, 64], F32, tag="pred")
            for j in range(RGb):
                nc.tensor.matmul(pred[:, 0:64], id16[:], rrow[:, gbases[b] + j, 64:128],
                                 start=(j == 0), stop=(j == RGb - 1))
            s0 = sp.tile([128, 64], F16, tag="hnm")
            nc.scalar.activation(s0[:], pred[:, 0:64], AF.Copy)
            ps = transp16(s0[:], 128, 64)
            nc.scalar.activation(ofm[:, b * 128:(b + 1) * 128], ps[0:64, 0:128], AF.Relu)
        wtil_ps = pp.tile([128, CH], F32, tag="mmq")
        nc.tensor.matmul(wtil_ps[0:64, 0:1], mol_lin_s[:], mdst_s[:], start=True, stop=True)
        wtil = cpool.tile([64, 1], F32)
        nc.vector.tensor_copy(wtil[:], wtil_ps[0:64, 0:1])
        rmsg = big.tile([128, GCH, 64], F16, tag="rmsg")
        for t in range(3):
            ddp = pp.tile([128, CH], F32, tag="mmq")
            nc.tensor.matmul(ddp[0:1, 0:GPAD], wtil[:], ofm[:], start=True, stop=True)
            dds = s1.tile([1, GPAD], F16, tag="dds")
            nc.scalar.activation(dds[:], ddp[0:1, 0:GPAD], AF.Copy)
            lgr = sp.tile([128, GCH, 1], F32, tag="lgr")
            for b in range(GBLK):
                RGb = int(RG[b])
                gb0 = gbases[b]
                psb = transp16(dds[:, b * 128:(b + 1) * 128], 1, 128)
                ddb = sp.tile([128, 1], F32, tag="ddb")
                nc.scalar.activation(ddb[:], psb[0:128, 0:1], AF.Copy)
                nc.vector.tensor_scalar(lgr[:, gb0:gb0 + RGb], asrc_ro[:, gb0:gb0 + RGb],
                                        ddb[:, 0:1], None, OP.add)
            nc.vector.scalar_tensor_tensor(lgr[:], lgr[:], NS, lgr[:], OP.mult, OP.max)
            nc.scalar.activation(lgr[:], lgr[:], AF.Exp)
            p16r = sp.tile([128, GCH, 1], F32, tag="p16r")
            nc.vector.tensor_mul(p16r[:], lgr[:], rmsk_s[:])
            nc.vector.tensor_mul(rmsg[:, :, 0:64], rrow[:, :, 0:64],
                                 p16r[:].to_broadcast([128, GCH, 64]))
            for b in range(GBLK):
                RGb = int(RG[b])
                gb0 = gbases[b]
                pred = prp.tile([128, 64], F32, tag="pred")
                for j in range(RGb):
                    nc.tensor.matmul(pred[:, 0:64], id16[:], rmsg[:, gb0 + j, 0:64],
                                     start=(j == 0), stop=(j == RGb - 1))
                rec = sp.tile([128, 1], F32, tag="rec")
                nc.vector.tensor_reduce(rec[:], p16r[:, gb0:gb0 + RGb], mybir.AxisListType.XY, OP.add)
                nc.vector.tensor_scalar(rec[:], rec[:], 1e-16, None, OP.add)
                nc.vector.reciprocal(rec[:], rec[:])
                hnm = sp.tile([128, 64], F16, tag="hnm")
                nc.scalar.activation(hnm[:], pred[:, 0:64], AF.Copy, scale=rec[:])
                ps = transp16(hnm[:], 128, 64)
                nc.scalar.activation(hro[:, b * 128:(b + 1) * 128], ps[0:64, 0:128], AF.Copy)
            elu_chunk(hro, molb_s[:], 0, GPAD, out_buf=mol_stk)
            nc.gpsimd.dma_start(mol_stk[64:128, :], ofm[:])
            gru(mol_stk, ofm, 3, ofm[:], GPAD)
        emb = s1.tile([64, GPAD], F16, tag="emb")
        fm_mm(emb[:], lin2T_s[:], ofm[:], GPAD, bias=lin2b_s[:])
        nemb = s1.tile([64, GPAD], F32, tag="nemb")
        for b in range(GBLK):
            gps = transp16(emb[:, b * 128:(b + 1) * 128], 64, 128)
            gm = sp.tile([128, 64], F32, tag="gm")
            nc.scalar.activation(gm[:], gps[0:128, 0:64], AF.Copy)
            mu = sp.tile([128, 1], F32, tag="mu")
            nc.vector.tensor_reduce(mu[:], gm[:, 0:64], mybir.AxisListType.X, OP.add)
            nc.vector.tensor_scalar(mu[:], mu[:], 1.0 / 64, None, OP.mult)
            xc = sp.tile([128, 64], F32, tag="xc")
            nc.vector.tensor_scalar(xc[:], gm[:, 0:64], mu[:], None, OP.subtract)
            sq = sp.tile([128, 64], F32, tag="sq")
            nc.scalar.activation(sq[:], xc[:], AF.Square)
            var = sp.tile([128, 1], F32, tag="var")
            nc.vector.tensor_reduce(var[:], sq[:], mybir.AxisListType.X, OP.add)
            nc.vector.tensor_scalar(var[:], var[:], 1.0 / 64, None, OP.mult)
            nc.vector.tensor_scalar(var[:], var[:], 1e-5, None, OP.add)
            nc.scalar.activation(var[:], var[:], AF.Sqrt)
            nc.vector.reciprocal(var[:], var[:])
            nc.scalar.activation(xc[:], xc[:], AF.Copy, scale=var[:])
            nc.vector.tensor_mul(xc[:], xc[:], lng_s[:, 0:64])
            nc.vector.tensor_add(xc[:], xc[:], lnb_s[:, 0:64])
            xc16 = sp.tile([128, 64], F16, tag="hnm")
            nc.scalar.activation(xc16[:], xc[:], AF.Copy)
            ps2 = transp16(xc16[:], 128, 64)
            nc.scalar.activation(nemb[:, b * 128:(b + 1) * 128], ps2[0:64, 0:128], AF.Copy)
        m1 = s1.tile([64, GPAD], F32, tag="m1")
        fm_mm(m1[:], h1T_s[:], nemb[:], GPAD, func=AF.Relu, bias=h1b_s[:])
        m2 = s1.tile([64, GPAD], F32, tag="m2")
        fm_mm(m2[:], h2T_s[:], m1[:], GPAD, func=AF.Relu, bias=h2b_s[:])
        yps = pp.tile([128, CH], F32, tag="mmq")
        nc.tensor.matmul(yps[0:1, 0:GPAD], h3T_s[:], m2[:], start=True, stop=True)
        ysb = s1.tile([1, GPAD], F32, tag="ysb")
        nc.scalar.activation(ysb[:], yps[0:1, 0:GPAD], AF.Identity, bias=h3b_s[:])
        nc.sync.dma_start(y_out[:], ysb[:])
        ctx.close()
    ctx2.__exit__(None, None, None)
    nc.finalize()
    return nc


_CACHE = {}


def kernel(**inputs):
    x = np.asarray(inputs["x"], np.float32)
    edge_attr = np.asarray(inputs["edge_attr"], np.float32)
    ei = np.asarray(inputs["edge_index"])
    batch = np.asarray(inputs["batch"])
    if "k" not in _CACHE:
        plan = build_plan(ei, batch)
        nc = build_nc(plan)
        _CACHE["k"] = (plan, nc)
    plan, nc = _CACHE["k"]

    gw = np.zeros((128, 4, 2, H), np.float16)
    gn = np.zeros((128, 4, 2, H), np.float16)
    gbx_rz = np.zeros((64, 8), np.float32)
    gbh_rz = np.zeros((64, 8), np.float32)
    gbx_n = np.zeros((H, 4), np.float32)
    gbh_n = np.zeros((H, 4), np.float32)
    packs = [
        (inputs["gru0_wx"], inputs["gru0_wh"], inputs["gru0_bx"], inputs["gru0_bh"]),
        (inputs["atom_gru_wx"][0], inputs["atom_gru_wh"][0], inputs["atom_gru_bx"][0], inputs["atom_gru_bh"][0]),
        (inputs["atom_gru_wx"][1], inputs["atom_gru_wh"][1], inputs["atom_gru_bx"][1], inputs["atom_gru_bh"][1]),
        (inputs["mol_gru_wx"], inputs["mol_gru_wh"], inputs["mol_gru_bx"], inputs["mol_gru_bh"]),
    ]
    for i, (wx, wh, bx, bh) in enumerate(packs):
        wx = np.asarray(wx, np.float32); wh = np.asarray(wh, np.float32)
        bx = np.asarray(bx, np.float32); bh = np.asarray(bh, np.float32)
        gw[0:64, i, 0] = wx[0:64].T; gw[64:128, i, 0] = wh[0:64].T
        gw[0:64, i, 1] = wx[64:128].T; gw[64:128, i, 1] = wh[64:128].T
        gn[0:64, i, 0] = wx[128:192].T; gn[64:128, i, 1] = wh[128:192].T
        gbx_rz[:, 2 * i] = bx[0:64]; gbx_rz[:, 2 * i + 1] = bx[64:128]
        gbh_rz[:, 2 * i] = bh[0:64]; gbh_rz[:, 2 * i + 1] = bh[64:128]
        gbx_n[:, i] = bx[128:192]; gbh_n[:, i] = bh[128:192]

    glw = np.asarray(inputs["gate_lin1_w"], np.float32)
    rep16 = lambda v: np.tile(np.asarray(v, np.float16).reshape(1, -1), (128, 1))
    rep32 = lambda v: np.tile(np.asarray(v, np.float32).reshape(1, -1), (128, 1))
    a = lambda k: np.asarray(inputs[k], np.float32)
    a16 = lambda k: np.asarray(inputs[k], np.float16)
    wts = dict(
        lin1T=a("lin1_w").T.copy(), lin1_b=a("lin1_b").reshape(H, 1),
        w1aT=glw[:, 0:64].T.astype(np.float16).copy(),
        w1bT=glw[:, 64:80].T.astype(np.float16).copy(),
        attl_rep=rep16(inputs["gate_att_l"]).reshape(128, 1, H),
        attr_rep=rep16(inputs["gate_att_r"]),
        g2T=a16("gate_lin2_w").T.copy(), gate_b=a("gate_bias").reshape(H, 1),
        atomT=np.stack([a16("atom_lin_w")[l].T for l in range(2)], 1),
        asrc_rep=np.stack([rep16(inputs["atom_att_src"][l]) for l in range(2)], 1),
        adst_rep=np.stack([rep16(inputs["atom_att_dst"][l]) for l in range(2)], 1),
        atom_b=a("atom_bias").T.copy(),
        molT=a16("mol_lin_w").T.copy(), mol_lin=a("mol_lin_w").copy(),
        matt_src_rep=rep16(inputs["mol_att_src"]).reshape(128, 1, H),
        matt_dst=a("mol_att_dst").reshape(H, 1),
        mol_b=a("mol_bias").reshape(H, 1),
        gruW=gw, gruN=gn, gbx_rz=gbx_rz, gbh_rz=gbh_rz, gbx_n=gbx_n, gbh_n=gbh_n,
        lin2T=a("lin2_w").T.copy(), lin2_b=a("lin2_b").reshape(H, 1),
        lng_rep=rep32(inputs["ln_g"]), lnb_rep=rep32(inputs["ln_b"]),
        h1T=a("h1_w").T.copy(), h1_b=a("h1_b").reshape(H, 1),
        h2T=a("h2_w").T.copy(), h2_b=a("h2_b").reshape(H, 1),
        h3T=a("h3_w").T.copy(), h3_b=a("h3_b").reshape(1, 1),
    )
    in_maps = []
    for c in range(NCORE):
        pc = plan["cores"][c]
        rc = plan["rcores"][c]
        xT = np.zeros((H, NPAD), np.float32)
        xT[:, 0:NPC] = x[c * NPC:(c + 1) * NPC][plan["perms"][c]].T
        eaT = np.zeros((16, plan["S"]), np.float16)
        val = pc["esel"] >= 0
        eaT[:, val] = edge_attr[pc["esel"][val]].T.astype(np.float16)
        im = dict(xT_in=xT, gidx=pc["gidx"], par=pc["par"], msk=pc["msk"], eaT=eaT,
                  rgidx=rc["rgidx"], rpar=rc["rpar"], rmsk=rc["rmsk"],
                  rmsk16=rc["rmsk16"], **wts)
        in_maps.append(im)
    res = run_bass_kernel_spmd(nc, in_maps, core_ids=list(range(NCORE)))
    if getattr(res, "exec_time_ns", None):
        print(f"HW exec time: {res.exec_time_ns} ns", flush=True)
    y = np.zeros(B, np.float32)
    for c in range(NCORE):
        ys = res.results[c]["y_out"].reshape(GPAD)[0:GPC]
        y[c * GPC + plan["gperms"][c]] = ys
    return y.reshape(B, 1)


# revision 37
# speedup vs baseline: 1.4255x; 1.0121x over previous
"""AttentiveFP forward on 8 Trainium2 NeuronCores (Bass/Tile).

Edges sharded by dst-owner core; per-core nodes sorted by in-degree with a
round-robin slot structure so segment softmax/sum become dense PSUM matmul
accumulation. Edge phase processed in large multi-block groups: one
dma_gather per ~34 rounds, wide DVE ops (parity select in place), DVE
leaky-relu (no ACT table ping-pong), single Exp per group. Layers 1-2 use
256B gather elements (64-f16 rows, att_src dot computed on the fly), halving
gather traffic and AllGather payload. fp16 node state feeds all PE matmuls.
"""
import numpy as np
from contextlib import ExitStack

import concourse.bass as bass
import concourse.tile as tile
from concourse import bacc, mybir
from concourse.bass_utils import run_bass_kernel_spmd
from concourse.masks import make_identity

F32 = mybir.dt.float32
F16 = mybir.dt.float16
I16 = mybir.dt.int16
AF = mybir.ActivationFunctionType
OP = mybir.AluOpType

NCORE = 8
N, E, B = 50000, 500000, 2048
H = 64
NS = 0.01
NPC = N // NCORE
GPC = B // NCORE
NBLK = (NPC + 127) // 128
NPAD = NBLK * 128
GBLK = (GPC + 127) // 128
GPAD = GBLK * 128
TROWS = NCORE * NPAD
MAXR = 10
CH = 512
GT = 16       # max rounds per edge-phase group


def _calls_for(R, maxr=MAXR):
    calls, base, bases = [], 0, []
    for r in R:
        bases.append(base)
        calls.append([(r0, min(r0 + maxr, int(r))) for r0 in range(0, int(r), maxr)])
        base += int(r)
    return calls, bases, base


def _wrap_into(gidx, arr, col0):
    n = arr.shape[0]
    blk = arr.reshape(n // 16, 16).T
    gidx[:16, col0:col0 + n // 16] = blk
    gidx[16:128, col0:col0 + n // 16] = np.tile(blk, (7, 1))


def build_plan(edge_index, batch):
    src = edge_index[0].astype(np.int64)
    dst = edge_index[1].astype(np.int64)
    owner = dst // NPC

    perms, degs_sorted, grp_starts, egrp = [], [], [], []
    sortpos = np.zeros(N, np.int64)
    for c in range(NCORE):
        n0 = c * NPC
        emask = np.nonzero(owner == c)[0]
        deg = np.bincount(dst[emask] - n0, minlength=NPC)
        order = np.argsort(-deg, kind="stable")
        perms.append(order)
        sortpos[n0 + order] = np.arange(NPC)
        dsorted = deg[order]
        degs_sorted.append(dsorted)
        eorder = np.argsort(sortpos[dst[emask]], kind="stable")
        egrp.append(emask[eorder])
        grp_starts.append(np.concatenate([[0], np.cumsum(dsorted)]))
    trow = (np.arange(N) // NPC) * NPAD + sortpos

    R = np.ones(NBLK, np.int64)
    for b in range(NBLK):
        for c in range(NCORE):
            d = degs_sorted[c][b * 128:(b + 1) * 128]
            if len(d):
                R[b] = max(R[b], int(d[0]))
    calls, bases, NCH = _calls_for(R)
    S = NCH * 128

    gsize = np.bincount(batch, minlength=B)
    gstart = np.concatenate([[0], np.cumsum(gsize)])
    gperms, gss = [], []
    for c in range(NCORE):
        gs = gsize[c * GPC:(c + 1) * GPC]
        gorder = np.argsort(-gs, kind="stable")
        gperms.append(gorder)
        gss.append(gs[gorder])
    RG = np.ones(GBLK, np.int64)
    for b in range(GBLK):
        for c in range(NCORE):
            d = gss[c][b * 128:(b + 1) * 128]
            if len(d):
                RG[b] = max(RG[b], int(d[0]))
    gcalls, gbases, GCH = _calls_for(RG, 8)
    SR = GCH * 128

    cores = []
    lanes = np.arange(128)
    for c in range(NCORE):
        gidx = np.zeros((128, S // 16), np.int16)
        par = np.zeros((128, NCH, 1), np.float16)
        msk = np.zeros((128, NCH, 1), np.float32)
        esel = np.full(NCH * 128, -1, np.int64)
        ds = degs_sorted[c]
        gst = grp_starts[c]
        eg = egrp[c]
        for b in range(NBLK):
            for (r0, r1) in calls[b]:
                ia = np.zeros((r1 - r0) * 128, np.int64)
                for r in range(r0, r1):
                    ch = bases[b] + r
                    p = b * 128 + lanes
                    pc = np.minimum(p, NPC - 1)
                    ok = (p < NPC) & (r < ds[pc])
                    eids = gst[pc] + r
                    ge = np.where(ok, eg[np.where(ok, np.minimum(eids, len(eg) - 1), 0)], -1)
                    esel[ch * 128 + lanes] = ge
                    rows = np.where(ok, trow[src[np.maximum(ge, 0)]], 0)
                    ia[(r - r0) * 128 + lanes] = rows >> 1
                    par[:, ch, 0] = (rows & 1).astype(np.float16)
                    msk[:, ch, 0] = ok.astype(np.float32)
                _wrap_into(gidx, ia, (bases[b] + r0) * 8)
        cores.append(dict(gidx=gidx, par=par, msk=msk, esel=esel))

    rcores = []
    for c in range(NCORE):
        rgidx = np.zeros((128, SR // 16), np.int16)
        rpar = np.zeros((128, GCH, 1), np.float16)
        rmsk = np.zeros((128, GCH, 1), np.float32)
        rmsk16 = np.zeros((128, GCH, 1), np.float16)
        gs = gss[c]
        gp = gperms[c]
        for b in range(GBLK):
            for (r0, r1) in gcalls[b]:
                ia = np.zeros((r1 - r0) * 128, np.int64)
                for r in range(r0, r1):
                    ch = gbases[b] + r
                    p = b * 128 + lanes
                    pc = np.minimum(p, GPC - 1)
                    ok = (p < GPC) & (r < gs[pc])
                    g = c * GPC + gp[pc]
                    node = np.where(ok, gstart[g] + r, 0)
                    rows = np.where(ok, trow[node], 0)
                    ia[(r - r0) * 128 + lanes] = rows >> 1
                    rpar[:, ch, 0] = (rows & 1).astype(np.float16)
                    rmsk[:, ch, 0] = ok.astype(np.float32)
                    rmsk16[:, ch, 0] = ok.astype(np.float16)
                _wrap_into(rgidx, ia, (gbases[b] + r0) * 8)
        rcores.append(dict(rgidx=rgidx, rpar=rpar, rmsk=rmsk, rmsk16=rmsk16))

    return dict(R=R, bases=bases, NCH=NCH, S=S,
                RG=RG, gbases=gbases, GCH=GCH, SR=SR,
                cores=cores, rcores=rcores, perms=perms, gperms=gperms)


def build_nc(plan):
    R, bases, NCH, S = plan["R"], plan["bases"], plan["NCH"], plan["S"]
    RG, gbases, GCH, SR = plan["RG"], plan["gbases"], plan["GCH"], plan["SR"]
    RGT = int(max(int(r) for r in RG))

    # split blocks into spans of <= GT rounds, pack spans into groups
    spans = []
    for b in range(NBLK):
        r = int(R[b])
        for s0 in range(0, r, GT):
            spans.append((b, s0, min(s0 + GT, r)))
    def pack_span_groups(cap):
        gs, cur, cur_r = [], [], 0
        for spn in spans:
            ln = spn[2] - spn[1]
            if cur and cur_r + ln > cap:
                gs.append(cur)
                cur, cur_r = [], 0
            cur.append(spn)
            cur_r += ln
        gs.append(cur)
        return gs
    sgroups = pack_span_groups(GT)
    GTS = max(sum(s[2] - s[1] for s in gl) for gl in sgroups)

    nc = bacc.Bacc("TRN2", target_bir_lowering=False, debug=False,
                   num_devices=NCORE, num_swdge_queues=4)

    def din(name, shape, dt=F32):
        return nc.dram_tensor(name, shape, dt, kind="ExternalInput")

    xT_in = din("xT_in", [H, NPAD])
    gidx_in = din("gidx", [128, S // 16], I16)
    par_in = din("par", [128, NCH, 1], F16)
    msk_in = din("msk", [128, NCH, 1], F32)
    eaT_in = din("eaT", [16, S], F16)
    rgidx_in = din("rgidx", [128, SR // 16], I16)
    rpar_in = din("rpar", [128, GCH, 1], F16)
    rmsk_in = din("rmsk", [128, GCH, 1], F32)
    rmsk16_in = din("rmsk16", [128, GCH, 1], F16)
    lin1T = din("lin1T", [H, H]); lin1_b = din("lin1_b", [H, 1])
    w1aT = din("w1aT", [H, H], F16); w1bT = din("w1bT", [16, H], F16)
    attl_rep = din("attl_rep", [128, 1, H], F16)
    attr_rep = din("attr_rep", [128, H], F16)
    g2T = din("g2T", [H, H], F16); gate_b = din("gate_b", [H, 1])
    atomT = din("atomT", [H, 2, H], F16)
    asrc_rep = din("asrc_rep", [128, 2, H], F16)
    adst_rep = din("adst_rep", [128, 2, H], F16)
    atom_b = din("atom_b", [H, 2])
    molT = din("molT", [H, H], F16)
    mol_lin = din("mol_lin", [H, H])
    matt_src_rep = din("matt_src_rep", [128, 1, H], F16)
    matt_dst = din("matt_dst", [H, 1])
    mol_b = din("mol_b", [H, 1])
    gruW = din("gruW", [128, 4, 2, H], F16)   # [K, widx, gate(r/z), M]
    gruN = din("gruN", [128, 4, 2, H], F16)   # [K, widx, (nx/nh), M] zero-padded
    gbx_rz = din("gbx_rz", [64, 8])
    gbh_rz = din("gbh_rz", [64, 8])
    gbx_n = din("gbx_n", [H, 4])
    gbh_n = din("gbh_n", [H, 4])
    lin2T = din("lin2T", [H, H]); lin2_b = din("lin2_b", [H, 1])
    lng_rep = din("lng_rep", [128, H]); lnb_rep = din("lnb_rep", [128, H])
    h1T = din("h1T", [H, H]); h1_b = din("h1_b", [H, 1])
    h2T = din("h2T", [H, H]); h2_b = din("h2_b", [H, 1])
    h3T = din("h3T", [H, 1]); h3_b = din("h3_b", [1, 1])

    y_out = nc.dram_tensor("y_out", [1, GPAD], F32, kind="ExternalOutput")

    cin0 = nc.dram_tensor("cin0", [NPAD, 128], F16)
    cout0 = nc.dram_tensor("cout0", [TROWS // 2, 256], F16, addr_space="Shared")
    cins = [cin0]
    couts = [cout0]
    for l in (1, 2):
        cins.append(nc.dram_tensor(f"cin{l}", [NPAD, 64], F16))
        couts.append(nc.dram_tensor(f"cout{l}", [TROWS // 2, 128], F16,
                                    addr_space="Shared"))
    cinr = nc.dram_tensor("cinr", [NPAD, 128], F16)
    coutr = nc.dram_tensor("coutr", [TROWS // 2, 256], F16, addr_space="Shared")

    ctx = ExitStack()
    ctx2 = nc.allow_low_precision(reason="fp16 edge tables/messages/state")
    ctx2.__enter__()
    with tile.TileContext(nc) as tc:
        cpool = ctx.enter_context(tc.tile_pool(name="const", bufs=1))
        wpool = ctx.enter_context(tc.tile_pool(name="wts", bufs=1))
        big = ctx.enter_context(tc.tile_pool(name="big", bufs=1))
        stkp = ctx.enter_context(tc.tile_pool(name="stkp", bufs=2))
        xsp = ctx.enter_context(tc.tile_pool(name="xsp", bufs=2))
        g0p = ctx.enter_context(tc.tile_pool(name="g0p", bufs=4))
        g12p = ctx.enter_context(tc.tile_pool(name="g12p", bufs=5))
        grp_ = ctx.enter_context(tc.tile_pool(name="gathr", bufs=3))
        eap = ctx.enter_context(tc.tile_pool(name="eap", bufs=1))
        sp = ctx.enter_context(tc.tile_pool(name="scr", bufs=2))
        s1 = ctx.enter_context(tc.tile_pool(name="scr1", bufs=1))
        pp = ctx.enter_context(tc.tile_pool(name="ps", bufs=2, space="PSUM"))
        prp = ctx.enter_context(tc.tile_pool(name="psp", bufs=2, space="PSUM"))
        rp = ctx.enter_context(tc.tile_pool(name="psr", bufs=2, space="PSUM"))
        zp = ctx.enter_context(tc.tile_pool(name="psz", bufs=2, space="PSUM"))

        id32 = cpool.tile([128, 128], F32)
        make_identity(nc, id32[:])
        id16 = cpool.tile([128, 128], F16)
        nc.vector.tensor_copy(id16[:], id32[:])

        def load(t, shape, dt=F32):
            s = wpool.tile(shape, dt, tag=f"w_{t.name}")
            nc.sync.dma_start(s[:], t[:])
            return s

        lin1T_s = load(lin1T, [H, H]); lin1b_s = load(lin1_b, [H, 1])
        w1aT_s = load(w1aT, [H, H], F16); w1bT_s = load(w1bT, [16, H], F16)
        attl_s = load(attl_rep, [128, 1, H], F16)
        attr_s = load(attr_rep, [128, H], F16)
        g2T_s = load(g2T, [H, H], F16); gateb_s = load(gate_b, [H, 1])
        atomT_s = load(atomT, [H, 2, H], F16)
        asrc_s = load(asrc_rep, [128, 2, H], F16)
        adst_s = load(adst_rep, [128, 2, H], F16)
        atomb_s = load(atom_b, [H, 2])
        molT_s = load(molT, [H, H], F16); mol_lin_s = load(mol_lin, [H, H])
        msrc_s = load(matt_src_rep, [128, 1, H], F16)
        mdst_s = load(matt_dst, [H, 1])
        molb_s = load(mol_b, [H, 1])
        gruW_s = load(gruW, [128, 4, 2, H], F16)
        gruN_s = load(gruN, [128, 4, 2, H], F16)
        gbxrz_s = load(gbx_rz, [64, 8])
        gbhrz_s = load(gbh_rz, [64, 8])
        gbxn_s = load(gbx_n, [H, 4])
        gbhn_s = load(gbh_n, [H, 4])
        lin2T_s = load(lin2T, [H, H]); lin2b_s = load(lin2_b, [H, 1])
        lng_s = load(lng_rep, [128, H]); lnb_s = load(lnb_rep, [128, H])
        h1T_s = load(h1T, [H, H]); h1b_s = load(h1_b, [H, 1])
        h2T_s = load(h2T, [H, H]); h2b_s = load(h2_b, [H, 1])
        h3T_s = load(h3T, [H, 1]); h3b_s = load(h3_b, [1, 1])
        gidx_s = load(gidx_in, [128, S // 16], I16)
        rgidx_s = load(rgidx_in, [128, SR // 16], I16)
        par_s = load(par_in, [128, NCH, 1], F16)
        msk_s = load(msk_in, [128, NCH, 1], F32)
        rpar_s = load(rpar_in, [128, GCH, 1], F16)
        rmsk_s = load(rmsk_in, [128, GCH, 1], F32)
        rmsk16_s = load(rmsk16_in, [128, GCH, 1], F16)

        def fm_mm(out_ap, lhsT, rhs, ncols, func=None, bias=0.0):
            M = lhsT.shape[-1]
            for c0 in range(0, ncols, CH):
                w = min(CH, ncols - c0)
                ps = pp.tile([128, CH], F32, tag="mmq")
                nc.tensor.matmul(ps[0:M, :w], lhsT, rhs[:, c0:c0 + w],
                                 start=True, stop=True)
                f = func
                if f is None:
                    f = AF.Copy if isinstance(bias, float) else AF.Identity
                nc.scalar.activation(out_ap[:, c0:c0 + w], ps[0:M, :w], f, bias=bias)

        def gru_bias(widx):
            br = sp.tile([64, 1], F32, tag=f"brz{widx}")
            nc.vector.tensor_add(br[:], gbxrz_s[:, 2 * widx:2 * widx + 1], gbhrz_s[:, 2 * widx:2 * widx + 1])
            bz = sp.tile([64, 1], F32, tag=f"bz{widx}")
            nc.vector.tensor_add(bz[:], gbxrz_s[:, 2 * widx + 1:2 * widx + 2], gbhrz_s[:, 2 * widx + 1:2 * widx + 2])
            return br, bz

        def gru_chunk(stk, xprev, widx, out_ap, c0, w, br, bz):
            if True:
                prz = pp.tile([128, CH], F32, tag="mmq")
                nc.tensor.matmul(prz[0:64, :w], gruW_s[:, widx, 0], stk[:, c0:c0 + w], start=True, stop=True)
                prz2 = pp.tile([128, CH], F32, tag="mmq")
                nc.tensor.matmul(prz2[0:64, :w], gruW_s[:, widx, 1], stk[:, c0:c0 + w], start=True, stop=True)
                rz = sp.tile([64, CH], F32, tag="rz")
                nc.scalar.activation(rz[:, :w], prz[0:64, :w], AF.Sigmoid, bias=br[:])
                zz = sp.tile([64, CH], F32, tag="zz")
                nc.scalar.activation(zz[:, :w], prz2[0:64, :w], AF.Sigmoid, bias=bz[:])
                pn1 = pp.tile([128, CH], F32, tag="mmq")
                nc.tensor.matmul(pn1[0:64, :w], gruN_s[:, widx, 0], stk[:, c0:c0 + w], start=True, stop=True)
                pn2 = pp.tile([128, CH], F32, tag="mmq")
                nc.tensor.matmul(pn2[0:64, :w], gruN_s[:, widx, 1], stk[:, c0:c0 + w], start=True, stop=True)
                hnb = sp.tile([64, CH], F32, tag="hnb")
                nc.scalar.activation(hnb[:, :w], pn2[0:64, :w], AF.Identity, bias=gbhn_s[:, widx:widx + 1])
                nc.vector.tensor_mul(hnb[:, :w], rz[:, :w], hnb[:, :w])
                nc.vector.tensor_add(hnb[:, :w], hnb[:, :w], pn1[0:64, :w])
                nn = sp.tile([64, CH], F32, tag="nn")
                nc.scalar.activation(nn[:, :w], hnb[:, :w], AF.Tanh, bias=gbxn_s[:, widx:widx + 1])
                d = rz  # rz buffer is dead past the mul above
                nc.vector.tensor_sub(d[:, :w], xprev[:, c0:c0 + w], nn[:, :w])
                nc.vector.tensor_mul(d[:, :w], zz[:, :w], d[:, :w])
                nc.vector.tensor_add(d[:, :w], nn[:, :w], d[:, :w])
                nc.scalar.activation(out_ap[:, c0:c0 + w], d[:, :w], AF.Relu)

        def gru(stk, xprev, widx, out_ap, ncols):
            br, bz = gru_bias(widx)
            for c0 in range(0, ncols, CH):
                gru_chunk(stk, xprev, widx, out_ap, c0, min(CH, ncols - c0), br, bz)

        def elu_chunk(buf, bias_ap, c0, w, pre_lhsT=None, out_buf=None):
            if pre_lhsT is not None:
                ps = pp.tile([128, CH], F32, tag="mmq")
                nc.tensor.matmul(ps[0:64, :w], pre_lhsT, buf[0:64, c0:c0 + w], start=True, stop=True)
                src = ps[0:64, :w]
            else:
                src = buf[0:64, c0:c0 + w]
            e1 = sp.tile([64, CH], F32, tag="rz")
            nc.scalar.activation(e1[:, :w], src, AF.Exp, bias=bias_ap)
            t1 = sp.tile([64, CH], F32, tag="zz")
            nc.scalar.activation(t1[:, :w], e1[:, :w], AF.Relu, bias=1.0, scale=-1.0)
            t2 = sp.tile([64, CH], F32, tag="hnb")
            nc.scalar.activation(t2[:, :w], src, AF.Relu, bias=bias_ap)
            dst = buf if out_buf is None else out_buf
            nc.vector.tensor_sub(dst[0:64, c0:c0 + w], t2[:, :w], t1[:, :w])

        def elu_inplace(buf, bias_ap, ncols, pre_lhsT=None):
            """buf[0:64, :] = elu((pre_lhsT.T @ buf[0:64]) + bias)."""
            for c0 in range(0, ncols, CH):
                elu_chunk(buf, bias_ap, c0, min(CH, ncols - c0), pre_lhsT)

        qn_state = [0]

        def edge_phase(layer, table, dvals, hdst, window_cb=None):
            """hdst[0:64, :NPAD] (f16) <- normalized aggregation (feature-major)."""
            W = 256 if layer == 0 else 128
            half = W // 2
            pool = g0p if layer == 0 else g12p
            gtile = GTS
            p16L = big.tile([128, NCH, 1], F32, tag=f"p16L{layer}")
            preds = {}
            for glist in sgroups:
                ch0 = bases[glist[0][0]] + glist[0][1]
                Gr = sum(s[2] - s[1] for s in glist)
                g = pool.tile([128, gtile, W], F16, tag=f"g{min(layer, 1)}")
                nc.gpsimd.dma_gather(
                    g[:, 0:Gr], table[:], gidx_s[:, ch0 * 8:(ch0 + Gr) * 8],
                    Gr * 128, Gr * 128, W, elem_step=W,
                    single_packet=False, queue_num=qn_state[0] % 4)
                qn_state[0] += 1
                # parity select in place: row -> g[:, :, half:W]
                nc.vector.tensor_sub(g[:, 0:Gr, half:W], g[:, 0:Gr, half:W], g[:, 0:Gr, 0:half])
                nc.vector.tensor_mul(g[:, 0:Gr, half:W], g[:, 0:Gr, half:W],
                                     par_s[:, ch0:ch0 + Gr].to_broadcast([128, Gr, half]))
                nc.vector.tensor_add(g[:, 0:Gr, half:W], g[:, 0:Gr, half:W], g[:, 0:Gr, 0:half])
                lg = sp.tile([128, gtile, 1], F32, tag="lg")
                ev = g[:, 0:Gr, 0:64]  # dead (g0) half reused as scratch
                if layer == 0:
                    for r0 in range(0, Gr, 8):
                        r1 = min(r0 + 8, Gr)
                        ea = eap.tile([16, 8 * 128], F16, tag="ea")
                        nc.sync.dma_start(ea[:, 0:(r1 - r0) * 128],
                                          eaT_in[:, (ch0 + r0) * 128:(ch0 + r1) * 128])
                        pz = zp.tile([128, 8, 64], F32, tag="z1")
                        for r in range(r0, r1):
                            nc.tensor.matmul(pz[:, r - r0], ea[:, (r - r0) * 128:(r - r0 + 1) * 128],
                                             w1bT_s[:], start=True, stop=True)
                        nc.vector.tensor_add(g[:, r0:r1, 192:256], g[:, r0:r1, 192:256],
                                             pz[:, 0:r1 - r0])
                        nc.vector.scalar_tensor_tensor(g[:, r0:r1, 192:256], g[:, r0:r1, 192:256],
                                                       NS, g[:, r0:r1, 192:256], OP.mult, OP.max)
                    nc.vector.tensor_mul(ev, g[:, 0:Gr, 192:256],
                                         attl_s[:].to_broadcast([128, Gr, 64]))
                else:
                    nc.vector.tensor_mul(ev, g[:, 0:Gr, 64:128],
                                         asrc_s[:, layer - 1:layer].to_broadcast([128, Gr, 64]))
                nc.vector.tensor_reduce(lg[:, 0:Gr], ev, mybir.AxisListType.X, OP.add)
                nc.vector.tensor_add(lg[:, 0:Gr], lg[:, 0:Gr], dvals[:, ch0:ch0 + Gr])
                nc.vector.scalar_tensor_tensor(lg[:, 0:Gr], lg[:, 0:Gr], NS, lg[:, 0:Gr],
                                               OP.mult, OP.max)
                nc.scalar.activation(lg[:, 0:Gr], lg[:, 0:Gr], AF.Exp)
                p16 = p16L[:, ch0:ch0 + Gr]
                nc.vector.tensor_mul(p16, lg[:, 0:Gr], msk_s[:, ch0:ch0 + Gr])
                # msg: x part scaled in place; denominator via per-block reduce
                nc.vector.tensor_mul(g[:, 0:Gr, half:half + 64], g[:, 0:Gr, half:half + 64],
                                     p16.to_broadcast([128, Gr, 64]))
                o = 0
                done_b = 0
                for (b, r0, r1) in glist:
                    Rb = int(R[b])
                    if b not in preds:
                        predt = prp.tile([128, 64], F32, tag="pred")
                        preds[b] = predt
                    pred = preds[b]
                    for j in range(r1 - r0):
                        nc.tensor.matmul(pred[:, 0:64], id16[:], g[:, o + j, half:half + 64],
                                         start=(r0 + j == 0), stop=(r0 + j == Rb - 1))
                    o += r1 - r0
                    if r1 < Rb:
                        continue
                    del preds[b]
                    done_b = b + 1
                    rec = sp.tile([128, 1], F32, tag="rec")
                    nc.vector.tensor_reduce(rec[:], p16L[:, bases[b]:bases[b] + Rb],
                                            mybir.AxisListType.XY, OP.add)
                    nc.vector.tensor_scalar(rec[:], rec[:], 1e-16, None, OP.add)
                    nc.vector.reciprocal(rec[:], rec[:])
                    hnm = sp.tile([128, 64], F16, tag="hnm")
                    nc.scalar.activation(hnm[:], pred[:, 0:64], AF.Copy, scale=rec[:])
                    ps = transp16(hnm[:], 128, 64)
                    nc.scalar.activation(hdst[0:64, b * 128:(b + 1) * 128], ps[0:64, 0:128], AF.Copy)
                if window_cb is not None and done_b:
                    window_cb(done_b)

        def transp16(in_ap, a, bdim):
            """Transpose [a, bdim] f16 SBUF -> PSUM [bdim, a]."""
            ps = rp.tile([128, 128], F16, tag="tp16")
            nc.tensor.transpose(ps[0:bdim, 0:a], in_ap, id16[0:a, 0:a])
            return ps

        def build_rows(cin_t, width, x16, lhsT_A, lhsT_B, dst_rep, dvals, dvx=None):
            """Per block: rows[:, 0:64] = (A @ x_b).T (A None -> x_b.T);
            cols 64:128 = (B @ x_b).T (B 'plain' -> x_b.T). dst_rep -> dvals."""
            cview = cin_t[:].rearrange("(b p) e -> b p e", p=128)
            for b in range(NBLK):
                build_rows_block(cview, width, x16, b, lhsT_A, lhsT_B, dst_rep, dvals, dvx)

        def build_rows_block(cview, width, x16, b, lhsT_A, lhsT_B, dst_rep, dvals, dvx=None):
            rows_b = sp.tile([128, 128], F16, tag="rb")
            xb = x16[:, b * 128:(b + 1) * 128]
            if lhsT_A is not None:
                psm = pp.tile([128, CH], F32, tag="mmq")
                nc.tensor.matmul(psm[0:64, 0:128], lhsT_A, xb, start=True, stop=True)
                xa = sp.tile([64, 128], F16, tag="xa")
                nc.scalar.activation(xa[:], psm[0:64, 0:128], AF.Copy)
                ps = transp16(xa[:], 64, 128)
            else:
                ps = transp16(xb, 64, 128)
            nc.scalar.activation(rows_b[:, 0:64], ps[0:128, 0:64], AF.Copy)
            if lhsT_B is not None:
                if isinstance(lhsT_B, str):
                    ps2 = transp16(xb, 64, 128)
                else:
                    psm2 = pp.tile([128, CH], F32, tag="mmq")
                    nc.tensor.matmul(psm2[0:64, 0:128], lhsT_B, xb, start=True, stop=True)
                    xa2 = sp.tile([64, 128], F16, tag="xa")
                    nc.scalar.activation(xa2[:], psm2[0:64, 0:128], AF.Copy)
                    ps2 = transp16(xa2[:], 64, 128)
                nc.scalar.activation(rows_b[:, 64:128], ps2[0:128, 0:64], AF.Copy)
            if dst_rep is not None:
                m = sp.tile([128, H], F32, tag="dvm")
                nc.vector.tensor_mul(m[:], rows_b[:, 0:64], dst_rep)
                nc.vector.tensor_reduce(dvals[:, b:b + 1], m[:], mybir.AxisListType.X, OP.add)
                Rb = int(R[b])
                nc.scalar.activation(dvx[:, bases[b]:bases[b] + Rb],
                                     msk_s[:, bases[b]:bases[b] + Rb], AF.Identity,
                                     scale=0.0, bias=dvals[:, b:b + 1])
            nc.sync.dma_start(cview[b], rows_b[:, 0:width])

        def make_window_cb(xcur_t, elu_bias, elu_pre, widx, xsep_cur_t, xsep_n_t,
                           xnew_t, cin_t, width, lA, lB, drep, dv_t, dvx_t=None):
            """After edge-phase blocks land in xcur_t[0:64], run the node phase
            (elu+gru) and next layer's row build for completed 512-col windows."""
            cview = cin_t[:].rearrange("(b p) e -> b p e", p=128)
            br, bz = gru_bias(widx)
            state = {"done": 0}

            def cb(done):
                while state["done"] + 4 <= done or (done >= NBLK and state["done"] < NBLK):
                    w0 = state["done"]
                    w1 = min(w0 + 4, NBLK)
                    c0 = w0 * 128
                    w = (w1 - w0) * 128
                    elu_chunk(xcur_t, elu_bias, c0, w, pre_lhsT=elu_pre)
                    gru_chunk(xcur_t, xsep_cur_t[:], widx, xsep_n_t[:], c0, w, br, bz)
                    if xnew_t is not None:
                        nc.sync.dma_start(xnew_t[64:128, c0:c0 + w], xsep_n_t[:, c0:c0 + w])
                    for b in range(w0, w1):
                        build_rows_block(cview, width, xsep_n_t[:], b, lA, lB, drep, dv_t, dvx_t)
                    state["done"] = w1
            return cb

        # ================== forward ==================
        stack0 = stkp.tile([128, NPAD], F16, tag="stk")
        xsep0 = xsp.tile([64, NPAD], F16, tag="xsep")
        dv0 = big.tile([128, NBLK, 1], F32, tag="rvals0")
        dvx0 = big.tile([128, NCH, 1], F32, tag="rvx0")
        cview0 = cins[0][:].rearrange("(b p) e -> b p e", p=128)
        for c0 in range(0, NPAD, CH):
            w = min(CH, NPAD - c0)
            xst = sp.tile([64, CH], F32, tag="nn")
            nc.sync.dma_start(xst[:, 0:w], xT_in[:, c0:c0 + w])
            fm_mm(xsep0[:, c0:c0 + w], lin1T_s[:], xst[:, 0:w], w, func=AF.Lrelu, bias=lin1b_s[:])
            nc.sync.dma_start(stack0[64:128, c0:c0 + w], xsep0[:, c0:c0 + w])
            for b in range(c0 // 128, min((c0 + w) // 128, NBLK)):
                build_rows_block(cview0, 128, xsep0[:], b, None, w1aT_s[:], attr_s[:], dv0, dvx0)
        nc.gpsimd.collective_compute("AllGather", OP.bypass, ins=[cins[0][:]],
                                     outs=[couts[0][:]], replica_groups=[list(range(NCORE))])
        stack1 = stkp.tile([128, NPAD], F16, tag="stk")
        xsep1 = xsp.tile([64, NPAD], F16, tag="xsep")
        dv1 = big.tile([128, NBLK, 1], F32, tag="rvals1")
        dvx1 = big.tile([128, NCH, 1], F32, tag="rvx1")
        cb0 = make_window_cb(stack0, gateb_s[:], g2T_s[:], 0, xsep0, xsep1,
                             stack1, cins[1], 64, atomT_s[:, 0], None, adst_s[:, 0], dv1, dvx1)
        edge_phase(0, couts[0], dvx0, stack0, window_cb=cb0)
        nc.gpsimd.collective_compute("AllGather", OP.bypass, ins=[cins[1][:]],
                                     outs=[couts[1][:]], replica_groups=[list(range(NCORE))])
        stack2 = stkp.tile([128, NPAD], F16, tag="stk")
        xsep2 = xsp.tile([64, NPAD], F16, tag="xsep")
        dv2 = big.tile([128, NBLK, 1], F32, tag="rvals2")
        dvx2 = big.tile([128, NCH, 1], F32, tag="rvx2")
        cb1 = make_window_cb(stack1, atomb_s[:, 0:1], None, 1, xsep1, xsep2,
                             stack2, cins[2], 64, atomT_s[:, 1], None, adst_s[:, 1], dv2, dvx2)
        edge_phase(1, couts[1], dvx1, stack1, window_cb=cb1)
        nc.gpsimd.collective_compute("AllGather", OP.bypass, ins=[cins[2][:]],
                                     outs=[couts[2][:]], replica_groups=[list(range(NCORE))])
        xfin = xsp.tile([64, NPAD], F16, tag="xsep")
        cb2 = make_window_cb(stack2, atomb_s[:, 1:2], None, 2, xsep2, xfin,
                             None, cinr, 128, molT_s[:], "plain", None, None)
        edge_phase(2, couts[2], dvx2, stack2, window_cb=cb2)
        nc.gpsimd.collective_compute("AllGather", OP.bypass, ins=[cinr[:]],
                                     outs=[coutr[:]], replica_groups=[list(range(NCORE))])
        # ---- readout: gather rows, resident rrow [xs | x], on-the-fly a_src
        rrow = big.tile([128, GCH, 128], F16, tag="rrow")
        asrc_ro = big.tile([128, GCH, 1], F32, tag="asr")
        ofm = big.tile([64, GPAD], F32, tag="ofm")
        hro = big.tile([64, GPAD], F32, tag="hro")
        mol_stk = big.tile([128, GPAD], F16, tag="mstk")
        def out0_block(b):
            RGb = int(RG[b])
            pred = prp.tile([128, 64], F32, tag="pred")
            for j in range(RGb):
                nc.tensor.matmul(pred[:, 0:64], id16[:], rrow[:, gbases[b] + j, 64:128],
                                 start=(j == 0), stop=(j == RGb - 1))
            s0 = sp.tile([128, 64], F16, tag="hnm")
            nc.scalar.activation(s0[:], pred[:, 0:64], AF.Copy)
            ps = transp16(s0[:], 128, 64)
            nc.scalar.activation(ofm[:, b * 128:(b + 1) * 128], ps[0:64, 0:128], AF.Relu)
        RH = 8
        for b in range(GBLK):
            RGb = int(RG[b])
            gb0 = gbases[b]
            for r0 in range(0, RGb, RH):
                cr = min(RH, RGb - r0)
                c0 = gb0 + r0
                gr = grp_.tile([128, RH, 256], F16, tag="gr")
                nc.gpsimd.dma_gather(
                    gr[:, 0:cr], coutr[:], rgidx_s[:, c0 * 8:(c0 + cr) * 8],
                    cr * 128, cr * 128, 256, elem_step=256,
                    single_packet=False, queue_num=qn_state[0] % 4)
                qn_state[0] += 1
                sl = rrow[:, c0:c0 + cr]
                nc.vector.tensor_sub(sl, gr[:, 0:cr, 128:256], gr[:, 0:cr, 0:128])
                nc.vector.tensor_mul(sl, sl, rpar_s[:, c0:c0 + cr].to_broadcast([128, cr, 128]))
                nc.vector.tensor_add(sl, sl, gr[:, 0:cr, 0:128])
                nc.vector.tensor_mul(rrow[:, c0:c0 + cr, 64:128], rrow[:, c0:c0 + cr, 64:128],
                                     rmsk16_s[:, c0:c0 + cr].to_broadcast([128, cr, 64]))
                evr = gr[:, 0:cr, 0:64]  # dead (g0) half reused as scratch
                nc.vector.tensor_mul(evr, rrow[:, c0:c0 + cr, 0:64],
                                     msrc_s[:].to_broadcast([128, cr, 64]))
                nc.vector.tensor_reduce(asrc_ro[:, c0:c0 + cr], evr,
                                        mybir.AxisListType.X, OP.add)
            out0_block(b)
        wtil_ps = pp.tile([128, CH], F32, tag="mmq")
        nc.tensor.matmul(wtil_ps[0:64, 0:1], mol_lin_s[:], mdst_s[:], start=True, stop=True)
        wtil = cpool.tile([64, 1], F32)
        nc.vector.tensor_copy(wtil[:], wtil_ps[0:64, 0:1])
        rmsg = big.tile([128, GCH, 64], F16, tag="rmsg")
        for t in range(3):
            ddp = pp.tile([128, CH], F32, tag="mmq")
            nc.tensor.matmul(ddp[0:1, 0:GPAD], wtil[:], ofm[:], start=True, stop=True)
            dds = s1.tile([1, GPAD], F16, tag="dds")
            nc.scalar.activation(dds[:], ddp[0:1, 0:GPAD], AF.Copy)
            lgr = sp.tile([128, GCH, 1], F32, tag="lgr")
            for b in range(GBLK):
                RGb = int(RG[b])
                gb0 = gbases[b]
                psb = transp16(dds[:, b * 128:(b + 1) * 128], 1, 128)
                ddb = sp.tile([128, 1], F32, tag="ddb")
                nc.scalar.activation(ddb[:], psb[0:128, 0:1], AF.Copy)
                nc.vector.tensor_scalar(lgr[:, gb0:gb0 + RGb], asrc_ro[:, gb0:gb0 + RGb],
                                        ddb[:, 0:1], None, OP.add)
            nc.vector.scalar_tensor_tensor(lgr[:], lgr[:], NS, lgr[:], OP.mult, OP.max)
            nc.scalar.activation(lgr[:], lgr[:], AF.Exp)
            p16r = sp.tile([128, GCH, 1], F32, tag="p16r")
            nc.vector.tensor_mul(p16r[:], lgr[:], rmsk_s[:])
            nc.vector.tensor_mul(rmsg[:, :, 0:64], rrow[:, :, 0:64],
                                 p16r[:].to_broadcast([128, GCH, 64]))
            for b in range(GBLK):
                RGb = int(RG[b])
                gb0 = gbases[b]
                pred = prp.tile([128, 64], F32, tag="pred")
                for j in range(RGb):
                    nc.tensor.matmul(pred[:, 0:64], id16[:], rmsg[:, gb0 + j, 0:64],
                                     start=(j == 0), stop=(j == RGb - 1))
                rec = sp.tile([128, 1], F32, tag="rec")
                nc.vector.tensor_reduce(rec[:], p16r[:, gb0:gb0 + RGb], mybir.AxisListType.XY, OP.add)
                nc.vector.tensor_scalar(rec[:], rec[:], 1e-16, None, OP.add)
                nc.vector.reciprocal(rec[:], rec[:])
                hnm = sp.tile([128, 64], F16, tag="hnm")
                nc.scalar.activation(hnm[:], pred[:, 0:64], AF.Copy, scale=rec[:])
                ps = transp16(hnm[:], 128, 64)
                nc.scalar.activation(hro[:, b * 128:(b + 1) * 128], ps[0:64, 0:128], AF.Copy)
            elu_chunk(hro, molb_s[:], 0, GPAD, out_buf=mol_stk)
            nc.gpsimd.dma_start(mol_stk[64:128, :], ofm[:])
            gru(mol_stk, ofm, 3, ofm[:], GPAD)
        emb = s1.tile([64, GPAD], F16, tag="emb")
        fm_mm(emb[:], lin2T_s[:], ofm[:], GPAD, bias=lin2b_s[:])
        nemb = s1.tile([64, GPAD], F32, tag="nemb")
        for b in range(GBLK):
            gps = transp16(emb[:, b * 128:(b + 1) * 128], 64, 128)
            gm = sp.tile([128, 64], F32, tag="gm")
            nc.scalar.activation(gm[:], gps[0:128, 0:64], AF.Copy)
            mu = sp.tile([128, 1], F32, tag="mu")
            nc.vector.tensor_reduce(mu[:], gm[:, 0:64], mybir.AxisListType.X, OP.add)
            nc.vector.tensor_scalar(mu[:], mu[:], 1.0 / 64, None, OP.mult)
            xc = sp.tile([128, 64], F32, tag="xc")
            nc.vector.tensor_scalar(xc[:], gm[:, 0:64], mu[:], None, OP.subtract)
            sq = sp.tile([128, 64], F32, tag="sq")
            nc.scalar.activation(sq[:], xc[:], AF.Square)
            var = sp.tile([128, 1], F32, tag="var")
            nc.vector.tensor_reduce(var[:], sq[:], mybir.AxisListType.X, OP.add)
            nc.vector.tensor_scalar(var[:], var[:], 1.0 / 64, None, OP.mult)
            nc.vector.tensor_scalar(var[:], var[:], 1e-5, None, OP.add)
            nc.scalar.activation(var[:], var[:], AF.Sqrt)
            nc.vector.reciprocal(var[:], var[:])
            nc.scalar.activation(xc[:], xc[:], AF.Copy, scale=var[:])
            nc.vector.tensor_mul(xc[:], xc[:], lng_s[:, 0:64])
            nc.vector.tensor_add(xc[:], xc[:], lnb_s[:, 0:64])
            xc16 = sp.tile([128, 64], F16, tag="hnm")
            nc.scalar.activation(xc16[:], xc[:], AF.Copy)
            ps2 = transp16(xc16[:], 128, 64)
            nc.scalar.activation(nemb[:, b * 128:(b + 1) * 128], ps2[0:64, 0:128], AF.Copy)
        m1 = s1.tile([64, GPAD], F32, tag="m1")
        fm_mm(m1[:], h1T_s[:], nemb[:], GPAD, func=AF.Relu, bias=h1b_s[:])
        m2 = s1.tile([64, GPAD], F32, tag="m2")
        fm_mm(m2[:], h2T_s[:], m1[:], GPAD, func=AF.Relu, bias=h2b_s[:])
        yps = pp.tile([128, CH], F32, tag="mmq")
        nc.tensor.matmul(yps[0:1, 0:GPAD], h3T_s[:], m2[:], start=True, stop=True)
        ysb = s1.tile([1, GPAD], F32, tag="ysb")
        nc.scalar.activation(ysb[:], yps[0:1, 0:GPAD], AF.Identity, bias=h3b_s[:])
        nc.sync.dma_start(y_out[:], ysb[:])
        ctx.close()
    ctx2.__exit__(None, None, None)
    nc.finalize()
    return nc


_CACHE = {}


def kernel(**inputs):
    x = np.asarray(inputs["x"], np.float32)
    edge_attr = np.asarray(inputs["edge_attr"], np.float32)
    ei = np.asarray(inputs["edge_index"])
    batch = np.asarray(inputs["batch"])
    if "k" not in _CACHE:
        plan = build_plan(ei, batch)
        nc = build_nc(plan)
        _CACHE["k"] = (plan, nc)
    plan, nc = _CACHE["k"]

    gw = np.zeros((128, 4, 2, H), np.float16)
    gn = np.zeros((128, 4, 2, H), np.float16)
    gbx_rz = np.zeros((64, 8), np.float32)
    gbh_rz = np.zeros((64, 8), np.float32)
    gbx_n = np.zeros((H, 4), np.float32)
    gbh_n = np.zeros((H, 4), np.float32)
    packs = [
        (inputs["gru0_wx"], inputs["gru0_wh"], inputs["gru0_bx"], inputs["gru0_bh"]),
        (inputs["atom_gru_wx"][0], inputs["atom_gru_wh"][0], inputs["atom_gru_bx"][0], inputs["atom_gru_bh"][0]),
        (inputs["atom_gru_wx"][1], inputs["atom_gru_wh"][1], inputs["atom_gru_bx"][1], inputs["atom_gru_bh"][1]),
        (inputs["mol_gru_wx"], inputs["mol_gru_wh"], inputs["mol_gru_bx"], inputs["mol_gru_bh"]),
    ]
    for i, (wx, wh, bx, bh) in enumerate(packs):
        wx = np.asarray(wx, np.float32); wh = np.asarray(wh, np.float32)
        bx = np.asarray(bx, np.float32); bh = np.asarray(bh, np.float32)
        gw[0:64, i, 0] = wx[0:64].T; gw[64:128, i, 0] = wh[0:64].T
        gw[0:64, i, 1] = wx[64:128].T; gw[64:128, i, 1] = wh[64:128].T
        gn[0:64, i, 0] = wx[128:192].T; gn[64:128, i, 1] = wh[128:192].T
        gbx_rz[:, 2 * i] = bx[0:64]; gbx_rz[:, 2 * i + 1] = bx[64:128]
        gbh_rz[:, 2 * i] = bh[0:64]; gbh_rz[:, 2 * i + 1] = bh[64:128]
        gbx_n[:, i] = bx[128:192]; gbh_n[:, i] = bh[128:192]

    glw = np.asarray(inputs["gate_lin1_w"], np.float32)
    rep16 = lambda v: np.tile(np.asarray(v, np.float16).reshape(1, -1), (128, 1))
    rep32 = lambda v: np.tile(np.asarray(v, np.float32).reshape(1, -1), (128, 1))
    a = lambda k: np.asarray(inputs[k], np.float32)
    a16 = lambda k: np.asarray(inputs[k], np.float16)
    wts = dict(
        lin1T=a("lin1_w").T.copy(), lin1_b=a("lin1_b").reshape(H, 1),
        w1aT=glw[:, 0:64].T.astype(np.float16).copy(),
        w1bT=glw[:, 64:80].T.astype(np.float16).copy(),
        attl_rep=rep16(inputs["gate_att_l"]).reshape(128, 1, H),
        attr_rep=rep16(inputs["gate_att_r"]),
        g2T=a16("gate_lin2_w").T.copy(), gate_b=a("gate_bias").reshape(H, 1),
        atomT=np.stack([a16("atom_lin_w")[l].T for l in range(2)], 1),
        asrc_rep=np.stack([rep16(inputs["atom_att_src"][l]) for l in range(2)], 1),
        adst_rep=np.stack([rep16(inputs["atom_att_dst"][l]) for l in range(2)], 1),
        atom_b=a("atom_bias").T.copy(),
        molT=a16("mol_lin_w").T.copy(), mol_lin=a("mol_lin_w").copy(),
        matt_src_rep=rep16(inputs["mol_att_src"]).reshape(128, 1, H),
        matt_dst=a("mol_att_dst").reshape(H, 1),
        mol_b=a("mol_bias").reshape(H, 1),
        gruW=gw, gruN=gn, gbx_rz=gbx_rz, gbh_rz=gbh_rz, gbx_n=gbx_n, gbh_n=gbh_n,
        lin2T=a("lin2_w").T.copy(), lin2_b=a("lin2_b").reshape(H, 1),
        lng_rep=rep32(inputs["ln_g"]), lnb_rep=rep32(inputs["ln_b"]),
        h1T=a("h1_w").T.copy(), h1_b=a("h1_b").reshape(H, 1),
        h2T=a("h2_w").T.copy(), h2_b=a("h2_b").reshape(H, 1),
        h3T=a("h3_w").T.copy(), h3_b=a("h3_b").reshape(1, 1),
    )
    in_maps = []
    for c in range(NCORE):
        pc = plan["cores"][c]
        rc = plan["rcores"][c]
        xT = np.zeros((H, NPAD), np.float32)
        xT[:, 0:NPC] = x[c * NPC:(c + 1) * NPC][plan["perms"][c]].T
        eaT = np.zeros((16, plan["S"]), np.float16)
        val = pc["esel"] >= 0
        eaT[:, val] = edge_attr[pc["esel"][val]].T.astype(np.float16)
        im = dict(xT_in=xT, gidx=pc["gidx"], par=pc["par"], msk=pc["msk"], eaT=eaT,
                  rgidx=rc["rgidx"], rpar=rc["rpar"], rmsk=rc["rmsk"],
                  rmsk16=rc["rmsk16"], **wts)
        in_maps.append(im)
    res = run_bass_kernel_spmd(nc, in_maps, core_ids=list(range(NCORE)))
    if getattr(res, "exec_time_ns", None):
        print(f"HW exec time: {res.exec_time_ns} ns", flush=True)
    y = np.zeros(B, np.float32)
    for c in range(NCORE):
        ys = res.results[c]["y_out"].reshape(GPAD)[0:GPC]
        y[c * GPC + plan["gperms"][c]] = ys
    return y.reshape(B, 1)
